# revision 2
# baseline (speedup 1.0000x reference)
"""Causal self-attention (B=2, T=2048, C=1024, H=16, D=64) with RoPE on TRN2.

Sharding: 8 cores = 2 (batch) x 4 (head-groups of 4 heads), no
inter-core communication: each core computes qkv + rope + causal
attention + a row-parallel o_proj partial for its 4 heads; the host
sums the 4 fp16 partials per batch.

Precision: x and the q/k/v weights ship as fp8-e4m3 (hi, residual)
pairs, pre-scaled x16 / x64 so the residuals stay out of the e4m3
subnormal range; 3 DoubleRow accumulation chains (x8 w8 + x8 dw8 +
dx8 w8) reproduce the fp16 projection to ~0.2% at 3/8 the PE cost
(DoubleRow contracts 256 dims at 0.5 cycles/col).  The 2^-10 shipping
scale is undone for free in the rope tables (q/k) and the v psum
bounce.  Roped q/k are quantized to fp8 on the fly -- the rope sub/add
writes fp8 tiles directly in a DoubleRow-ready layout (head h in
partitions [32h,32h+32), re dims cols [0,T), im cols [T,2T)) -- so
QK^T runs as one fp8-DR matmul per (head, k-block) at 0.5 cycles/col:
2x the fp16 rate with only a single e4m3 quantization of q/k (rel err
1.65e-2 vs the 2e-2 gate).  P and V stay fp16 (peaked attention rows
pass V errors straight through, so V cannot be fp8), o_proj fp16.

Attention computes S^T = (K Q^T) per 128-row k-block so probabilities
exit exp() already transposed for P^T @ V.  V tiles carry 64 all-ones
columns: the PV matmul then yields psum rows 0-63 = y, rows 64-127 =
the softmax denominator replicated 64x (zero extra PE cycles since
matmul cost is free-dim only) -- normalization is one DVE reciprocal +
multiply per psum bank, emitted as soon as that bank closes.  No max
subtraction: logits are O(+-8) and exp fits fp16 range.

Scheduling: emission order is per-engine queue order.  exp on ACT
(0.833 ns/col) now outweighs the fp8-DR scores + fp16 PV (0.625
ns/col), so attention is ACT-paced and all projection/o_proj work is
pumped into the attention stream as ~0.4us PE filler units
(chain-quarters of the DR accumulations).  Heads are emitted with a
one-unit lookahead (next head's first scores before this head's last
pv) so the exp stream does not drain at head boundaries.  Engine
assignment is phase-aware: rope/v psum bounces ride on ACT (which has
slack outside tile C) and the o_proj bounces on DVE; the tail
alternates so two DMA queues drain the output in parallel.
"""

import sys
import os

sys.path.insert(0, "/opt/trn_rl_repo")

import numpy as np
from contextlib import ExitStack

import concourse.bass as bass
import concourse.bacc as bacc
import concourse.mybir as mybir
import concourse.tile as tile

F32 = mybir.dt.float32
F16 = mybir.dt.float16
F8 = mybir.dt.float8e4
DR = mybir.MatmulPerfMode.DoubleRow

# problem constants (hardcoded per contract)
B, T, C, NH, D = 2, 2048, 1024, 16, 64
# schedule knobs (env-overridable for tuning sweeps)
PUMP_A = int(os.environ.get("K_PUMP_A", "2"))
PUMP_B = int(os.environ.get("K_PUMP_B", "2"))
PUMP_C0 = int(os.environ.get("K_PUMP_C0", "5"))
PUMP_C = int(os.environ.get("K_PUMP_C", "4"))
KEEP_WARM = int(os.environ.get("K_KEEP_WARM", "0"))
WARM_N = int(os.environ.get("K_WARM_N", "8"))
PAIR = int(os.environ.get("K_PAIR", "0"))
FASTK = int(os.environ.get("K_FASTK", "0"))
AB23 = int(os.environ.get("K_AB23", "1"))
HL = 4            # local heads per core
NCORE = 8
CH = 512          # qkv T-chunk width
NCHUNK = T // CH  # 4
SCALE = 1.0 / 8.0  # 1/sqrt(D)
NKB = T // 128    # 16 k-blocks


def _splits(a, b):
    """Split [a, b) at 512 boundaries (psum bank = 512 f32)."""
    out = []
    while a < b:
        nxt = min(b, (a // 512 + 1) * 512)
        out.append((a, nxt))
        a = nxt
    return out


def build_nc():
    nc = bacc.Bacc("TRN2", debug=False, num_devices=NCORE)

    # DR-packed fp8 operands: "p (i j c)" layout pairs contraction dims
    # 256j+128i+p so one DoubleRow matmul contracts 256 dims in 0.5
    # cycles/col.  x and the q/k/v weights ship as (hi, residual) fp8
    # pairs; 3 accumulation chains (x8 w8 + x8 dw8 + dx8 w8) reproduce
    # the fp16 product to ~0.2% at 3/8 the PE cost.
    x8_d = nc.dram_tensor("x8", [128, 8 * T], F8, kind="ExternalInput").ap()
    dx8_d = nc.dram_tensor("dx8", [128, 8 * T], F8, kind="ExternalInput").ap()
    wqk8_d = nc.dram_tensor("wqk8", [128, 4096], F8, kind="ExternalInput").ap()
    dwqk8_d = nc.dram_tensor("dwqk8", [128, 4096], F8,
                             kind="ExternalInput").ap()
    wv8_d = nc.dram_tensor("wv8", [128, 2048], F8, kind="ExternalInput").ap()
    dwv8_d = nc.dram_tensor("dwv8", [128, 2048], F8,
                            kind="ExternalInput").ap()
    wo_d = nc.dram_tensor("wo", [256, C], F16, kind="ExternalInput").ap()
    ccT_d = nc.dram_tensor("ccT", [128, T], F16, kind="ExternalInput").ap()
    ssT_d = nc.dram_tensor("ssT", [128, T], F16, kind="ExternalInput").ap()
    outT_d = nc.dram_tensor("outT", [C, T], F16, kind="ExternalOutput").ap()

    with tile.TileContext(nc) as tc, ExitStack() as ctx:
        const = ctx.enter_context(tc.tile_pool(name="const", bufs=1))
        rtp = ctx.enter_context(tc.tile_pool(name="rtp", bufs=2))
        pp = ctx.enter_context(tc.tile_pool(name="pp", bufs=5))
        nrm = ctx.enter_context(tc.tile_pool(name="nrm", bufs=3))
        obp = ctx.enter_context(tc.tile_pool(name="obp", bufs=4))
        psum = ctx.enter_context(tc.tile_pool(name="psum", bufs=2, space="PSUM"))

        # ---- persistent SBUF tensors ----
        # q/k weights (hi + residual) land first so the first real
        # matmuls can start while the rest is still in flight.
        wqk8 = const.tile([128, 4096], F8, tag="wqk8", name="wqk8")
        dwqk8 = const.tile([128, 4096], F8, tag="dwqk8", name="dwqk8")
        nc.scalar.dma_start(out=wqk8[:], in_=wqk8_d[:])
        nc.scalar.dma_start(out=dwqk8[:], in_=dwqk8_d[:])
        # views "p i (j m)": i-halves of the 256-dim contraction pairs
        wqk8_v = wqk8[:].rearrange("p (i jm) -> p i jm", i=2)
        dwqk8_v = dwqk8[:].rearrange("p (i jm) -> p i jm", i=2)

        # remaining input DMAs spread across issue queues so nothing
        # serializes behind the wqk/x loads
        cc = const.tile([128, T], F16, tag="cc")
        ss = const.tile([128, T], F16, tag="ss")
        nc.scalar.dma_start(out=cc[:, 0:1024], in_=ccT_d[:, 0:1024])
        nc.scalar.dma_start(out=ss[:, 0:1024], in_=ssT_d[:, 0:1024])

        wv8 = const.tile([128, 2048], F8, tag="wv8", name="wv8")
        dwv8 = const.tile([128, 2048], F8, tag="dwv8", name="dwv8")
        nc.scalar.dma_start(out=wv8[:], in_=wv8_d[:])
        nc.scalar.dma_start(out=dwv8[:], in_=dwv8_d[:])
        wv8_v = wv8[:].rearrange("p (i jd) -> p i jd", i=2)
        dwv8_v = dwv8[:].rearrange("p (i jd) -> p i jd", i=2)
        # back halves of the rope tables are only needed by chunks 2/3
        nc.scalar.dma_start(out=cc[:, 1024:2048], in_=ccT_d[:, 1024:2048])
        nc.scalar.dma_start(out=ss[:, 1024:2048], in_=ssT_d[:, 1024:2048])

        # PE warm-up: dependency-free matmuls that cover the input-DMA wait
        # and carry the tensor engine through its p-state ramp before the
        # first real matmul issues.
        warm = const.tile([128, 512], F16, tag="warm")
        nc.gpsimd.memset(warm[:], 0.0)
        pw = psum.tile([128, 512], F32, tag="qk", name="pw")
        for _ in range(WARM_N):
            nc.tensor.matmul(
                pw[:, 0:512], lhsT=warm[:, 0:128], rhs=warm[:],
                start=True, stop=True)

        wo_all = const.tile([128, 2 * C], F16, tag="wo", name="wo")
        nc.sync.dma_start(
            out=wo_all[:].rearrange("p (kt n) -> p kt n", n=C),
            in_=wo_d.rearrange("(kt p) n -> p kt n", p=128))
        wo_sb = [wo_all[:, kb * C:(kb + 1) * C] for kb in range(2)]

        # q8/k8: fp8e4 tiles [128, 2T].  head h lives in partitions
        # [32h, 32h+32); re dims at cols [0,T), im dims at cols [T,2T).
        # This matches the wqk column order (re of h0..h3, im of h0..h3),
        # so rope's sub/add write them directly -- no relayout copies --
        # and QK^T runs as one fp8 DoubleRow matmul per (head, k-block):
        # lhsT=[32,2,128] k-view, rhs=[32,2,n] q-view contract 64 dims.
        q8 = const.tile([128, 2 * T], F8, tag="q8", name="q8")
        k8 = const.tile([128, 2 * T], F8, tag="k8", name="k8")
        q8v = q8[:].rearrange("p (i t) -> p i t", i=2)
        k8v = k8[:].rearrange("p (i t) -> p i t", i=2)
        # v: [128 kpos, 4 heads x 16 blocks x 128] fp16; cols 0-63 of each
        # block = v dims, cols 64-127 = ones (denominator rows of PV psum)
        vT = const.tile([128, HL * NKB * 128], F16, tag="vT", name="vT")
        vT_v = vT[:].rearrange("p (h b c) -> p h b c", h=HL, b=NKB)
        nc.gpsimd.memset(vT_v[:, :, :, 64:128], 1.0)
        # y^T tiles: [128, T] x2 (4 heads x 64 dims)
        yT = [const.tile([128, T], F16, tag=f"yT{kb}", name=f"yT{kb}")
              for kb in range(2)]

        # x (hi + residual) fp8, persistent DR layout "p (i j t)"
        x8 = const.tile([128, 8 * T], F8, tag="x8", name="x8")
        dx8 = const.tile([128, 8 * T], F8, tag="dx8", name="dx8")
        x8_v = x8[:].rearrange("p (i jt) -> p i jt", i=2)
        dx8_v = dx8[:].rearrange("p (i jt) -> p i jt", i=2)
        x8_4v = x8[:].rearrange("p (i j t) -> p i j t", i=2, j=4)
        dx8_4v = dx8[:].rearrange("p (i j t) -> p i j t", i=2, j=4)
        x8_d4v = x8_d.rearrange("p (i j t) -> p i j t", i=2, j=4)
        dx8_d4v = dx8_d.rearrange("p (i j t) -> p i j t", i=2, j=4)

        def x_load(n, ways=2):
            t0 = n * CH
            nc.sync.dma_start(out=x8_4v[:, :, :, t0:t0 + CH],
                              in_=x8_d4v[:, :, :, t0:t0 + CH])
            nc.sync.dma_start(out=dx8_4v[:, :, :, t0:t0 + CH],
                              in_=dx8_d4v[:, :, :, t0:t0 + CH])

        CHAINS_QK = ((x8_v, wqk8_v), (x8_v, dwqk8_v), (dx8_v, wqk8_v))
        CHAINS_V = ((x8_v, wv8_v), (x8_v, dwv8_v), (dx8_v, wv8_v))

        def qk_units(n, g, act_bounce=False, splits=((0, CH),)):
            """q (g=0) / k (g=1) projection + rope for chunk n as a list
            of ~0.4us PE filler units (chain-quarters of the pre/pim psum
            accumulations; rope rides on the last one).  `splits` breaks
            the chunk into column ranges emitted as separate unit groups
            (used to fast-path the first k-block before the first exp)."""
            t0 = n * CH

            def mk(a, b):
                st = {}

                def mm(name, m, ci):
                    def u():
                        if ci == 0:
                            st[name] = psum.tile([128, CH], F32, tag="qk",
                                                 name=name)
                        xs, ws = CHAINS_QK[ci]
                        for j in range(4):
                            nc.tensor.matmul(
                                st[name][:, a:b],
                                lhsT=ws[:, :, j * 512 + m * 128:
                                        j * 512 + (m + 1) * 128],
                                rhs=xs[:, :, j * T + t0 + a:j * T + t0 + b],
                                start=(ci == 0 and j == 0),
                                stop=(ci == 2 and j == 3),
                                perf_mode=DR,
                            )
                    return u

                def rope():
                    mul = mybir.AluOpType.mult
                    sub = mybir.AluOpType.subtract
                    add = mybir.AluOpType.add
                    pre, pim = st["psre"], st["psim"]
                    w = b - a
                    ccn = cc[:, t0 + a:t0 + b]
                    ssn = ss[:, t0 + a:t0 + b]
                    t1 = rtp.tile([128, CH], F16, tag="t1")
                    t2 = rtp.tile([128, CH], F16, tag="t2")
                    t3 = rtp.tile([128, CH], F16, tag="t3")
                    t4 = rtp.tile([128, CH], F16, tag="t4")
                    # psum -> fp16 bounce: ACT while it still has slack
                    # (early chunks), DVE once exp saturates ACT; the
                    # rope arithmetic runs in DVE 4x fp16 mode.
                    preb = rtp.tile([128, CH], F16, tag="preb")
                    pimb = rtp.tile([128, CH], F16, tag="pimb")
                    if act_bounce:
                        nc.scalar.copy(preb[:, 0:w], pre[:, a:b])
                        nc.scalar.copy(pimb[:, 0:w], pim[:, a:b])
                    else:
                        nc.vector.tensor_copy(preb[:, 0:w], pre[:, a:b])
                        nc.vector.tensor_copy(pimb[:, 0:w], pim[:, a:b])
                    preb_, pimb_ = preb[:, 0:w], pimb[:, 0:w]
                    nc.vector.tensor_tensor(t1[:, 0:w], preb_, ccn, mul)
                    nc.vector.tensor_tensor(t2[:, 0:w], pimb_, ssn, mul)
                    nc.vector.tensor_tensor(t3[:, 0:w], preb_, ssn, mul)
                    nc.vector.tensor_tensor(t4[:, 0:w], pimb_, ccn, mul)
                    # sub/add write the fp8 q8/k8 tiles directly (re
                    # half / im half): no relayout copies needed.
                    o = q8 if g == 0 else k8
                    nc.vector.tensor_tensor(
                        o[:, t0 + a:t0 + b], t1[:, 0:w], t2[:, 0:w], sub)
                    nc.vector.tensor_tensor(
                        o[:, T + t0 + a:T + t0 + b], t3[:, 0:w],
                        t4[:, 0:w], add)

                last = mm("psim", 2 * g + 1, 2)
                return [mm("psre", 2 * g, 0), mm("psre", 2 * g, 1),
                        mm("psre", 2 * g, 2), mm("psim", 2 * g + 1, 0),
                        mm("psim", 2 * g + 1, 1),
                        lambda: (last(), rope())]

            out = []
            for (a, b) in splits:
                out.extend(mk(a, b))
            return out

        def qk_g(n, g, act_bounce=False, splits=((0, CH),)):
            for u in qk_units(n, g, act_bounce, splits):
                u()

        def v_units(n, tb, act_bounce=False):
            """v projection for 128-row block tb of chunk n: 3 chain
            units of ~0.2us; the psum->fp16 bounce rides on the last."""
            t0 = n * CH + tb * 128
            st = {}

            def mm(ci):
                def u():
                    if ci == 0:
                        st["psv"] = psum.tile([128, CH], F32, tag="qk",
                                              name="psv")
                    xs, ws = CHAINS_V[ci]
                    for j in range(4):
                        nc.tensor.matmul(
                            st["psv"][:, 0:256],
                            lhsT=xs[:, :, j * T + t0:j * T + t0 + 128],
                            rhs=ws[:, :, j * 256:(j + 1) * 256],
                            start=(ci == 0 and j == 0),
                            stop=(ci == 2 and j == 3),
                            perf_mode=DR,
                        )
                    if ci == 2:
                        blk = 4 * n + tb
                        dst = vT_v[:, :, blk, 0:64]
                        src = st["psv"][:, 0:256].rearrange(
                            "p (h d) -> p h d", d=64)
                        # gpsimd cannot read PSUM; the bounce (psum holds
                        # 1024*v -- x/w ship pre-scaled x16/x64 to keep
                        # fp8 residuals out of the e4m3 subnormal range)
                        # folds in the 2^-10 fix free.  ACT takes it in
                        # the A/B phases where exp leaves it slack.
                        if act_bounce:
                            nc.scalar.activation(
                                dst, src,
                                mybir.ActivationFunctionType.Copy,
                                scale=1.0 / 1024.0)
                        else:
                            nc.vector.tensor_scalar_mul(
                                dst, src, 1.0 / 1024.0)
                return u

            return [mm(0), mm(1), mm(2)]

        def v_tb(n, tb):
            for u in v_units(n, tb):
                u()

        filler_q = []

        def dummy(n=2):
            # keep-warm matmuls: PE p-state drops 2x after an idle gap and
            # needs 3us of continuous execution to recover; padding known
            # exp-bound stretches keeps the real matmuls at full clock.
            pd = psum.tile([128, 512], F32, tag="qk", name="pd")
            for _ in range(n):
                nc.tensor.matmul(
                    pd[:, 0:512], lhsT=warm[:, 0:128], rhs=warm[:],
                    start=True, stop=True)

        def pump(keep_warm=0):
            if filler_q:
                filler_q.pop(0)()
            elif keep_warm:
                dummy(keep_warm)

        def attention_units(h, q0, qn, pump_every=0, at_blocks=None,
                            keep_warm=0, pair_full=False, norm_act=False):
            """One head, q-cols [q0, q0+qn) as a list of emission units.

            units[kb] = at_blocks/pump + scores(kb) + pv(kb-1); the last
            unit is pv(nkb-1) + final normalize.  The scheduler emits the
            NEXT head's units[0] just before this head's last unit: that
            head-boundary scores runs on PE while the last exp still
            streams on ACT, so ACT never drains between heads.  (The pst
            double-buffer is free at exactly that point: exp(nkb-2) has
            been consumed by the preceding pv.)

            at_blocks: {kb: [unit, ...]} -- mandatory work units emitted
            just before scores_block(kb); used for dependencies of later
            pv_blocks (e.g. v tiles), unlike best-effort pump fillers.
            """
            qv = q8v[32 * h:32 * h + 32]
            kv = k8v[32 * h:32 * h + 32]
            r0 = 64 * (h % 2)
            nkb = (q0 + qn) // 128
            fd = q0 // 128  # first diagonal block
            st_ = {}
            Ps = {}

            def block_off(kb):
                return 128 * (kb - fd) if kb >= fd else 0

            def scores_block(kb):
                if kb == 0:
                    st_["psy"] = psum.tile([128, qn], F32, tag="y",
                                           name="psy", bufs=1)
                off = block_off(kb)
                pst = psum.tile([128, qn], F32, tag="st", name="pst")
                for (a, b) in _splits(off, qn):
                    nc.tensor.matmul(
                        pst[:, a:b],
                        lhsT=kv[:, :, kb * 128:(kb + 1) * 128],
                        rhs=qv[:, :, q0 + a:q0 + b],
                        start=True,
                        stop=True,
                        perf_mode=DR,
                        tile_position=(32 * h, 0),
                    )
                P = pp.tile([128, 1024], F16, tag="P")
                Ps[kb] = (P, 0)
                nc.scalar.activation(
                    P[:, off:qn], pst[:, off:qn],
                    mybir.ActivationFunctionType.Exp, scale=SCALE)
                if kb >= fd:
                    # zero strictly-upper triangle of the leading 128 cols
                    nc.gpsimd.affine_select(
                        out=P[:, off:off + 128],
                        in_=P[:, off:off + 128],
                        compare_op=mybir.AluOpType.is_ge,
                        fill=0.0,
                        base=0,
                        pattern=[[1, 128]],
                        channel_multiplier=-1,
                    )

            def scores_pair(kb):
                # two qn=512 k-blocks share one [128,1024] pst / P pair:
                # a single exp call covers both, halving the ACT per-call
                # access-latency overhead and doubling the exp lookahead
                # the pst double-buffer can hold.  For diagonal blocks the
                # gap columns [512, 512+off1) are dead: matmul start
                # zeroes the bank region, exp of them is masked/unread
                # (pv reads only [off:512] of each half).
                if kb == 0:
                    st_["psy"] = psum.tile([128, qn], F32, tag="y",
                                           name="psy", bufs=1)
                pst = psum.tile([128, 1024], F32, tag="st", name="pst")
                for sub in (0, 1):
                    off = block_off(kb + sub)
                    nc.tensor.matmul(
                        pst[:, 512 * sub + off:512 * sub + 512],
                        lhsT=kv[:, :, (kb + sub) * 128:(kb + sub + 1) * 128],
                        rhs=qv[:, :, q0 + off:q0 + 512],
                        start=True,
                        stop=True,
                        perf_mode=DR,
                        tile_position=(32 * h, 0),
                    )
                off0 = block_off(kb)
                P = pp.tile([128, 1024], F16, tag="P")
                Ps[kb] = (P, 0)
                Ps[kb + 1] = (P, 512)
                nc.scalar.activation(
                    P[:, off0:1024], pst[:, off0:1024],
                    mybir.ActivationFunctionType.Exp, scale=SCALE)
                for sub in (0, 1):
                    off = block_off(kb + sub)
                    if kb + sub >= fd:
                        nc.gpsimd.affine_select(
                            out=P[:, 512 * sub + off:512 * sub + off + 128],
                            in_=P[:, 512 * sub + off:512 * sub + off + 128],
                            compare_op=mybir.AluOpType.is_ge,
                            fill=0.0,
                            base=0,
                            pattern=[[1, 128]],
                            channel_multiplier=-1,
                        )

            def pv_block(kb):
                off = block_off(kb)
                P, coff = Ps.pop(kb)
                psy = st_["psy"]
                for (a, b) in _splits(off, qn):
                    # last writer of the psum bank holding col a is diag
                    # block fd + 4*(a//512) + 3
                    kb_stop = min(fd + 4 * (a // 512) + 3, nkb - 1)
                    nc.tensor.matmul(
                        psy[:, a:b],
                        lhsT=vT_v[:, h, kb, :],
                        rhs=P[:, coff + a:coff + b],
                        start=(kb == 0),
                        stop=(kb == kb_stop),
                    )

            def normalize(a, b):
                # psum rows 64-127 all hold the denominator row l (ones
                # cols of vT): reciprocal + one multiply per psum bank,
                # emitted as soon as that bank's accumulation closes.
                # (walrus forbids two PSUM reads in one TensorTensor, so
                # a single divide is not possible.)
                psy = st_["psy"]
                rlb = nrm.tile([64, 512], F32, tag="rlb")
                nc.vector.reciprocal(rlb[:, 0:b - a], psy[64:128, a:b])
                nc.vector.tensor_tensor(
                    yT[h // 2][r0:r0 + 64, q0 + a:q0 + b],
                    psy[0:64, a:b], rlb[:, 0:b - a], mybir.AluOpType.mult)

            def mk_unit(kb, scores_fn, pvs):
                def u():
                    if at_blocks and kb in at_blocks:
                        for ab in at_blocks[kb]:
                            ab()
                    if pump_every and kb % pump_every == 0:
                        pump(keep_warm)
                    if scores_fn:
                        scores_fn(kb)
                    for pkb in pvs:
                        pv_block(pkb)
                        if pkb == min(fd + 3, nkb - 1) and qn > 512:
                            normalize(0, 512)  # bank 0 closed early
                return u

            units = []
            if PAIR and pair_full and qn == 512 and nkb % 2 == 0:
                # all blocks in exp-pairs; pv's trail one scores event
                pend = []
                for kb in range(0, nkb, 2):
                    units.append(mk_unit(kb, scores_pair, pend))
                    pend = [kb, kb + 1]
                st_["pend"] = pend
            else:
                for kb in range(nkb):
                    units.append(mk_unit(
                        kb, scores_block, [kb - 1] if kb > 0 else []))
                st_["pend"] = [nkb - 1]

            def final():
                for pkb in st_["pend"]:
                    pv_block(pkb)
                normalize(512 if qn > 512 else 0, qn)

            return units + [final]

        def run_heads(seq):
            """Emit head unit-lists with one-unit cross-head lookahead:
            the next head's scores(0) goes out before this head's final
            pv, so the exp stream never drains at a head boundary."""
            for i, units in enumerate(seq):
                for u in units[1 if i else 0:-1]:
                    u()
                if i + 1 < len(seq):
                    seq[i + 1][0]()
                units[-1]()

        def o_proj(nt, mo, tail=False):
            """Output block: feat rows [128*mo ..+128), q [512*nt ..+512)."""
            ob = obp.tile([128, 512], F16, tag="ob", name="ob")
            ps = psum.tile([128, CH], F32, tag="qk", name="psob")
            for kb in range(2):
                nc.tensor.matmul(
                    ps[:, 0:512],
                    lhsT=wo_sb[kb][:, mo * 128:(mo + 1) * 128],
                    rhs=yT[kb][:, nt * 512:(nt + 1) * 512],
                    start=(kb == 0),
                    stop=(kb == 1),
                )
            # in the tail ACT is idle once the exps are done: it takes half
            # the psum bounces there
            if tail and mo % 2 == 1:
                nc.scalar.copy(ob[:], ps[:, 0:512])
            else:
                nc.vector.tensor_copy(ob[:], ps[:, 0:512])
            # keep DMA issue off the ACT queue while exps run; in the tail
            # ACT is free and a second queue doubles drain bandwidth
            ring = nc.scalar if (tail and mo % 2 == 1) else nc.sync
            ring.dma_start(
                out=outT_d[mo * 128:(mo + 1) * 128, nt * 512:(nt + 1) * 512],
                in_=ob[:])

        # ---- schedule ----
        # Emission order == per-engine queue order.  ACT (exp) is the
        # attention pacer now that scores+PV run fp8-DR/fp16 (PE 0.625
        # vs ACT 0.833 ns per score column), so all qkv/o_proj work is
        # pumped into the attention stream as ~0.4us filler units.
        # Dependency safety comes from emission order: a filler is
        # always emitted before the instruction that needs it.
        x_load(0)
        x_load(1)
        qk_g(0, 0, act_bounce=True)
        # k chunk 0 split: cols 0-256 (k-blocks 0/1) go out first so the
        # first exp fires ~2.5us earlier; the rest rides as at_blocks of
        # head 0 before scores(2) needs it.
        if FASTK:
            k0u = qk_units(0, 1, act_bounce=True,
                           splits=((0, 256), (256, CH)))
            for u in k0u[:6]:
                u()
            k0rest = k0u[6:]
        else:
            qk_g(0, 1, act_bounce=True)
            k0rest = []
        # tile A (q 0-512): needs only chunk 0 q/k; v blocks land as
        # at_blocks just before their pv consumer.  The chunk-1 qk and v
        # work soaks up the rope-latency wait before the first scores.
        filler_q.extend(qk_units(1, 0, act_bounce=True))
        filler_q.extend(qk_units(1, 1, act_bounce=True))
        for tb in range(4):
            filler_q.extend(v_units(1, tb, act_bounce=bool(AB23)))
        run_heads([
            attention_units(0, 0, 512, pump_every=PUMP_A, norm_act=True, at_blocks={
                1: k0rest + v_units(0, 0, act_bounce=bool(AB23)),
                2: v_units(0, 1, act_bounce=bool(AB23)),
                3: v_units(0, 2, act_bounce=bool(AB23))
                   + v_units(0, 3, act_bounce=bool(AB23)),
            }),
            attention_units(1, 0, 512, pump_every=PUMP_A, norm_act=True, pair_full=True),
            attention_units(2, 0, 512, pump_every=PUMP_A, norm_act=True, pair_full=True),
            attention_units(3, 0, 512, pump_every=PUMP_A, norm_act=True, pair_full=True),
        ])
        while filler_q:
            pump()
        # tile B (q 512-1024): fillers: chunks 2,3 qk.
        x_load(2)
        filler_q.extend(qk_units(2, 0, act_bounce=bool(AB23)))
        filler_q.extend(qk_units(2, 1, act_bounce=bool(AB23)))
        filler_q.append(lambda: x_load(3))
        filler_q.extend(qk_units(3, 0, act_bounce=bool(AB23)))
        filler_q.extend(qk_units(3, 1, act_bounce=bool(AB23)))
        run_heads([
            attention_units(0, 512, 512, pump_every=PUMP_B, pair_full=True, norm_act=True),
            attention_units(1, 512, 512, pump_every=PUMP_B, pair_full=True, norm_act=True),
            attention_units(2, 512, 512, pump_every=PUMP_B, pair_full=True, norm_act=True),
            attention_units(3, 512, 512, pump_every=PUMP_B, pair_full=True, norm_act=True),
        ])
        while filler_q:
            pump()
        # tail o_proj helpers (q 1024-2048): per mo one [128, 1024] ob
        # filled in two halves; nt=2 halves are emitted inside the last
        # head's attention as soon as its early psum bank is normalized.
        tail_obs = {}

        def tail_half(nt, mo):
            if mo not in tail_obs:
                tail_obs[mo] = obp.tile([128, 1024], F16, tag="obt",
                                        name="obt", bufs=8)
            ob = tail_obs[mo]
            ps = psum.tile([128, CH], F32, tag="qk", name="psob")
            for kb in range(2):
                nc.tensor.matmul(
                    ps[:, 0:512],
                    lhsT=wo_sb[kb][:, mo * 128:(mo + 1) * 128],
                    rhs=yT[kb][:, nt * 512:(nt + 1) * 512],
                    start=(kb == 0),
                    stop=(kb == 1),
                )
            half = nt - 2
            if nt == 3 and mo % 2 == 1:
                # true tail: exps are done, ACT is free
                nc.scalar.copy(ob[:, half * 512:(half + 1) * 512],
                               ps[:, 0:512])
            else:
                nc.vector.tensor_copy(
                    ob[:, half * 512:(half + 1) * 512], ps[:, 0:512])
            # flush each 512-half as soon as its copy lands; the tail
            # copies alternate ACT/DVE so two DMA queues drain in parallel
            ring = nc.scalar if (nt == 3 and mo % 2 == 1) else nc.sync
            ring.dma_start(
                out=outT_d[mo * 128:(mo + 1) * 128,
                           nt * 512:(nt + 1) * 512],
                in_=ob[:, half * 512:(half + 1) * 512])

        # tile C (q 1024-2048): v chunks 2/3 are emitted at fixed blocks of
        # the first head (hard deps of pv blocks 8-15); o_proj of q 0-1024
        # is order-free filler spread across all four heads.
        filler_q.extend(
            (lambda nt=nt, mo=mo: o_proj(nt, mo))
            for nt in range(2) for mo in range(8))
        run_heads([
            attention_units(0, 1024, 1024, pump_every=PUMP_C0, at_blocks={
                5: v_units(2, 0) + v_units(2, 1),
                7: v_units(2, 2) + v_units(2, 3),
                9: v_units(3, 0) + v_units(3, 1),
                11: v_units(3, 2) + v_units(3, 3),
            }),
            attention_units(1, 1024, 1024, pump_every=PUMP_C,
                            keep_warm=KEEP_WARM),
            attention_units(2, 1024, 1024, pump_every=PUMP_C,
                            keep_warm=KEEP_WARM),
            attention_units(3, 1024, 1024, pump_every=PUMP_C,
                            keep_warm=KEEP_WARM, at_blocks={
                13: [lambda mo=mo: tail_half(2, mo) for mo in range(4)],
                15: [lambda mo=mo: tail_half(2, mo) for mo in range(4, 8)],
            }),
        ])
        while filler_q:
            pump()
        for mo in range(8):
            tail_half(3, mo)

    nc.compile()
    return nc


def _dr_pack(a, scale):
    """[1024, M] f32 -> fp8 (hi, residual) pair in DR layout [128, 8M].

    DR layout "p (i j m)": element (p, i, j, m) = a[256j + 128i + p, m], so
    one DoubleRow matmul contracts dim pairs (256j+p, 256j+128+p).
    `scale` lifts the values so both them and their residuals quantize in
    the e4m3 normal range; the device compensates (rope tables / v copy).
    """
    import ml_dtypes
    f8 = ml_dtypes.float8_e4m3
    a = a * scale
    M = a.shape[1]
    hi = a.astype(f8)
    lo = (a - hi.astype(np.float32)).astype(f8)
    out = []
    for t in (hi, lo):
        t = t.reshape(4, 2, 128, M).transpose(2, 1, 0, 3).reshape(128, 8 * M)
        out.append(np.ascontiguousarray(t))
    return out


def shard_inputs(x, freqs_cos, freqs_sin, Wqkv, Wo):
    """Build the 8 per-core input maps (host-side sharding)."""
    x = np.asarray(x, dtype=np.float32)
    Wqkv = np.asarray(Wqkv, dtype=np.float32)
    Wo = np.asarray(Wo, dtype=np.float32)
    # cos/sin tables transposed and replicated x4 (one copy per local
    # head), pre-divided by 1024 to undo the x16/x64 fp8 shipping scales
    ccT = np.tile(np.asarray(freqs_cos, dtype=np.float32).T, (4, 1)) / 1024.0
    ssT = np.tile(np.asarray(freqs_sin, dtype=np.float32).T, (4, 1)) / 1024.0
    ccT = np.ascontiguousarray(ccT).astype(np.float16)
    ssT = np.ascontiguousarray(ssT).astype(np.float16)
    x8s = [_dr_pack(x[b].T, 16.0) for b in range(B)]

    in_maps = []
    for c in range(NCORE):
        b, hg = c // 4, c % 4
        re = [np.arange(g * 64, g * 64 + 64, 2)
              for g in range(4 * hg, 4 * hg + 4)]
        im = [np.arange(g * 64 + 1, g * 64 + 64, 2)
              for g in range(4 * hg, 4 * hg + 4)]
        qcols = np.concatenate(re + im)
        kcols = C + qcols
        wqk8, dwqk8 = _dr_pack(Wqkv[:, np.concatenate([qcols, kcols])], 64.0)
        wv8, dwv8 = _dr_pack(
            Wqkv[:, 2 * C + hg * 256: 2 * C + hg * 256 + 256], 64.0)
        wo = np.ascontiguousarray(
            Wo[hg * 256: hg * 256 + 256, :]).astype(np.float16)
        in_maps.append({
            "x8": x8s[b][0], "dx8": x8s[b][1],
            "wqk8": wqk8, "dwqk8": dwqk8, "wv8": wv8, "dwv8": dwv8,
            "wo": wo, "ccT": ccT, "ssT": ssT,
        })
    return in_maps


_NC_CACHE = None


def _get_nc():
    global _NC_CACHE
    if _NC_CACHE is None:
        _NC_CACHE = build_nc()
    return _NC_CACHE


def run(inputs, trace=False):
    from concourse.bass_utils import run_bass_kernel_spmd

    nc = _get_nc()
    in_maps = shard_inputs(**inputs)
    res = run_bass_kernel_spmd(nc, in_maps, list(range(NCORE)), trace=trace)
    out = np.empty((B, T, C), dtype=np.float32)
    for b in range(B):
        acc = res.results[4 * b]["outT"].astype(np.float32)
        for c in range(4 * b + 1, 4 * b + 4):
            acc = acc + res.results[c]["outT"].astype(np.float32)
        out[b] = acc.T
    return out, res


def kernel(**inputs):
    out, _ = run(inputs)
    return out



# revision 3
# speedup vs baseline: 1.0053x; 1.0053x over previous
"""Causal self-attention (B=2, T=2048, C=1024, H=16, D=64) with RoPE on TRN2.

Sharding: 8 cores = 2 (batch) x 4 (head-groups of 4 heads), no
inter-core communication: each core computes qkv + rope + causal
attention + a row-parallel o_proj partial for its 4 heads; the host
sums the 4 fp16 partials per batch.

Precision: x and the q/k/v weights ship as fp8-e4m3 (hi, residual)
pairs, pre-scaled x16 / x64 so the residuals stay out of the e4m3
subnormal range; 3 DoubleRow accumulation chains (x8 w8 + x8 dw8 +
dx8 w8) reproduce the fp16 projection to ~0.2% at 3/8 the PE cost
(DoubleRow contracts 256 dims at 0.5 cycles/col).  The 2^-10 shipping
scale is undone for free in the rope tables (q/k) and the v psum
bounce.  Roped q/k are quantized to fp8 on the fly -- the rope sub/add
writes fp8 tiles directly in a DoubleRow-ready layout (head h in
partitions [32h,32h+32), re dims cols [0,T), im cols [T,2T)) -- so
QK^T runs as one fp8-DR matmul per (head, k-block) at 0.5 cycles/col:
2x the fp16 rate with only a single e4m3 quantization of q/k (rel err
1.65e-2 vs the 2e-2 gate).  P and V stay fp16 (peaked attention rows
pass V errors straight through, so V cannot be fp8), o_proj fp16.

Attention computes S^T = (K Q^T) per 128-row k-block so probabilities
exit exp() already transposed for P^T @ V.  V tiles carry 64 all-ones
columns: the PV matmul then yields psum rows 0-63 = y, rows 64-127 =
the softmax denominator replicated 64x (zero extra PE cycles since
matmul cost is free-dim only) -- normalization is one DVE reciprocal +
multiply per psum bank, emitted as soon as that bank closes.  No max
subtraction: logits are O(+-8) and exp fits fp16 range.

Scheduling: emission order is per-engine queue order.  exp on ACT
(0.833 ns/col) now outweighs the fp8-DR scores + fp16 PV (0.625
ns/col), so attention is ACT-paced and all projection/o_proj work is
pumped into the attention stream as ~0.4us PE filler units
(chain-quarters of the DR accumulations).  Heads are emitted with a
one-unit lookahead (next head's first scores before this head's last
pv) so the exp stream does not drain at head boundaries; rope/v psum
bounces ride on ACT only where exp leaves it slack.  The o_proj tail
flushes q 1024-2048 as four two-mo strided DMAs (each DMA costs
~625ns on the shared HWDGE regardless of size) and cycles its psum
tiles across the qk/st/y tags -- the attention banks are free by then
-- so the final o_proj chain is not serialized on the 2-buffer psum
rotation.
"""

import sys
import os

sys.path.insert(0, "/opt/trn_rl_repo")

import numpy as np
from contextlib import ExitStack

import concourse.bass as bass
import concourse.bacc as bacc
import concourse.mybir as mybir
import concourse.tile as tile

F32 = mybir.dt.float32
F16 = mybir.dt.float16
F8 = mybir.dt.float8e4
DR = mybir.MatmulPerfMode.DoubleRow

# problem constants (hardcoded per contract)
B, T, C, NH, D = 2, 2048, 1024, 16, 64
# schedule knobs (env-overridable for tuning sweeps)
PUMP_A = int(os.environ.get("K_PUMP_A", "2"))
PUMP_B = int(os.environ.get("K_PUMP_B", "2"))
PUMP_C0 = int(os.environ.get("K_PUMP_C0", "5"))
PUMP_C = int(os.environ.get("K_PUMP_C", "4"))
KEEP_WARM = int(os.environ.get("K_KEEP_WARM", "0"))
WARM_N = int(os.environ.get("K_WARM_N", "8"))
PAIR = int(os.environ.get("K_PAIR", "0"))
FASTK = int(os.environ.get("K_FASTK", "0"))
AB23 = int(os.environ.get("K_AB23", "1"))
SAPOOL = int(os.environ.get("K_SAPOOL", "0"))
KW3 = int(os.environ.get("K_KW3", "0"))
HL = 4            # local heads per core
NCORE = 8
CH = 512          # qkv T-chunk width
NCHUNK = T // CH  # 4
SCALE = 1.0 / 8.0  # 1/sqrt(D)
NKB = T // 128    # 16 k-blocks


def _splits(a, b):
    """Split [a, b) at 512 boundaries (psum bank = 512 f32)."""
    out = []
    while a < b:
        nxt = min(b, (a // 512 + 1) * 512)
        out.append((a, nxt))
        a = nxt
    return out


def build_nc():
    nc = bacc.Bacc("TRN2", debug=False, num_devices=NCORE)

    # DR-packed fp8 operands: "p (i j c)" layout pairs contraction dims
    # 256j+128i+p so one DoubleRow matmul contracts 256 dims in 0.5
    # cycles/col.  x and the q/k/v weights ship as (hi, residual) fp8
    # pairs; 3 accumulation chains (x8 w8 + x8 dw8 + dx8 w8) reproduce
    # the fp16 product to ~0.2% at 3/8 the PE cost.
    x8_d = nc.dram_tensor("x8", [128, 8 * T], F8, kind="ExternalInput").ap()
    dx8_d = nc.dram_tensor("dx8", [128, 8 * T], F8, kind="ExternalInput").ap()
    wqk8_d = nc.dram_tensor("wqk8", [128, 4096], F8, kind="ExternalInput").ap()
    dwqk8_d = nc.dram_tensor("dwqk8", [128, 4096], F8,
                             kind="ExternalInput").ap()
    wv8_d = nc.dram_tensor("wv8", [128, 2048], F8, kind="ExternalInput").ap()
    dwv8_d = nc.dram_tensor("dwv8", [128, 2048], F8,
                            kind="ExternalInput").ap()
    wo_d = nc.dram_tensor("wo", [256, C], F16, kind="ExternalInput").ap()
    ccT_d = nc.dram_tensor("ccT", [128, T], F16, kind="ExternalInput").ap()
    ssT_d = nc.dram_tensor("ssT", [128, T], F16, kind="ExternalInput").ap()
    outT_d = nc.dram_tensor("outT", [C, T], F16, kind="ExternalOutput").ap()

    with tile.TileContext(nc) as tc, ExitStack() as ctx:
        const = ctx.enter_context(tc.tile_pool(name="const", bufs=1))
        rtp = ctx.enter_context(tc.tile_pool(name="rtp", bufs=2))
        pp = ctx.enter_context(tc.tile_pool(name="pp", bufs=5))
        nrm = ctx.enter_context(tc.tile_pool(name="nrm", bufs=3))
        obp = ctx.enter_context(tc.tile_pool(name="obp", bufs=4))
        psum = ctx.enter_context(tc.tile_pool(name="psum", bufs=2, space="PSUM"))

        # ---- persistent SBUF tensors ----
        # q/k weights (hi + residual) land first so the first real
        # matmuls can start while the rest is still in flight.
        wqk8 = const.tile([128, 4096], F8, tag="wqk8", name="wqk8")
        dwqk8 = const.tile([128, 4096], F8, tag="dwqk8", name="dwqk8")
        nc.scalar.dma_start(out=wqk8[:], in_=wqk8_d[:])
        nc.scalar.dma_start(out=dwqk8[:], in_=dwqk8_d[:])
        # views "p i (j m)": i-halves of the 256-dim contraction pairs
        wqk8_v = wqk8[:].rearrange("p (i jm) -> p i jm", i=2)
        dwqk8_v = dwqk8[:].rearrange("p (i jm) -> p i jm", i=2)

        # remaining input DMAs spread across issue queues so nothing
        # serializes behind the wqk/x loads
        cc = const.tile([128, T], F16, tag="cc")
        ss = const.tile([128, T], F16, tag="ss")
        nc.scalar.dma_start(out=cc[:, 0:1024], in_=ccT_d[:, 0:1024])
        nc.scalar.dma_start(out=ss[:, 0:1024], in_=ssT_d[:, 0:1024])

        wv8 = const.tile([128, 2048], F8, tag="wv8", name="wv8")
        dwv8 = const.tile([128, 2048], F8, tag="dwv8", name="dwv8")
        nc.scalar.dma_start(out=wv8[:], in_=wv8_d[:])
        nc.scalar.dma_start(out=dwv8[:], in_=dwv8_d[:])
        wv8_v = wv8[:].rearrange("p (i jd) -> p i jd", i=2)
        dwv8_v = dwv8[:].rearrange("p (i jd) -> p i jd", i=2)
        # back halves of the rope tables are only needed by chunks 2/3
        nc.scalar.dma_start(out=cc[:, 1024:2048], in_=ccT_d[:, 1024:2048])
        nc.scalar.dma_start(out=ss[:, 1024:2048], in_=ssT_d[:, 1024:2048])

        # PE warm-up: dependency-free matmuls that cover the input-DMA wait
        # and carry the tensor engine through its p-state ramp before the
        # first real matmul issues.
        warm = const.tile([128, 512], F16, tag="warm")
        nc.gpsimd.memset(warm[:], 0.0)
        pw = psum.tile([128, 512], F32, tag="qk", name="pw")
        for _ in range(WARM_N):
            nc.tensor.matmul(
                pw[:, 0:512], lhsT=warm[:, 0:128], rhs=warm[:],
                start=True, stop=True)

        wo_all = const.tile([128, 2 * C], F16, tag="wo", name="wo")
        nc.sync.dma_start(
            out=wo_all[:].rearrange("p (kt n) -> p kt n", n=C),
            in_=wo_d.rearrange("(kt p) n -> p kt n", p=128))
        wo_sb = [wo_all[:, kb * C:(kb + 1) * C] for kb in range(2)]

        # q8/k8: fp8e4 tiles [128, 2T].  head h lives in partitions
        # [32h, 32h+32); re dims at cols [0,T), im dims at cols [T,2T).
        # This matches the wqk column order (re of h0..h3, im of h0..h3),
        # so rope's sub/add write them directly -- no relayout copies --
        # and QK^T runs as one fp8 DoubleRow matmul per (head, k-block):
        # lhsT=[32,2,128] k-view, rhs=[32,2,n] q-view contract 64 dims.
        q8 = const.tile([128, 2 * T], F8, tag="q8", name="q8")
        k8 = const.tile([128, 2 * T], F8, tag="k8", name="k8")
        q8v = q8[:].rearrange("p (i t) -> p i t", i=2)
        k8v = k8[:].rearrange("p (i t) -> p i t", i=2)
        # v: [128 kpos, 4 heads x 16 blocks x 128] fp16; cols 0-63 of each
        # block = v dims, cols 64-127 = ones (denominator rows of PV psum)
        vT = const.tile([128, HL * NKB * 128], F16, tag="vT", name="vT")
        vT_v = vT[:].rearrange("p (h b c) -> p h b c", h=HL, b=NKB)
        nc.gpsimd.memset(vT_v[:, :, :, 64:128], 1.0)
        # y^T tiles: [128, T] x2 (4 heads x 64 dims)
        yT = [const.tile([128, T], F16, tag=f"yT{kb}", name=f"yT{kb}")
              for kb in range(2)]

        # x (hi + residual) fp8, persistent DR layout "p (i j t)"
        x8 = const.tile([128, 8 * T], F8, tag="x8", name="x8")
        dx8 = const.tile([128, 8 * T], F8, tag="dx8", name="dx8")
        x8_v = x8[:].rearrange("p (i jt) -> p i jt", i=2)
        dx8_v = dx8[:].rearrange("p (i jt) -> p i jt", i=2)
        x8_4v = x8[:].rearrange("p (i j t) -> p i j t", i=2, j=4)
        dx8_4v = dx8[:].rearrange("p (i j t) -> p i j t", i=2, j=4)
        x8_d4v = x8_d.rearrange("p (i j t) -> p i j t", i=2, j=4)
        dx8_d4v = dx8_d.rearrange("p (i j t) -> p i j t", i=2, j=4)

        def x_load(n, ways=2):
            t0 = n * CH
            nc.sync.dma_start(out=x8_4v[:, :, :, t0:t0 + CH],
                              in_=x8_d4v[:, :, :, t0:t0 + CH])
            nc.sync.dma_start(out=dx8_4v[:, :, :, t0:t0 + CH],
                              in_=dx8_d4v[:, :, :, t0:t0 + CH])

        CHAINS_QK = ((x8_v, wqk8_v), (x8_v, dwqk8_v), (dx8_v, wqk8_v))
        CHAINS_V = ((x8_v, wv8_v), (x8_v, dwv8_v), (dx8_v, wv8_v))

        def qk_units(n, g, act_bounce=False, splits=((0, CH),)):
            """q (g=0) / k (g=1) projection + rope for chunk n as a list
            of ~0.4us PE filler units (chain-quarters of the pre/pim psum
            accumulations; rope rides on the last one).  `splits` breaks
            the chunk into column ranges emitted as separate unit groups
            (used to fast-path the first k-block before the first exp)."""
            t0 = n * CH

            def mk(a, b):
                st = {}

                def mm(name, m, ci):
                    def u():
                        if ci == 0:
                            st[name] = psum.tile([128, CH], F32, tag="qk",
                                                 name=name)
                        xs, ws = CHAINS_QK[ci]
                        for j in range(4):
                            nc.tensor.matmul(
                                st[name][:, a:b],
                                lhsT=ws[:, :, j * 512 + m * 128:
                                        j * 512 + (m + 1) * 128],
                                rhs=xs[:, :, j * T + t0 + a:j * T + t0 + b],
                                start=(ci == 0 and j == 0),
                                stop=(ci == 2 and j == 3),
                                perf_mode=DR,
                            )
                    return u

                def rope():
                    mul = mybir.AluOpType.mult
                    sub = mybir.AluOpType.subtract
                    add = mybir.AluOpType.add
                    pre, pim = st["psre"], st["psim"]
                    w = b - a
                    ccn = cc[:, t0 + a:t0 + b]
                    ssn = ss[:, t0 + a:t0 + b]
                    t1 = rtp.tile([128, CH], F16, tag="t1")
                    t2 = rtp.tile([128, CH], F16, tag="t2")
                    t3 = rtp.tile([128, CH], F16, tag="t3")
                    t4 = rtp.tile([128, CH], F16, tag="t4")
                    # psum -> fp16 bounce: ACT while it still has slack
                    # (early chunks), DVE once exp saturates ACT; the
                    # rope arithmetic runs in DVE 4x fp16 mode.
                    preb = rtp.tile([128, CH], F16, tag="preb")
                    pimb = rtp.tile([128, CH], F16, tag="pimb")
                    if act_bounce:
                        nc.scalar.copy(preb[:, 0:w], pre[:, a:b])
                        nc.scalar.copy(pimb[:, 0:w], pim[:, a:b])
                    else:
                        nc.vector.tensor_copy(preb[:, 0:w], pre[:, a:b])
                        nc.vector.tensor_copy(pimb[:, 0:w], pim[:, a:b])
                    preb_, pimb_ = preb[:, 0:w], pimb[:, 0:w]
                    nc.vector.tensor_tensor(t1[:, 0:w], preb_, ccn, mul)
                    nc.vector.tensor_tensor(t2[:, 0:w], pimb_, ssn, mul)
                    nc.vector.tensor_tensor(t3[:, 0:w], preb_, ssn, mul)
                    nc.vector.tensor_tensor(t4[:, 0:w], pimb_, ccn, mul)
                    # sub/add write the fp8 q8/k8 tiles directly (re
                    # half / im half): no relayout copies needed.  Pool
                    # (mostly idle) can take them to unload DVE.
                    eng = nc.gpsimd if SAPOOL else nc.vector
                    o = q8 if g == 0 else k8
                    eng.tensor_tensor(
                        o[:, t0 + a:t0 + b], t1[:, 0:w], t2[:, 0:w], sub)
                    eng.tensor_tensor(
                        o[:, T + t0 + a:T + t0 + b], t3[:, 0:w],
                        t4[:, 0:w], add)

                last = mm("psim", 2 * g + 1, 2)
                return [mm("psre", 2 * g, 0), mm("psre", 2 * g, 1),
                        mm("psre", 2 * g, 2), mm("psim", 2 * g + 1, 0),
                        mm("psim", 2 * g + 1, 1),
                        lambda: (last(), rope())]

            out = []
            for (a, b) in splits:
                out.extend(mk(a, b))
            return out

        def qk_g(n, g, act_bounce=False, splits=((0, CH),)):
            for u in qk_units(n, g, act_bounce, splits):
                u()

        def v_units(n, tb, act_bounce=False):
            """v projection for 128-row block tb of chunk n: 3 chain
            units of ~0.2us; the psum->fp16 bounce rides on the last."""
            t0 = n * CH + tb * 128
            st = {}

            def mm(ci):
                def u():
                    if ci == 0:
                        st["psv"] = psum.tile([128, CH], F32, tag="qk",
                                              name="psv")
                    xs, ws = CHAINS_V[ci]
                    for j in range(4):
                        nc.tensor.matmul(
                            st["psv"][:, 0:256],
                            lhsT=xs[:, :, j * T + t0:j * T + t0 + 128],
                            rhs=ws[:, :, j * 256:(j + 1) * 256],
                            start=(ci == 0 and j == 0),
                            stop=(ci == 2 and j == 3),
                            perf_mode=DR,
                        )
                    if ci == 2:
                        blk = 4 * n + tb
                        dst = vT_v[:, :, blk, 0:64]
                        src = st["psv"][:, 0:256].rearrange(
                            "p (h d) -> p h d", d=64)
                        # gpsimd cannot read PSUM; the bounce (psum holds
                        # 1024*v -- x/w ship pre-scaled x16/x64 to keep
                        # fp8 residuals out of the e4m3 subnormal range)
                        # folds in the 2^-10 fix free.  ACT takes it in
                        # the A/B phases where exp leaves it slack.
                        if act_bounce:
                            nc.scalar.activation(
                                dst, src,
                                mybir.ActivationFunctionType.Copy,
                                scale=1.0 / 1024.0)
                        else:
                            nc.vector.tensor_scalar_mul(
                                dst, src, 1.0 / 1024.0)
                return u

            return [mm(0), mm(1), mm(2)]

        def v_tb(n, tb):
            for u in v_units(n, tb):
                u()

        filler_q = []

        def dummy(n=2):
            # keep-warm matmuls: PE p-state drops 2x after an idle gap and
            # needs 3us of continuous execution to recover; padding known
            # exp-bound stretches keeps the real matmuls at full clock.
            pd = psum.tile([128, 512], F32, tag="qk", name="pd")
            for _ in range(n):
                nc.tensor.matmul(
                    pd[:, 0:512], lhsT=warm[:, 0:128], rhs=warm[:],
                    start=True, stop=True)

        def pump(keep_warm=0):
            if filler_q:
                filler_q.pop(0)()
            elif keep_warm:
                dummy(keep_warm)

        def attention_units(h, q0, qn, pump_every=0, at_blocks=None,
                            keep_warm=0, pair_full=False, norm_act=False):
            """One head, q-cols [q0, q0+qn) as a list of emission units.

            units[kb] = at_blocks/pump + scores(kb) + pv(kb-1); the last
            unit is pv(nkb-1) + final normalize.  The scheduler emits the
            NEXT head's units[0] just before this head's last unit: that
            head-boundary scores runs on PE while the last exp still
            streams on ACT, so ACT never drains between heads.  (The pst
            double-buffer is free at exactly that point: exp(nkb-2) has
            been consumed by the preceding pv.)

            at_blocks: {kb: [unit, ...]} -- mandatory work units emitted
            just before scores_block(kb); used for dependencies of later
            pv_blocks (e.g. v tiles), unlike best-effort pump fillers.
            """
            qv = q8v[32 * h:32 * h + 32]
            kv = k8v[32 * h:32 * h + 32]
            r0 = 64 * (h % 2)
            nkb = (q0 + qn) // 128
            fd = q0 // 128  # first diagonal block
            st_ = {}
            Ps = {}

            def block_off(kb):
                return 128 * (kb - fd) if kb >= fd else 0

            def scores_block(kb):
                if kb == 0:
                    st_["psy"] = psum.tile([128, qn], F32, tag="y",
                                           name="psy", bufs=1)
                off = block_off(kb)
                pst = psum.tile([128, qn], F32, tag="st", name="pst")
                for (a, b) in _splits(off, qn):
                    nc.tensor.matmul(
                        pst[:, a:b],
                        lhsT=kv[:, :, kb * 128:(kb + 1) * 128],
                        rhs=qv[:, :, q0 + a:q0 + b],
                        start=True,
                        stop=True,
                        perf_mode=DR,
                        tile_position=(32 * h, 0),
                    )
                P = pp.tile([128, 1024], F16, tag="P")
                Ps[kb] = (P, 0)
                nc.scalar.activation(
                    P[:, off:qn], pst[:, off:qn],
                    mybir.ActivationFunctionType.Exp, scale=SCALE)
                if kb >= fd:
                    # zero strictly-upper triangle of the leading 128 cols
                    nc.gpsimd.affine_select(
                        out=P[:, off:off + 128],
                        in_=P[:, off:off + 128],
                        compare_op=mybir.AluOpType.is_ge,
                        fill=0.0,
                        base=0,
                        pattern=[[1, 128]],
                        channel_multiplier=-1,
                    )

            def scores_pair(kb):
                # two qn=512 k-blocks share one [128,1024] pst / P pair:
                # a single exp call covers both, halving the ACT per-call
                # access-latency overhead and doubling the exp lookahead
                # the pst double-buffer can hold.  For diagonal blocks the
                # gap columns [512, 512+off1) are dead: matmul start
                # zeroes the bank region, exp of them is masked/unread
                # (pv reads only [off:512] of each half).
                if kb == 0:
                    st_["psy"] = psum.tile([128, qn], F32, tag="y",
                                           name="psy", bufs=1)
                pst = psum.tile([128, 1024], F32, tag="st", name="pst")
                for sub in (0, 1):
                    off = block_off(kb + sub)
                    nc.tensor.matmul(
                        pst[:, 512 * sub + off:512 * sub + 512],
                        lhsT=kv[:, :, (kb + sub) * 128:(kb + sub + 1) * 128],
                        rhs=qv[:, :, q0 + off:q0 + 512],
                        start=True,
                        stop=True,
                        perf_mode=DR,
                        tile_position=(32 * h, 0),
                    )
                off0 = block_off(kb)
                P = pp.tile([128, 1024], F16, tag="P")
                Ps[kb] = (P, 0)
                Ps[kb + 1] = (P, 512)
                nc.scalar.activation(
                    P[:, off0:1024], pst[:, off0:1024],
                    mybir.ActivationFunctionType.Exp, scale=SCALE)
                for sub in (0, 1):
                    off = block_off(kb + sub)
                    if kb + sub >= fd:
                        nc.gpsimd.affine_select(
                            out=P[:, 512 * sub + off:512 * sub + off + 128],
                            in_=P[:, 512 * sub + off:512 * sub + off + 128],
                            compare_op=mybir.AluOpType.is_ge,
                            fill=0.0,
                            base=0,
                            pattern=[[1, 128]],
                            channel_multiplier=-1,
                        )

            def pv_block(kb):
                off = block_off(kb)
                P, coff = Ps.pop(kb)
                psy = st_["psy"]
                for (a, b) in _splits(off, qn):
                    # last writer of the psum bank holding col a is diag
                    # block fd + 4*(a//512) + 3
                    kb_stop = min(fd + 4 * (a // 512) + 3, nkb - 1)
                    nc.tensor.matmul(
                        psy[:, a:b],
                        lhsT=vT_v[:, h, kb, :],
                        rhs=P[:, coff + a:coff + b],
                        start=(kb == 0),
                        stop=(kb == kb_stop),
                    )

            def normalize(a, b):
                # psum rows 64-127 all hold the denominator row l (ones
                # cols of vT): reciprocal + one multiply per psum bank,
                # emitted as soon as that bank's accumulation closes.
                # (walrus forbids two PSUM reads in one TensorTensor, so
                # a single divide is not possible.)
                psy = st_["psy"]
                rlb = nrm.tile([64, 512], F32, tag="rlb")
                nc.vector.reciprocal(rlb[:, 0:b - a], psy[64:128, a:b])
                nc.vector.tensor_tensor(
                    yT[h // 2][r0:r0 + 64, q0 + a:q0 + b],
                    psy[0:64, a:b], rlb[:, 0:b - a], mybir.AluOpType.mult)

            def mk_unit(kb, scores_fn, pvs):
                def u():
                    if at_blocks and kb in at_blocks:
                        for ab in at_blocks[kb]:
                            ab()
                    if pump_every and kb % pump_every == 0:
                        pump(keep_warm)
                    if scores_fn:
                        scores_fn(kb)
                    for pkb in pvs:
                        pv_block(pkb)
                        if pkb == min(fd + 3, nkb - 1) and qn > 512:
                            normalize(0, 512)  # bank 0 closed early
                return u

            units = []
            if PAIR and pair_full and qn == 512 and nkb % 2 == 0:
                # all blocks in exp-pairs; pv's trail one scores event
                pend = []
                for kb in range(0, nkb, 2):
                    units.append(mk_unit(kb, scores_pair, pend))
                    pend = [kb, kb + 1]
                st_["pend"] = pend
            else:
                for kb in range(nkb):
                    units.append(mk_unit(
                        kb, scores_block, [kb - 1] if kb > 0 else []))
                st_["pend"] = [nkb - 1]

            def final():
                for pkb in st_["pend"]:
                    pv_block(pkb)
                normalize(512 if qn > 512 else 0, qn)

            return units + [final]

        def run_heads(seq):
            """Emit head unit-lists with one-unit cross-head lookahead:
            the next head's scores(0) goes out before this head's final
            pv, so the exp stream never drains at a head boundary."""
            for i, units in enumerate(seq):
                for u in units[1 if i else 0:-1]:
                    u()
                if i + 1 < len(seq):
                    seq[i + 1][0]()
                units[-1]()

        def o_proj(nt, mo, tail=False):
            """Output block: feat rows [128*mo ..+128), q [512*nt ..+512)."""
            ob = obp.tile([128, 512], F16, tag="ob", name="ob")
            ps = psum.tile([128, CH], F32, tag="qk", name="psob")
            for kb in range(2):
                nc.tensor.matmul(
                    ps[:, 0:512],
                    lhsT=wo_sb[kb][:, mo * 128:(mo + 1) * 128],
                    rhs=yT[kb][:, nt * 512:(nt + 1) * 512],
                    start=(kb == 0),
                    stop=(kb == 1),
                )
            # in the tail ACT is idle once the exps are done: it takes half
            # the psum bounces there
            if tail and mo % 2 == 1:
                nc.scalar.copy(ob[:], ps[:, 0:512])
            else:
                nc.vector.tensor_copy(ob[:], ps[:, 0:512])
            # keep DMA issue off the ACT queue while exps run; in the tail
            # ACT is free and a second queue doubles drain bandwidth
            ring = nc.scalar if (tail and mo % 2 == 1) else nc.sync
            ring.dma_start(
                out=outT_d[mo * 128:(mo + 1) * 128, nt * 512:(nt + 1) * 512],
                in_=ob[:])

        # ---- schedule ----
        # Emission order == per-engine queue order.  ACT (exp) is the
        # attention pacer now that scores+PV run fp8-DR/fp16 (PE 0.625
        # vs ACT 0.833 ns per score column), so all qkv/o_proj work is
        # pumped into the attention stream as ~0.4us filler units.
        # Dependency safety comes from emission order: a filler is
        # always emitted before the instruction that needs it.
        x_load(0)
        x_load(1)
        qk_g(0, 0, act_bounce=True)
        # k chunk 0 split: cols 0-256 (k-blocks 0/1) go out first so the
        # first exp fires ~2.5us earlier; the rest rides as at_blocks of
        # head 0 before scores(2) needs it.
        if FASTK:
            k0u = qk_units(0, 1, act_bounce=True,
                           splits=((0, 256), (256, CH)))
            for u in k0u[:6]:
                u()
            k0rest = k0u[6:]
        else:
            qk_g(0, 1, act_bounce=True)
            k0rest = []
        # tile A (q 0-512): needs only chunk 0 q/k; v blocks land as
        # at_blocks just before their pv consumer.  The chunk-1 qk and v
        # work soaks up the rope-latency wait before the first scores.
        filler_q.extend(qk_units(1, 0, act_bounce=True))
        filler_q.extend(qk_units(1, 1, act_bounce=True))
        for tb in range(4):
            filler_q.extend(v_units(1, tb, act_bounce=bool(AB23)))
        run_heads([
            attention_units(0, 0, 512, pump_every=PUMP_A, norm_act=True, at_blocks={
                1: k0rest + v_units(0, 0, act_bounce=bool(AB23)),
                2: v_units(0, 1, act_bounce=bool(AB23)),
                3: v_units(0, 2, act_bounce=bool(AB23))
                   + v_units(0, 3, act_bounce=bool(AB23)),
            }),
            attention_units(1, 0, 512, pump_every=PUMP_A, norm_act=True, pair_full=True),
            attention_units(2, 0, 512, pump_every=PUMP_A, norm_act=True, pair_full=True),
            attention_units(3, 0, 512, pump_every=PUMP_A, norm_act=True, pair_full=True),
        ])
        while filler_q:
            pump()
        # tile B (q 512-1024): fillers: chunks 2,3 qk.
        x_load(2)
        filler_q.extend(qk_units(2, 0, act_bounce=bool(AB23)))
        filler_q.extend(qk_units(2, 1, act_bounce=bool(AB23)))
        filler_q.append(lambda: x_load(3))
        filler_q.extend(qk_units(3, 0, act_bounce=bool(AB23)))
        filler_q.extend(qk_units(3, 1, act_bounce=bool(AB23)))
        run_heads([
            attention_units(0, 512, 512, pump_every=PUMP_B, pair_full=True, norm_act=True),
            attention_units(1, 512, 512, pump_every=PUMP_B, pair_full=True, norm_act=True),
            attention_units(2, 512, 512, pump_every=PUMP_B, pair_full=True, norm_act=True),
            attention_units(3, 512, 512, pump_every=PUMP_B, pair_full=True, norm_act=True),
        ])
        while filler_q:
            pump()
        # tail o_proj helpers (q 1024-2048): per mo one [128, 1024] ob
        # filled in two halves; nt=2 halves are emitted inside the last
        # head's attention as soon as its early psum bank is normalized.
        tail_obs = {}

        # single [128, 8x1024] tile for the q 1024-2048 output: per mo,
        # cols [1024mo, +512) = nt2 half, [+512, +1024) = nt3 half.
        # Keeping it one tile lets the tail flush as FOUR two-mo strided
        # DMAs instead of 16 narrow ones: each DMA costs ~625ns on the
        # shared HWDGE regardless of size, and that serialization was
        # the dominant term of the post-attention tail.
        obt_all = const.tile([128, 8 * 1024], F16, tag="obt", name="obt")

        def tail_half(nt, mo):
            # nt3 runs after the last attention: the st/y psum banks are
            # free, so cycling tags gives 5 buffers in rotation instead
            # of 2 and breaks the psum-recycle serialization of the tail.
            if nt == 3:
                tag, bufs = [("qk", 2), ("st", 2), ("y", 1)][mo % 3]
                ps = psum.tile([128, CH], F32, tag=tag, name="psob",
                               bufs=bufs)
            else:
                ps = psum.tile([128, CH], F32, tag="qk", name="psob")
            for kb in range(2):
                nc.tensor.matmul(
                    ps[:, 0:512],
                    lhsT=wo_sb[kb][:, mo * 128:(mo + 1) * 128],
                    rhs=yT[kb][:, nt * 512:(nt + 1) * 512],
                    start=(kb == 0),
                    stop=(kb == 1),
                )
            c0 = 1024 * mo + 512 * (nt - 2)
            if nt == 3 and mo % 2 == 1:
                # true tail: exps are done, ACT is free
                nc.scalar.copy(obt_all[:, c0:c0 + 512], ps[:, 0:512])
            else:
                nc.vector.tensor_copy(obt_all[:, c0:c0 + 512],
                                      ps[:, 0:512])
            if nt == 2:
                # nt2 halves flush per-mo while attention still runs
                nc.sync.dma_start(
                    out=outT_d[mo * 128:(mo + 1) * 128, 1024:1536],
                    in_=obt_all[:, c0:c0 + 512])
            elif mo % 2 == 1:
                # nt3: one strided DMA flushes the (mo-1, mo) pair
                ring = nc.scalar if mo % 4 == 1 else nc.sync
                dst = outT_d.rearrange("(m p) t -> p m t", p=128)
                src = obt_all[:].rearrange("p (m t) -> p m t", t=1024)
                ring.dma_start(
                    out=dst[:, mo - 1:mo + 1, 1536:2048],
                    in_=src[:, mo - 1:mo + 1, 512:1024])

        # tile C (q 1024-2048): v chunks 2/3 are emitted at fixed blocks of
        # the first head (hard deps of pv blocks 8-15); o_proj of q 0-1024
        # is order-free filler spread across all four heads.
        filler_q.extend(
            (lambda nt=nt, mo=mo: o_proj(nt, mo))
            for nt in range(2) for mo in range(8))
        run_heads([
            attention_units(0, 1024, 1024, pump_every=PUMP_C0, at_blocks={
                5: v_units(2, 0) + v_units(2, 1),
                7: v_units(2, 2) + v_units(2, 3),
                9: v_units(3, 0) + v_units(3, 1),
                11: v_units(3, 2) + v_units(3, 3),
            }),
            attention_units(1, 1024, 1024, pump_every=PUMP_C,
                            keep_warm=KEEP_WARM),
            attention_units(2, 1024, 1024, pump_every=PUMP_C,
                            keep_warm=KEEP_WARM),
            attention_units(3, 1024, 1024, pump_every=PUMP_C,
                            keep_warm=KW3, at_blocks={
                13: [lambda mo=mo: tail_half(2, mo) for mo in range(2)],
                14: [lambda mo=mo: tail_half(2, mo) for mo in range(2, 5)],
                15: [lambda mo=mo: tail_half(2, mo) for mo in range(5, 8)],
            }),
        ])
        while filler_q:
            pump()
        for mo in range(8):
            tail_half(3, mo)

    nc.compile()
    return nc


def _dr_pack(a, scale):
    """[1024, M] f32 -> fp8 (hi, residual) pair in DR layout [128, 8M].

    DR layout "p (i j m)": element (p, i, j, m) = a[256j + 128i + p, m], so
    one DoubleRow matmul contracts dim pairs (256j+p, 256j+128+p).
    `scale` lifts the values so both them and their residuals quantize in
    the e4m3 normal range; the device compensates (rope tables / v copy).
    """
    import ml_dtypes
    f8 = ml_dtypes.float8_e4m3
    a = a * scale
    M = a.shape[1]
    hi = a.astype(f8)
    lo = (a - hi.astype(np.float32)).astype(f8)
    out = []
    for t in (hi, lo):
        t = t.reshape(4, 2, 128, M).transpose(2, 1, 0, 3).reshape(128, 8 * M)
        out.append(np.ascontiguousarray(t))
    return out


def shard_inputs(x, freqs_cos, freqs_sin, Wqkv, Wo):
    """Build the 8 per-core input maps (host-side sharding)."""
    x = np.asarray(x, dtype=np.float32)
    Wqkv = np.asarray(Wqkv, dtype=np.float32)
    Wo = np.asarray(Wo, dtype=np.float32)
    # cos/sin tables transposed and replicated x4 (one copy per local
    # head), pre-divided by 1024 to undo the x16/x64 fp8 shipping scales
    ccT = np.tile(np.asarray(freqs_cos, dtype=np.float32).T, (4, 1)) / 1024.0
    ssT = np.tile(np.asarray(freqs_sin, dtype=np.float32).T, (4, 1)) / 1024.0
    ccT = np.ascontiguousarray(ccT).astype(np.float16)
    ssT = np.ascontiguousarray(ssT).astype(np.float16)
    x8s = [_dr_pack(x[b].T, 16.0) for b in range(B)]

    in_maps = []
    for c in range(NCORE):
        b, hg = c // 4, c % 4
        re = [np.arange(g * 64, g * 64 + 64, 2)
              for g in range(4 * hg, 4 * hg + 4)]
        im = [np.arange(g * 64 + 1, g * 64 + 64, 2)
              for g in range(4 * hg, 4 * hg + 4)]
        qcols = np.concatenate(re + im)
        kcols = C + qcols
        wqk8, dwqk8 = _dr_pack(Wqkv[:, np.concatenate([qcols, kcols])], 64.0)
        wv8, dwv8 = _dr_pack(
            Wqkv[:, 2 * C + hg * 256: 2 * C + hg * 256 + 256], 64.0)
        wo = np.ascontiguousarray(
            Wo[hg * 256: hg * 256 + 256, :]).astype(np.float16)
        in_maps.append({
            "x8": x8s[b][0], "dx8": x8s[b][1],
            "wqk8": wqk8, "dwqk8": dwqk8, "wv8": wv8, "dwv8": dwv8,
            "wo": wo, "ccT": ccT, "ssT": ssT,
        })
    return in_maps


_NC_CACHE = None


def _get_nc():
    global _NC_CACHE
    if _NC_CACHE is None:
        _NC_CACHE = build_nc()
    return _NC_CACHE


def run(inputs, trace=False):
    from concourse.bass_utils import run_bass_kernel_spmd

    nc = _get_nc()
    in_maps = shard_inputs(**inputs)
    res = run_bass_kernel_spmd(nc, in_maps, list(range(NCORE)), trace=trace)
    out = np.empty((B, T, C), dtype=np.float32)
    for b in range(B):
        acc = res.results[4 * b]["outT"].astype(np.float32)
        for c in range(4 * b + 1, 4 * b + 4):
            acc = acc + res.results[c]["outT"].astype(np.float32)
        out[b] = acc.T
    return out, res


def kernel(**inputs):
    out, _ = run(inputs)
    return out



# revision 4
# speedup vs baseline: 1.0066x; 1.0013x over previous
"""Causal self-attention (B=2, T=2048, C=1024, H=16, D=64) with RoPE on TRN2.

Sharding: 8 cores = 2 (batch) x 4 (head-groups of 4 heads), no
inter-core communication: each core computes qkv + rope + causal
attention + a row-parallel o_proj partial for its 4 heads; the host
sums the 4 fp16 partials per batch.

Precision: x and the q/k/v weights ship as fp8-e4m3 (hi, residual)
pairs, pre-scaled x16 / x64 so the residuals stay out of the e4m3
subnormal range; 3 DoubleRow accumulation chains (x8 w8 + x8 dw8 +
dx8 w8) reproduce the fp16 projection to ~0.2% at 3/8 the PE cost
(DoubleRow contracts 256 dims at 0.5 cycles/col).  The 2^-10 shipping
scale is undone for free in the rope tables (q/k) and the v psum
bounce.  Roped q/k are quantized to fp8 on the fly -- the rope sub/add
writes fp8 tiles directly in a DoubleRow-ready layout (head h in
partitions [32h,32h+32), re dims cols [0,T), im cols [T,2T)) -- so
QK^T runs as one fp8-DR matmul per (head, k-block) at 0.5 cycles/col:
2x the fp16 rate with only a single e4m3 quantization of q/k (rel err
1.65e-2 vs the 2e-2 gate).  P and V stay fp16 (peaked attention rows
pass V errors straight through, so V cannot be fp8), o_proj fp16.

Attention computes S^T = (K Q^T) per 128-row k-block so probabilities
exit exp() already transposed for P^T @ V.  V tiles carry 64 all-ones
columns: the PV matmul then yields psum rows 0-63 = y, rows 64-127 =
the softmax denominator replicated 64x (zero extra PE cycles since
matmul cost is free-dim only) -- normalization is one DVE reciprocal +
multiply per psum bank, emitted as soon as that bank closes.  No max
subtraction: logits are O(+-8) and exp fits fp16 range.

Scheduling: emission order is per-engine queue order.  exp on ACT
(0.833 ns/col) outweighs the fp8-DR scores + fp16 PV (0.625 ns/col),
so attention is ACT-paced and all projection/o_proj work is pumped
into the attention stream as ~0.4us PE filler units (chain-quarters
of the DR accumulations).  Heads are emitted with a one-unit
lookahead (next head's first scores before this head's last pv) so
the exp stream does not drain at head boundaries; rope/v psum bounces
ride on ACT only where exp leaves it slack.  DMA issues are kept off
queues whose sequencer is latency-critical (each issue holds its SEQ
~1.3us): the late-needed weight/table loads are emitted mid-schedule.
The o_proj tail is ordered so head 3's final normalize is not queued
behind tail copies on DVE, pads PE p-state over the normalize wait,
flushes q 1024-2048 as a few wide strided DMAs (each DMA costs
~625ns on the shared HWDGE regardless of size) and cycles its psum
tiles across the qk/st/y tags -- the attention banks are free by
then -- so the final o_proj chain is not serialized on the 2-buffer
psum rotation.
"""

import sys
import os

sys.path.insert(0, "/opt/trn_rl_repo")

import numpy as np
from contextlib import ExitStack

import concourse.bass as bass
import concourse.bacc as bacc
import concourse.mybir as mybir
import concourse.tile as tile

F32 = mybir.dt.float32
F16 = mybir.dt.float16
F8 = mybir.dt.float8e4
DR = mybir.MatmulPerfMode.DoubleRow

# problem constants (hardcoded per contract)
B, T, C, NH, D = 2, 2048, 1024, 16, 64
# schedule knobs (env-overridable for tuning sweeps)
PUMP_A = int(os.environ.get("K_PUMP_A", "2"))
PUMP_B = int(os.environ.get("K_PUMP_B", "2"))
PUMP_C0 = int(os.environ.get("K_PUMP_C0", "5"))
PUMP_C = int(os.environ.get("K_PUMP_C", "4"))
KEEP_WARM = int(os.environ.get("K_KEEP_WARM", "0"))
WARM_N = int(os.environ.get("K_WARM_N", "8"))
PAIR = int(os.environ.get("K_PAIR", "0"))
FASTK = int(os.environ.get("K_FASTK", "0"))
AB23 = int(os.environ.get("K_AB23", "1"))
SAPOOL = int(os.environ.get("K_SAPOOL", "0"))
KW3 = int(os.environ.get("K_KW3", "0"))
HL = 4            # local heads per core
NCORE = 8
CH = 512          # qkv T-chunk width
NCHUNK = T // CH  # 4
SCALE = 1.0 / 8.0  # 1/sqrt(D)
NKB = T // 128    # 16 k-blocks


def _splits(a, b):
    """Split [a, b) at 512 boundaries (psum bank = 512 f32)."""
    out = []
    while a < b:
        nxt = min(b, (a // 512 + 1) * 512)
        out.append((a, nxt))
        a = nxt
    return out


def build_nc():
    nc = bacc.Bacc("TRN2", debug=False, num_devices=NCORE)

    # DR-packed fp8 operands: "p (i j c)" layout pairs contraction dims
    # 256j+128i+p so one DoubleRow matmul contracts 256 dims in 0.5
    # cycles/col.  x and the q/k/v weights ship as (hi, residual) fp8
    # pairs; 3 accumulation chains (x8 w8 + x8 dw8 + dx8 w8) reproduce
    # the fp16 product to ~0.2% at 3/8 the PE cost.
    x8_d = nc.dram_tensor("x8", [128, 8 * T], F8, kind="ExternalInput").ap()
    dx8_d = nc.dram_tensor("dx8", [128, 8 * T], F8, kind="ExternalInput").ap()
    wqk8_d = nc.dram_tensor("wqk8", [128, 4096], F8, kind="ExternalInput").ap()
    dwqk8_d = nc.dram_tensor("dwqk8", [128, 4096], F8,
                             kind="ExternalInput").ap()
    wv8_d = nc.dram_tensor("wv8", [128, 2048], F8, kind="ExternalInput").ap()
    dwv8_d = nc.dram_tensor("dwv8", [128, 2048], F8,
                            kind="ExternalInput").ap()
    wo_d = nc.dram_tensor("wo", [256, C], F16, kind="ExternalInput").ap()
    ccT_d = nc.dram_tensor("ccT", [128, T], F16, kind="ExternalInput").ap()
    ssT_d = nc.dram_tensor("ssT", [128, T], F16, kind="ExternalInput").ap()
    outT_d = nc.dram_tensor("outT", [C, T], F16, kind="ExternalOutput").ap()

    with tile.TileContext(nc) as tc, ExitStack() as ctx:
        const = ctx.enter_context(tc.tile_pool(name="const", bufs=1))
        rtp = ctx.enter_context(tc.tile_pool(name="rtp", bufs=2))
        pp = ctx.enter_context(tc.tile_pool(name="pp", bufs=5))
        nrm = ctx.enter_context(tc.tile_pool(name="nrm", bufs=3))
        obp = ctx.enter_context(tc.tile_pool(name="obp", bufs=4))
        psum = ctx.enter_context(tc.tile_pool(name="psum", bufs=2, space="PSUM"))

        # ---- persistent SBUF tensors ----
        # q/k weights (hi + residual) land first so the first real
        # matmuls can start while the rest is still in flight.
        wqk8 = const.tile([128, 4096], F8, tag="wqk8", name="wqk8")
        dwqk8 = const.tile([128, 4096], F8, tag="dwqk8", name="dwqk8")
        nc.scalar.dma_start(out=wqk8[:], in_=wqk8_d[:])
        nc.scalar.dma_start(out=dwqk8[:], in_=dwqk8_d[:])
        # views "p i (j m)": i-halves of the 256-dim contraction pairs
        wqk8_v = wqk8[:].rearrange("p (i jm) -> p i jm", i=2)
        dwqk8_v = dwqk8[:].rearrange("p (i jm) -> p i jm", i=2)

        # remaining input DMAs stay OFF the scalar queue: each DMA issue
        # holds its SEQ ~1.3us, and ACT's sequencer must be free for the
        # first rope bounces.  Early-needed tensors ride the sync queue;
        # late-needed ones go through Pool's SWDGE (Pool is idle early).
        cc = const.tile([128, T], F16, tag="cc")
        ss = const.tile([128, T], F16, tag="ss")
        nc.scalar.dma_start(out=cc[:, 0:1024], in_=ccT_d[:, 0:1024])
        nc.scalar.dma_start(out=ss[:, 0:1024], in_=ssT_d[:, 0:1024])

        wv8 = const.tile([128, 2048], F8, tag="wv8", name="wv8")
        dwv8 = const.tile([128, 2048], F8, tag="dwv8", name="dwv8")
        wv8_v = wv8[:].rearrange("p (i jd) -> p i jd", i=2)
        dwv8_v = dwv8[:].rearrange("p (i jd) -> p i jd", i=2)

        # PE warm-up: dependency-free matmuls that cover the input-DMA wait
        # and carry the tensor engine through its p-state ramp before the
        # first real matmul issues.
        warm = const.tile([128, 512], F16, tag="warm")
        nc.gpsimd.memset(warm[:], 0.0)
        pw = psum.tile([128, 512], F32, tag="qk", name="pw")
        for _ in range(WARM_N):
            nc.tensor.matmul(
                pw[:, 0:512], lhsT=warm[:, 0:128], rhs=warm[:],
                start=True, stop=True)

        wo_all = const.tile([128, 2 * C], F16, tag="wo", name="wo")
        nc.sync.dma_start(
            out=wo_all[:].rearrange("p (kt n) -> p kt n", n=C),
            in_=wo_d.rearrange("(kt p) n -> p kt n", p=128))
        wo_sb = [wo_all[:, kb * C:(kb + 1) * C] for kb in range(2)]

        # q8/k8: fp8e4 tiles [128, 2T].  head h lives in partitions
        # [32h, 32h+32); re dims at cols [0,T), im dims at cols [T,2T).
        # This matches the wqk column order (re of h0..h3, im of h0..h3),
        # so rope's sub/add write them directly -- no relayout copies --
        # and QK^T runs as one fp8 DoubleRow matmul per (head, k-block):
        # lhsT=[32,2,128] k-view, rhs=[32,2,n] q-view contract 64 dims.
        q8 = const.tile([128, 2 * T], F8, tag="q8", name="q8")
        k8 = const.tile([128, 2 * T], F8, tag="k8", name="k8")
        q8v = q8[:].rearrange("p (i t) -> p i t", i=2)
        k8v = k8[:].rearrange("p (i t) -> p i t", i=2)
        # v: [128 kpos, 4 heads x 16 blocks x 128] fp16; cols 0-63 of each
        # block = v dims, cols 64-127 = ones (denominator rows of PV psum)
        vT = const.tile([128, HL * NKB * 128], F16, tag="vT", name="vT")
        vT_v = vT[:].rearrange("p (h b c) -> p h b c", h=HL, b=NKB)
        nc.gpsimd.memset(vT_v[:, :, :, 64:128], 1.0)
        # y^T tiles: [128, T] x2 (4 heads x 64 dims)
        yT = [const.tile([128, T], F16, tag=f"yT{kb}", name=f"yT{kb}")
              for kb in range(2)]

        # x (hi + residual) fp8, persistent DR layout "p (i j t)"
        x8 = const.tile([128, 8 * T], F8, tag="x8", name="x8")
        dx8 = const.tile([128, 8 * T], F8, tag="dx8", name="dx8")
        x8_v = x8[:].rearrange("p (i jt) -> p i jt", i=2)
        dx8_v = dx8[:].rearrange("p (i jt) -> p i jt", i=2)
        x8_4v = x8[:].rearrange("p (i j t) -> p i j t", i=2, j=4)
        dx8_4v = dx8[:].rearrange("p (i j t) -> p i j t", i=2, j=4)
        x8_d4v = x8_d.rearrange("p (i j t) -> p i j t", i=2, j=4)
        dx8_d4v = dx8_d.rearrange("p (i j t) -> p i j t", i=2, j=4)

        def x_load(n, ways=2):
            t0 = n * CH
            nc.sync.dma_start(out=x8_4v[:, :, :, t0:t0 + CH],
                              in_=x8_d4v[:, :, :, t0:t0 + CH])
            nc.sync.dma_start(out=dx8_4v[:, :, :, t0:t0 + CH],
                              in_=dx8_d4v[:, :, :, t0:t0 + CH])

        CHAINS_QK = ((x8_v, wqk8_v), (x8_v, dwqk8_v), (dx8_v, wqk8_v))
        CHAINS_V = ((x8_v, wv8_v), (x8_v, dwv8_v), (dx8_v, wv8_v))

        def qk_units(n, g, act_bounce=False, splits=((0, CH),)):
            """q (g=0) / k (g=1) projection + rope for chunk n as a list
            of ~0.4us PE filler units (chain-quarters of the pre/pim psum
            accumulations; rope rides on the last one).  `splits` breaks
            the chunk into column ranges emitted as separate unit groups
            (used to fast-path the first k-block before the first exp)."""
            t0 = n * CH

            def mk(a, b):
                st = {}

                def mm(name, m, ci):
                    def u():
                        if ci == 0:
                            st[name] = psum.tile([128, CH], F32, tag="qk",
                                                 name=name)
                        xs, ws = CHAINS_QK[ci]
                        for j in range(4):
                            nc.tensor.matmul(
                                st[name][:, a:b],
                                lhsT=ws[:, :, j * 512 + m * 128:
                                        j * 512 + (m + 1) * 128],
                                rhs=xs[:, :, j * T + t0 + a:j * T + t0 + b],
                                start=(ci == 0 and j == 0),
                                stop=(ci == 2 and j == 3),
                                perf_mode=DR,
                            )
                    return u

                def rope():
                    mul = mybir.AluOpType.mult
                    sub = mybir.AluOpType.subtract
                    add = mybir.AluOpType.add
                    pre, pim = st["psre"], st["psim"]
                    w = b - a
                    ccn = cc[:, t0 + a:t0 + b]
                    ssn = ss[:, t0 + a:t0 + b]
                    t1 = rtp.tile([128, CH], F16, tag="t1")
                    t2 = rtp.tile([128, CH], F16, tag="t2")
                    t3 = rtp.tile([128, CH], F16, tag="t3")
                    t4 = rtp.tile([128, CH], F16, tag="t4")
                    # psum -> fp16 bounce: ACT while it still has slack
                    # (early chunks), DVE once exp saturates ACT; the
                    # rope arithmetic runs in DVE 4x fp16 mode.
                    preb = rtp.tile([128, CH], F16, tag="preb")
                    pimb = rtp.tile([128, CH], F16, tag="pimb")
                    if act_bounce:
                        nc.scalar.copy(preb[:, 0:w], pre[:, a:b])
                        nc.scalar.copy(pimb[:, 0:w], pim[:, a:b])
                    else:
                        nc.vector.tensor_copy(preb[:, 0:w], pre[:, a:b])
                        nc.vector.tensor_copy(pimb[:, 0:w], pim[:, a:b])
                    preb_, pimb_ = preb[:, 0:w], pimb[:, 0:w]
                    nc.vector.tensor_tensor(t1[:, 0:w], preb_, ccn, mul)
                    nc.vector.tensor_tensor(t2[:, 0:w], pimb_, ssn, mul)
                    nc.vector.tensor_tensor(t3[:, 0:w], preb_, ssn, mul)
                    nc.vector.tensor_tensor(t4[:, 0:w], pimb_, ccn, mul)
                    # sub/add write the fp8 q8/k8 tiles directly (re
                    # half / im half): no relayout copies needed.  Pool
                    # (mostly idle) can take them to unload DVE.
                    eng = nc.gpsimd if SAPOOL else nc.vector
                    o = q8 if g == 0 else k8
                    eng.tensor_tensor(
                        o[:, t0 + a:t0 + b], t1[:, 0:w], t2[:, 0:w], sub)
                    eng.tensor_tensor(
                        o[:, T + t0 + a:T + t0 + b], t3[:, 0:w],
                        t4[:, 0:w], add)

                last = mm("psim", 2 * g + 1, 2)
                return [mm("psre", 2 * g, 0), mm("psre", 2 * g, 1),
                        mm("psre", 2 * g, 2), mm("psim", 2 * g + 1, 0),
                        mm("psim", 2 * g + 1, 1),
                        lambda: (last(), rope())]

            out = []
            for (a, b) in splits:
                out.extend(mk(a, b))
            return out

        def qk_g(n, g, act_bounce=False, splits=((0, CH),)):
            for u in qk_units(n, g, act_bounce, splits):
                u()

        def v_units(n, tb, act_bounce=False):
            """v projection for 128-row block tb of chunk n: 3 chain
            units of ~0.2us; the psum->fp16 bounce rides on the last."""
            t0 = n * CH + tb * 128
            st = {}

            def mm(ci):
                def u():
                    if ci == 0:
                        st["psv"] = psum.tile([128, CH], F32, tag="qk",
                                              name="psv")
                    xs, ws = CHAINS_V[ci]
                    for j in range(4):
                        nc.tensor.matmul(
                            st["psv"][:, 0:256],
                            lhsT=xs[:, :, j * T + t0:j * T + t0 + 128],
                            rhs=ws[:, :, j * 256:(j + 1) * 256],
                            start=(ci == 0 and j == 0),
                            stop=(ci == 2 and j == 3),
                            perf_mode=DR,
                        )
                    if ci == 2:
                        blk = 4 * n + tb
                        dst = vT_v[:, :, blk, 0:64]
                        src = st["psv"][:, 0:256].rearrange(
                            "p (h d) -> p h d", d=64)
                        # gpsimd cannot read PSUM; the bounce (psum holds
                        # 1024*v -- x/w ship pre-scaled x16/x64 to keep
                        # fp8 residuals out of the e4m3 subnormal range)
                        # folds in the 2^-10 fix free.  ACT takes it in
                        # the A/B phases where exp leaves it slack.
                        if act_bounce:
                            nc.scalar.activation(
                                dst, src,
                                mybir.ActivationFunctionType.Copy,
                                scale=1.0 / 1024.0)
                        else:
                            nc.vector.tensor_scalar_mul(
                                dst, src, 1.0 / 1024.0)
                return u

            return [mm(0), mm(1), mm(2)]

        def v_tb(n, tb):
            for u in v_units(n, tb):
                u()

        filler_q = []

        def dummy(n=2):
            # keep-warm matmuls: PE p-state drops 2x after an idle gap and
            # needs 3us of continuous execution to recover; padding known
            # exp-bound stretches keeps the real matmuls at full clock.
            pd = psum.tile([128, 512], F32, tag="qk", name="pd")
            for _ in range(n):
                nc.tensor.matmul(
                    pd[:, 0:512], lhsT=warm[:, 0:128], rhs=warm[:],
                    start=True, stop=True)

        def pump(keep_warm=0):
            if filler_q:
                filler_q.pop(0)()
            elif keep_warm:
                dummy(keep_warm)

        def attention_units(h, q0, qn, pump_every=0, at_blocks=None,
                            keep_warm=0, pair_full=False, norm_act=False):
            """One head, q-cols [q0, q0+qn) as a list of emission units.

            units[kb] = at_blocks/pump + scores(kb) + pv(kb-1); the last
            unit is pv(nkb-1) + final normalize.  The scheduler emits the
            NEXT head's units[0] just before this head's last unit: that
            head-boundary scores runs on PE while the last exp still
            streams on ACT, so ACT never drains between heads.  (The pst
            double-buffer is free at exactly that point: exp(nkb-2) has
            been consumed by the preceding pv.)

            at_blocks: {kb: [unit, ...]} -- mandatory work units emitted
            just before scores_block(kb); used for dependencies of later
            pv_blocks (e.g. v tiles), unlike best-effort pump fillers.
            """
            qv = q8v[32 * h:32 * h + 32]
            kv = k8v[32 * h:32 * h + 32]
            r0 = 64 * (h % 2)
            nkb = (q0 + qn) // 128
            fd = q0 // 128  # first diagonal block
            st_ = {}
            Ps = {}

            def block_off(kb):
                return 128 * (kb - fd) if kb >= fd else 0

            def scores_block(kb):
                if kb == 0:
                    st_["psy"] = psum.tile([128, qn], F32, tag="y",
                                           name="psy", bufs=1)
                off = block_off(kb)
                pst = psum.tile([128, qn], F32, tag="st", name="pst")
                for (a, b) in _splits(off, qn):
                    nc.tensor.matmul(
                        pst[:, a:b],
                        lhsT=kv[:, :, kb * 128:(kb + 1) * 128],
                        rhs=qv[:, :, q0 + a:q0 + b],
                        start=True,
                        stop=True,
                        perf_mode=DR,
                        tile_position=(32 * h, 0),
                    )
                P = pp.tile([128, 1024], F16, tag="P")
                Ps[kb] = (P, 0)
                nc.scalar.activation(
                    P[:, off:qn], pst[:, off:qn],
                    mybir.ActivationFunctionType.Exp, scale=SCALE)
                if kb >= fd:
                    # zero strictly-upper triangle of the leading 128 cols
                    nc.gpsimd.affine_select(
                        out=P[:, off:off + 128],
                        in_=P[:, off:off + 128],
                        compare_op=mybir.AluOpType.is_ge,
                        fill=0.0,
                        base=0,
                        pattern=[[1, 128]],
                        channel_multiplier=-1,
                    )

            def scores_pair(kb):
                # two qn=512 k-blocks share one [128,1024] pst / P pair:
                # a single exp call covers both, halving the ACT per-call
                # access-latency overhead and doubling the exp lookahead
                # the pst double-buffer can hold.  For diagonal blocks the
                # gap columns [512, 512+off1) are dead: matmul start
                # zeroes the bank region, exp of them is masked/unread
                # (pv reads only [off:512] of each half).
                if kb == 0:
                    st_["psy"] = psum.tile([128, qn], F32, tag="y",
                                           name="psy", bufs=1)
                pst = psum.tile([128, 1024], F32, tag="st", name="pst")
                for sub in (0, 1):
                    off = block_off(kb + sub)
                    nc.tensor.matmul(
                        pst[:, 512 * sub + off:512 * sub + 512],
                        lhsT=kv[:, :, (kb + sub) * 128:(kb + sub + 1) * 128],
                        rhs=qv[:, :, q0 + off:q0 + 512],
                        start=True,
                        stop=True,
                        perf_mode=DR,
                        tile_position=(32 * h, 0),
                    )
                off0 = block_off(kb)
                P = pp.tile([128, 1024], F16, tag="P")
                Ps[kb] = (P, 0)
                Ps[kb + 1] = (P, 512)
                nc.scalar.activation(
                    P[:, off0:1024], pst[:, off0:1024],
                    mybir.ActivationFunctionType.Exp, scale=SCALE)
                for sub in (0, 1):
                    off = block_off(kb + sub)
                    if kb + sub >= fd:
                        nc.gpsimd.affine_select(
                            out=P[:, 512 * sub + off:512 * sub + off + 128],
                            in_=P[:, 512 * sub + off:512 * sub + off + 128],
                            compare_op=mybir.AluOpType.is_ge,
                            fill=0.0,
                            base=0,
                            pattern=[[1, 128]],
                            channel_multiplier=-1,
                        )

            def pv_block(kb):
                off = block_off(kb)
                P, coff = Ps.pop(kb)
                psy = st_["psy"]
                for (a, b) in _splits(off, qn):
                    # last writer of the psum bank holding col a is diag
                    # block fd + 4*(a//512) + 3
                    kb_stop = min(fd + 4 * (a // 512) + 3, nkb - 1)
                    nc.tensor.matmul(
                        psy[:, a:b],
                        lhsT=vT_v[:, h, kb, :],
                        rhs=P[:, coff + a:coff + b],
                        start=(kb == 0),
                        stop=(kb == kb_stop),
                    )

            def normalize(a, b):
                # psum rows 64-127 all hold the denominator row l (ones
                # cols of vT): reciprocal + one multiply per psum bank,
                # emitted as soon as that bank's accumulation closes.
                # (walrus forbids two PSUM reads in one TensorTensor, so
                # a single divide is not possible.)
                psy = st_["psy"]
                rlb = nrm.tile([64, 512], F32, tag="rlb")
                nc.vector.reciprocal(rlb[:, 0:b - a], psy[64:128, a:b])
                nc.vector.tensor_tensor(
                    yT[h // 2][r0:r0 + 64, q0 + a:q0 + b],
                    psy[0:64, a:b], rlb[:, 0:b - a], mybir.AluOpType.mult)

            def mk_unit(kb, scores_fn, pvs):
                def u():
                    if at_blocks and kb in at_blocks:
                        for ab in at_blocks[kb]:
                            ab()
                    if pump_every and kb % pump_every == 0:
                        pump(keep_warm)
                    if scores_fn:
                        scores_fn(kb)
                    for pkb in pvs:
                        pv_block(pkb)
                        if pkb == min(fd + 3, nkb - 1) and qn > 512:
                            normalize(0, 512)  # bank 0 closed early
                return u

            units = []
            if PAIR and pair_full and qn == 512 and nkb % 2 == 0:
                # all blocks in exp-pairs; pv's trail one scores event
                pend = []
                for kb in range(0, nkb, 2):
                    units.append(mk_unit(kb, scores_pair, pend))
                    pend = [kb, kb + 1]
                st_["pend"] = pend
            else:
                for kb in range(nkb):
                    units.append(mk_unit(
                        kb, scores_block, [kb - 1] if kb > 0 else []))
                st_["pend"] = [nkb - 1]

            def final():
                for pkb in st_["pend"]:
                    pv_block(pkb)
                normalize(512 if qn > 512 else 0, qn)

            return units + [final]

        def run_heads(seq):
            """Emit head unit-lists with one-unit cross-head lookahead:
            the next head's scores(0) goes out before this head's final
            pv, so the exp stream never drains at a head boundary."""
            for i, units in enumerate(seq):
                for u in units[1 if i else 0:-1]:
                    u()
                if i + 1 < len(seq):
                    seq[i + 1][0]()
                units[-1]()

        def o_proj(nt, mo, tail=False):
            """Output block: feat rows [128*mo ..+128), q [512*nt ..+512)."""
            ob = obp.tile([128, 512], F16, tag="ob", name="ob")
            ps = psum.tile([128, CH], F32, tag="qk", name="psob")
            for kb in range(2):
                nc.tensor.matmul(
                    ps[:, 0:512],
                    lhsT=wo_sb[kb][:, mo * 128:(mo + 1) * 128],
                    rhs=yT[kb][:, nt * 512:(nt + 1) * 512],
                    start=(kb == 0),
                    stop=(kb == 1),
                )
            # in the tail ACT is idle once the exps are done: it takes half
            # the psum bounces there
            if tail and mo % 2 == 1:
                nc.scalar.copy(ob[:], ps[:, 0:512])
            else:
                nc.vector.tensor_copy(ob[:], ps[:, 0:512])
            # keep DMA issue off the ACT queue while exps run; in the tail
            # ACT is free and a second queue doubles drain bandwidth
            ring = nc.scalar if (tail and mo % 2 == 1) else nc.sync
            ring.dma_start(
                out=outT_d[mo * 128:(mo + 1) * 128, nt * 512:(nt + 1) * 512],
                in_=ob[:])

        # ---- schedule ----
        # Emission order == per-engine queue order.  ACT (exp) is the
        # attention pacer now that scores+PV run fp8-DR/fp16 (PE 0.625
        # vs ACT 0.833 ns per score column), so all qkv/o_proj work is
        # pumped into the attention stream as ~0.4us filler units.
        # Dependency safety comes from emission order: a filler is
        # always emitted before the instruction that needs it.
        x_load(0)
        qk_g(0, 0, act_bounce=True)
        # k chunk 0 split: cols 0-256 (k-blocks 0/1) go out first so the
        # first exp fires ~2.5us earlier; the rest rides as at_blocks of
        # head 0 before scores(2) needs it.
        if FASTK:
            k0u = qk_units(0, 1, act_bounce=True,
                           splits=((0, 256), (256, CH)))
            for u in k0u[:6]:
                u()
            k0rest = k0u[6:]
        else:
            qk_g(0, 1, act_bounce=True)
            k0rest = []
        nc.scalar.dma_start(out=wv8[:], in_=wv8_d[:])
        nc.scalar.dma_start(out=dwv8[:], in_=dwv8_d[:])
        x_load(1)
        # tile A (q 0-512): needs only chunk 0 q/k; v blocks land as
        # at_blocks just before their pv consumer.  The chunk-1 qk and v
        # work soaks up the rope-latency wait before the first scores.
        filler_q.extend(qk_units(1, 0, act_bounce=True))
        filler_q.extend(qk_units(1, 1, act_bounce=True))
        for tb in range(4):
            filler_q.extend(v_units(1, tb, act_bounce=bool(AB23)))
        run_heads([
            attention_units(0, 0, 512, pump_every=PUMP_A, norm_act=True, at_blocks={
                1: k0rest + v_units(0, 0, act_bounce=bool(AB23)),
                2: v_units(0, 1, act_bounce=bool(AB23)),
                3: v_units(0, 2, act_bounce=bool(AB23))
                   + v_units(0, 3, act_bounce=bool(AB23)),
            }),
            attention_units(1, 0, 512, pump_every=PUMP_A, norm_act=True, pair_full=True),
            attention_units(2, 0, 512, pump_every=PUMP_A, norm_act=True, pair_full=True),
            attention_units(3, 0, 512, pump_every=PUMP_A, norm_act=True, pair_full=True),
        ])
        while filler_q:
            pump()
        # tile B (q 512-1024): fillers: chunks 2,3 qk.
        x_load(2)
        nc.scalar.dma_start(out=cc[:, 1024:2048], in_=ccT_d[:, 1024:2048])
        nc.scalar.dma_start(out=ss[:, 1024:2048], in_=ssT_d[:, 1024:2048])
        filler_q.extend(qk_units(2, 0, act_bounce=bool(AB23)))
        filler_q.extend(qk_units(2, 1, act_bounce=bool(AB23)))
        filler_q.append(lambda: x_load(3))
        filler_q.extend(qk_units(3, 0, act_bounce=bool(AB23)))
        filler_q.extend(qk_units(3, 1, act_bounce=bool(AB23)))
        run_heads([
            attention_units(0, 512, 512, pump_every=PUMP_B, pair_full=True, norm_act=True),
            attention_units(1, 512, 512, pump_every=PUMP_B, pair_full=True, norm_act=True),
            attention_units(2, 512, 512, pump_every=PUMP_B, pair_full=True, norm_act=True),
            attention_units(3, 512, 512, pump_every=PUMP_B, pair_full=True, norm_act=True),
        ])
        while filler_q:
            pump()
        # tail o_proj helpers (q 1024-2048): per mo one [128, 1024] ob
        # filled in two halves; nt=2 halves are emitted inside the last
        # head's attention as soon as its early psum bank is normalized.
        tail_obs = {}

        # single [128, 8x1024] tile for the q 1024-2048 output: per mo,
        # cols [1024mo, +512) = nt2 half, [+512, +1024) = nt3 half.
        # Keeping it one tile lets the tail flush as FOUR two-mo strided
        # DMAs instead of 16 narrow ones: each DMA costs ~625ns on the
        # shared HWDGE regardless of size, and that serialization was
        # the dominant term of the post-attention tail.
        obt_all = const.tile([128, 8 * 1024], F16, tag="obt", name="obt")

        def tail_half(nt, mo):
            # nt3 runs after the last attention: the st/y psum banks are
            # free, so cycling tags gives 5 buffers in rotation instead
            # of 2 and breaks the psum-recycle serialization of the tail.
            if nt == 3:
                tag, bufs = [("qk", 2), ("st", 2), ("y", 1)][mo % 3]
                ps = psum.tile([128, CH], F32, tag=tag, name="psob",
                               bufs=bufs)
            else:
                ps = psum.tile([128, CH], F32, tag="qk", name="psob")
            for kb in range(2):
                nc.tensor.matmul(
                    ps[:, 0:512],
                    lhsT=wo_sb[kb][:, mo * 128:(mo + 1) * 128],
                    rhs=yT[kb][:, nt * 512:(nt + 1) * 512],
                    start=(kb == 0),
                    stop=(kb == 1),
                )
            c0 = 1024 * mo + 512 * (nt - 2)
            if (nt == 3 and mo % 2 == 1) or (nt == 2 and mo >= 5):
                # true tail: exps are done, ACT is free
                nc.scalar.copy(obt_all[:, c0:c0 + 512], ps[:, 0:512])
            else:
                nc.vector.tensor_copy(obt_all[:, c0:c0 + 512],
                                      ps[:, 0:512])
            if nt == 2 and mo < 5:
                # nt2 halves flush per-mo while attention still runs
                nc.sync.dma_start(
                    out=outT_d[mo * 128:(mo + 1) * 128, 1024:1536],
                    in_=obt_all[:, c0:c0 + 512])
            elif nt == 2 and mo == 7:
                # the three post-attention nt2 halves go as ONE strided
                # DMA: each DMA costs ~625ns on the shared HWDGE, which
                # is the serializer of the final flush
                dst = outT_d.rearrange("(m p) t -> p m t", p=128)
                src_ = obt_all[:].rearrange("p (m t) -> p m t", t=1024)
                nc.sync.dma_start(out=dst[:, 5:8, 1024:1536],
                                  in_=src_[:, 5:8, 0:512])
            elif mo % 2 == 1:
                # nt3: one strided DMA flushes the (mo-1, mo) pair
                ring = nc.scalar if mo % 4 == 1 else nc.sync
                dst = outT_d.rearrange("(m p) t -> p m t", p=128)
                src = obt_all[:].rearrange("p (m t) -> p m t", t=1024)
                ring.dma_start(
                    out=dst[:, mo - 1:mo + 1, 1536:2048],
                    in_=src[:, mo - 1:mo + 1, 512:1024])

        # tile C (q 1024-2048): v chunks 2/3 are emitted at fixed blocks of
        # the first head (hard deps of pv blocks 8-15); o_proj of q 0-1024
        # is order-free filler spread across all four heads.
        filler_q.extend(
            (lambda nt=nt, mo=mo: o_proj(nt, mo))
            for nt in range(2) for mo in range(8))
        run_heads([
            attention_units(0, 1024, 1024, pump_every=PUMP_C0, at_blocks={
                5: v_units(2, 0) + v_units(2, 1),
                7: v_units(2, 2) + v_units(2, 3),
                9: v_units(3, 0) + v_units(3, 1),
                11: v_units(3, 2) + v_units(3, 3),
            }),
            attention_units(1, 1024, 1024, pump_every=PUMP_C,
                            keep_warm=KEEP_WARM),
            attention_units(2, 1024, 1024, pump_every=PUMP_C,
                            keep_warm=KEEP_WARM),
            attention_units(3, 1024, 1024, pump_every=PUMP_C,
                            keep_warm=KW3, at_blocks={
                13: [lambda mo=mo: tail_half(2, mo) for mo in range(2)],
                14: [lambda mo=mo: tail_half(2, mo) for mo in range(2, 5)],
            }),
        ])
        while filler_q:
            pump()
        # tail: keep-warm over the final-normalize wait so the o_proj
        # matmuls run at full p-state; the three remaining nt2 units are
        # emitted here (not as kb15 at_blocks) so their DVE copies queue
        # BEHIND head 3's final normalize instead of delaying it.
        dummy(3)
        tail_half(3, 0)
        tail_half(2, 5)
        tail_half(3, 1)
        tail_half(2, 6)
        tail_half(3, 2)
        tail_half(2, 7)
        for mo in range(3, 8):
            tail_half(3, mo)

    nc.compile()
    return nc


def _dr_pack(a, scale):
    """[1024, M] f32 -> fp8 (hi, residual) pair in DR layout [128, 8M].

    DR layout "p (i j m)": element (p, i, j, m) = a[256j + 128i + p, m], so
    one DoubleRow matmul contracts dim pairs (256j+p, 256j+128+p).
    `scale` lifts the values so both them and their residuals quantize in
    the e4m3 normal range; the device compensates (rope tables / v copy).
    """
    import ml_dtypes
    f8 = ml_dtypes.float8_e4m3
    a = a * scale
    M = a.shape[1]
    hi = a.astype(f8)
    lo = (a - hi.astype(np.float32)).astype(f8)
    out = []
    for t in (hi, lo):
        t = t.reshape(4, 2, 128, M).transpose(2, 1, 0, 3).reshape(128, 8 * M)
        out.append(np.ascontiguousarray(t))
    return out


def shard_inputs(x, freqs_cos, freqs_sin, Wqkv, Wo):
    """Build the 8 per-core input maps (host-side sharding)."""
    x = np.asarray(x, dtype=np.float32)
    Wqkv = np.asarray(Wqkv, dtype=np.float32)
    Wo = np.asarray(Wo, dtype=np.float32)
    # cos/sin tables transposed and replicated x4 (one copy per local
    # head), pre-divided by 1024 to undo the x16/x64 fp8 shipping scales
    ccT = np.tile(np.asarray(freqs_cos, dtype=np.float32).T, (4, 1)) / 1024.0
    ssT = np.tile(np.asarray(freqs_sin, dtype=np.float32).T, (4, 1)) / 1024.0
    ccT = np.ascontiguousarray(ccT).astype(np.float16)
    ssT = np.ascontiguousarray(ssT).astype(np.float16)
    x8s = [_dr_pack(x[b].T, 16.0) for b in range(B)]

    in_maps = []
    for c in range(NCORE):
        b, hg = c // 4, c % 4
        re = [np.arange(g * 64, g * 64 + 64, 2)
              for g in range(4 * hg, 4 * hg + 4)]
        im = [np.arange(g * 64 + 1, g * 64 + 64, 2)
              for g in range(4 * hg, 4 * hg + 4)]
        qcols = np.concatenate(re + im)
        kcols = C + qcols
        wqk8, dwqk8 = _dr_pack(Wqkv[:, np.concatenate([qcols, kcols])], 64.0)
        wv8, dwv8 = _dr_pack(
            Wqkv[:, 2 * C + hg * 256: 2 * C + hg * 256 + 256], 64.0)
        wo = np.ascontiguousarray(
            Wo[hg * 256: hg * 256 + 256, :]).astype(np.float16)
        in_maps.append({
            "x8": x8s[b][0], "dx8": x8s[b][1],
            "wqk8": wqk8, "dwqk8": dwqk8, "wv8": wv8, "dwv8": dwv8,
            "wo": wo, "ccT": ccT, "ssT": ssT,
        })
    return in_maps


_NC_CACHE = None


def _get_nc():
    global _NC_CACHE
    if _NC_CACHE is None:
        _NC_CACHE = build_nc()
    return _NC_CACHE


def run(inputs, trace=False):
    from concourse.bass_utils import run_bass_kernel_spmd

    nc = _get_nc()
    in_maps = shard_inputs(**inputs)
    res = run_bass_kernel_spmd(nc, in_maps, list(range(NCORE)), trace=trace)
    out = np.empty((B, T, C), dtype=np.float32)
    for b in range(B):
        acc = res.results[4 * b]["outT"].astype(np.float32)
        for c in range(4 * b + 1, 4 * b + 4):
            acc = acc + res.results[c]["outT"].astype(np.float32)
        out[b] = acc.T
    return out, res


def kernel(**inputs):
    out, _ = run(inputs)
    return out



# revision 5
# speedup vs baseline: 1.0189x; 1.0123x over previous
"""Causal self-attention (B=2, T=2048, C=1024, H=16, D=64) with RoPE on TRN2.

Sharding: 8 cores = 2 (batch) x 4 (head-groups of 4 heads), no
inter-core communication: each core computes qkv + rope + causal
attention + a row-parallel o_proj partial for its 4 heads; the host
sums the 4 fp16 partials per batch.

Precision: x and the q/k/v weights ship as fp8-e4m3 (hi, residual)
pairs, pre-scaled x16 / x64 so the residuals stay out of the e4m3
subnormal range; 3 DoubleRow accumulation chains (x8 w8 + x8 dw8 +
dx8 w8) reproduce the fp16 projection to ~0.2% at 3/8 the PE cost
(DoubleRow contracts 256 dims at 0.5 cycles/col).  The 2^-10 shipping
scale is undone for free in the rope tables (q/k) and the v psum
bounce.  Roped q/k are quantized to fp8 on the fly -- the rope sub/add
writes fp8 tiles directly in a DoubleRow-ready layout (head h in
partitions [32h,32h+32), re dims cols [0,T), im cols [T,2T)) -- so
QK^T runs as one fp8-DR matmul per (head, k-block) at 0.5 cycles/col:
2x the fp16 rate with only a single e4m3 quantization of q/k (rel err
1.65e-2 vs the 2e-2 gate).  P and V stay fp16 (peaked attention rows
pass V errors straight through, so V cannot be fp8), o_proj fp16.

Attention computes S^T = (K Q^T) per 128-row k-block so probabilities
exit exp() already transposed for P^T @ V.  V tiles carry 64 all-ones
columns: the PV matmul then yields psum rows 0-63 = y, rows 64-127 =
the softmax denominator replicated 64x (zero extra PE cycles since
matmul cost is free-dim only) -- normalization is one DVE reciprocal +
multiply per psum bank, emitted as soon as that bank closes.  No max
subtraction: logits are O(+-8) and exp fits fp16 range.

Scheduling: emission order is per-engine queue order.  exp on ACT
(0.833 ns/col) outweighs the fp8-DR scores + fp16 PV (0.625 ns/col),
so attention is ACT-paced and all projection/o_proj work is pumped
into the attention stream as ~0.4us PE filler units (chain-quarters
of the DR accumulations).  Heads are emitted with a one-unit
lookahead (next head's first scores before this head's last pv) so
the exp stream does not drain at head boundaries; rope/v psum bounces
ride on ACT only where exp leaves it slack.  DMA issues are kept off
queues whose sequencer is latency-critical (each issue holds its SEQ
~1.3us): the late-needed weight/table loads are emitted mid-schedule.
The o_proj tail is ordered so head 3's final normalize is not queued
behind tail copies on DVE, pads PE p-state over the normalize wait,
flushes q 1024-2048 as a few wide strided DMAs (each DMA costs
~625ns on the shared HWDGE regardless of size) and cycles its psum
tiles across the qk/st/y tags -- the attention banks are free by
then -- so the final o_proj chain is not serialized on the 2-buffer
psum rotation.
"""

import sys
import os

sys.path.insert(0, "/opt/trn_rl_repo")

import numpy as np
from contextlib import ExitStack

import concourse.bass as bass
import concourse.bacc as bacc
import concourse.mybir as mybir
import concourse.tile as tile

F32 = mybir.dt.float32
F16 = mybir.dt.float16
F8 = mybir.dt.float8e4
DR = mybir.MatmulPerfMode.DoubleRow

# problem constants (hardcoded per contract)
B, T, C, NH, D = 2, 2048, 1024, 16, 64
# schedule knobs (env-overridable for tuning sweeps)
PUMP_A = int(os.environ.get("K_PUMP_A", "2"))
PUMP_B = int(os.environ.get("K_PUMP_B", "3"))
PUMP_C0 = int(os.environ.get("K_PUMP_C0", "5"))
PUMP_C = int(os.environ.get("K_PUMP_C", "4"))
KEEP_WARM = int(os.environ.get("K_KEEP_WARM", "0"))
WARM_N = int(os.environ.get("K_WARM_N", "8"))
PAIR = int(os.environ.get("K_PAIR", "0"))
FASTK = int(os.environ.get("K_FASTK", "0"))
AB23 = int(os.environ.get("K_AB23", "0"))
SAPOOL = int(os.environ.get("K_SAPOOL", "0"))
VB01 = int(os.environ.get("K_VB01", "1"))
KW3 = int(os.environ.get("K_KW3", "0"))
HL = 4            # local heads per core
NCORE = 8
CH = 512          # qkv T-chunk width
NCHUNK = T // CH  # 4
SCALE = 1.0 / 8.0  # 1/sqrt(D)
NKB = T // 128    # 16 k-blocks


def _splits(a, b):
    """Split [a, b) at 512 boundaries (psum bank = 512 f32)."""
    out = []
    while a < b:
        nxt = min(b, (a // 512 + 1) * 512)
        out.append((a, nxt))
        a = nxt
    return out


def build_nc():
    nc = bacc.Bacc("TRN2", debug=False, num_devices=NCORE)

    # DR-packed fp8 operands: "p (i j c)" layout pairs contraction dims
    # 256j+128i+p so one DoubleRow matmul contracts 256 dims in 0.5
    # cycles/col.  x and the q/k/v weights ship as (hi, residual) fp8
    # pairs; 3 accumulation chains (x8 w8 + x8 dw8 + dx8 w8) reproduce
    # the fp16 product to ~0.2% at 3/8 the PE cost.
    x8_d = nc.dram_tensor("x8", [128, 8 * T], F8, kind="ExternalInput").ap()
    dx8_d = nc.dram_tensor("dx8", [128, 8 * T], F8, kind="ExternalInput").ap()
    wqk8_d = nc.dram_tensor("wqk8", [128, 4096], F8, kind="ExternalInput").ap()
    dwqk8_d = nc.dram_tensor("dwqk8", [128, 4096], F8,
                             kind="ExternalInput").ap()
    wv8_d = nc.dram_tensor("wv8", [128, 2048], F8, kind="ExternalInput").ap()
    dwv8_d = nc.dram_tensor("dwv8", [128, 2048], F8,
                            kind="ExternalInput").ap()
    wo_d = nc.dram_tensor("wo", [256, C], F16, kind="ExternalInput").ap()
    ccT_d = nc.dram_tensor("ccT", [128, T], F16, kind="ExternalInput").ap()
    ssT_d = nc.dram_tensor("ssT", [128, T], F16, kind="ExternalInput").ap()
    outT_d = nc.dram_tensor("outT", [C, T], F16, kind="ExternalOutput").ap()

    with tile.TileContext(nc) as tc, ExitStack() as ctx:
        const = ctx.enter_context(tc.tile_pool(name="const", bufs=1))
        rtp = ctx.enter_context(tc.tile_pool(name="rtp", bufs=2))
        pp = ctx.enter_context(tc.tile_pool(name="pp", bufs=5))
        nrm = ctx.enter_context(tc.tile_pool(name="nrm", bufs=3))
        obp = ctx.enter_context(tc.tile_pool(name="obp", bufs=4))
        psum = ctx.enter_context(tc.tile_pool(name="psum", bufs=2, space="PSUM"))

        # ---- persistent SBUF tensors ----
        # q/k weights (hi + residual) land first so the first real
        # matmuls can start while the rest is still in flight.
        wqk8 = const.tile([128, 4096], F8, tag="wqk8", name="wqk8")
        dwqk8 = const.tile([128, 4096], F8, tag="dwqk8", name="dwqk8")
        nc.scalar.dma_start(out=wqk8[:], in_=wqk8_d[:])
        nc.scalar.dma_start(out=dwqk8[:], in_=dwqk8_d[:])
        # views "p i (j m)": i-halves of the 256-dim contraction pairs
        wqk8_v = wqk8[:].rearrange("p (i jm) -> p i jm", i=2)
        dwqk8_v = dwqk8[:].rearrange("p (i jm) -> p i jm", i=2)

        # remaining input DMAs stay OFF the scalar queue: each DMA issue
        # holds its SEQ ~1.3us, and ACT's sequencer must be free for the
        # first rope bounces.  Early-needed tensors ride the sync queue;
        # late-needed ones go through Pool's SWDGE (Pool is idle early).
        cc = const.tile([128, T], F16, tag="cc")
        ss = const.tile([128, T], F16, tag="ss")
        nc.scalar.dma_start(out=cc[:, 0:1024], in_=ccT_d[:, 0:1024])
        nc.scalar.dma_start(out=ss[:, 0:1024], in_=ssT_d[:, 0:1024])

        wv8 = const.tile([128, 2048], F8, tag="wv8", name="wv8")
        dwv8 = const.tile([128, 2048], F8, tag="dwv8", name="dwv8")
        wv8_v = wv8[:].rearrange("p (i jd) -> p i jd", i=2)
        dwv8_v = dwv8[:].rearrange("p (i jd) -> p i jd", i=2)

        # PE warm-up: dependency-free matmuls that cover the input-DMA wait
        # and carry the tensor engine through its p-state ramp before the
        # first real matmul issues.
        warm = const.tile([128, 512], F16, tag="warm")
        nc.gpsimd.memset(warm[:], 0.0)
        pw = psum.tile([128, 512], F32, tag="qk", name="pw")
        for _ in range(WARM_N):
            nc.tensor.matmul(
                pw[:, 0:512], lhsT=warm[:, 0:128], rhs=warm[:],
                start=True, stop=True)

        wo_all = const.tile([128, 2 * C], F16, tag="wo", name="wo")
        nc.sync.dma_start(
            out=wo_all[:].rearrange("p (kt n) -> p kt n", n=C),
            in_=wo_d.rearrange("(kt p) n -> p kt n", p=128))
        wo_sb = [wo_all[:, kb * C:(kb + 1) * C] for kb in range(2)]

        # q8/k8: fp8e4 tiles [128, 2T].  head h lives in partitions
        # [32h, 32h+32); re dims at cols [0,T), im dims at cols [T,2T).
        # This matches the wqk column order (re of h0..h3, im of h0..h3),
        # so rope's sub/add write them directly -- no relayout copies --
        # and QK^T runs as one fp8 DoubleRow matmul per (head, k-block):
        # lhsT=[32,2,128] k-view, rhs=[32,2,n] q-view contract 64 dims.
        q8 = const.tile([128, 2 * T], F8, tag="q8", name="q8")
        k8 = const.tile([128, 2 * T], F8, tag="k8", name="k8")
        q8v = q8[:].rearrange("p (i t) -> p i t", i=2)
        k8v = k8[:].rearrange("p (i t) -> p i t", i=2)
        # v: [128 kpos, 4 heads x 16 blocks x 128] fp16; cols 0-63 of each
        # block = v dims, cols 64-127 = ones (denominator rows of PV psum)
        vT = const.tile([128, HL * NKB * 128], F16, tag="vT", name="vT")
        vT_v = vT[:].rearrange("p (h b c) -> p h b c", h=HL, b=NKB)
        nc.gpsimd.memset(vT_v[:, :, :, 64:128], 1.0)
        # y^T tiles: [128, T] x2 (4 heads x 64 dims)
        yT = [const.tile([128, T], F16, tag=f"yT{kb}", name=f"yT{kb}")
              for kb in range(2)]

        # x (hi + residual) fp8, persistent DR layout "p (i j t)"
        x8 = const.tile([128, 8 * T], F8, tag="x8", name="x8")
        dx8 = const.tile([128, 8 * T], F8, tag="dx8", name="dx8")
        x8_v = x8[:].rearrange("p (i jt) -> p i jt", i=2)
        dx8_v = dx8[:].rearrange("p (i jt) -> p i jt", i=2)
        x8_4v = x8[:].rearrange("p (i j t) -> p i j t", i=2, j=4)
        dx8_4v = dx8[:].rearrange("p (i j t) -> p i j t", i=2, j=4)
        x8_d4v = x8_d.rearrange("p (i j t) -> p i j t", i=2, j=4)
        dx8_d4v = dx8_d.rearrange("p (i j t) -> p i j t", i=2, j=4)

        def x_load(n, ways=2):
            t0 = n * CH
            nc.sync.dma_start(out=x8_4v[:, :, :, t0:t0 + CH],
                              in_=x8_d4v[:, :, :, t0:t0 + CH])
            nc.sync.dma_start(out=dx8_4v[:, :, :, t0:t0 + CH],
                              in_=dx8_d4v[:, :, :, t0:t0 + CH])

        CHAINS_QK = ((x8_v, wqk8_v), (x8_v, dwqk8_v), (dx8_v, wqk8_v))
        CHAINS_V = ((x8_v, wv8_v), (x8_v, dwv8_v), (dx8_v, wv8_v))

        def qk_units(n, g, act_bounce=False, splits=((0, CH),),
                     tags=(("qk", 2), ("qk", 2))):
            """q (g=0) / k (g=1) projection + rope for chunk n as a list
            of ~0.4us PE filler units (chain-quarters of the pre/pim psum
            accumulations; rope rides on the last one).  `splits` breaks
            the chunk into column ranges emitted as separate unit groups
            (used to fast-path the first k-block before the first exp)."""
            t0 = n * CH

            def mk(a, b):
                st = {}

                def mm(name, m, ci):
                    def u():
                        if ci == 0:
                            tg, bf = tags[0 if name == "psre" else 1]
                            st[name] = psum.tile([128, CH], F32, tag=tg,
                                                 name=name, bufs=bf)
                        xs, ws = CHAINS_QK[ci]
                        for j in range(4):
                            nc.tensor.matmul(
                                st[name][:, a:b],
                                lhsT=ws[:, :, j * 512 + m * 128:
                                        j * 512 + (m + 1) * 128],
                                rhs=xs[:, :, j * T + t0 + a:j * T + t0 + b],
                                start=(ci == 0 and j == 0),
                                stop=(ci == 2 and j == 3),
                                perf_mode=DR,
                            )
                    return u

                def rope():
                    mul = mybir.AluOpType.mult
                    sub = mybir.AluOpType.subtract
                    add = mybir.AluOpType.add
                    pre, pim = st["psre"], st["psim"]
                    w = b - a
                    ccn = cc[:, t0 + a:t0 + b]
                    ssn = ss[:, t0 + a:t0 + b]
                    t1 = rtp.tile([128, CH], F16, tag="t1")
                    t2 = rtp.tile([128, CH], F16, tag="t2")
                    t3 = rtp.tile([128, CH], F16, tag="t3")
                    t4 = rtp.tile([128, CH], F16, tag="t4")
                    # psum -> fp16 bounce: ACT while it still has slack
                    # (early chunks), DVE once exp saturates ACT; the
                    # rope arithmetic runs in DVE 4x fp16 mode.
                    preb = rtp.tile([128, CH], F16, tag="preb")
                    pimb = rtp.tile([128, CH], F16, tag="pimb")
                    if act_bounce:
                        nc.scalar.copy(preb[:, 0:w], pre[:, a:b])
                        nc.scalar.copy(pimb[:, 0:w], pim[:, a:b])
                    else:
                        nc.vector.tensor_copy(preb[:, 0:w], pre[:, a:b])
                        nc.vector.tensor_copy(pimb[:, 0:w], pim[:, a:b])
                    preb_, pimb_ = preb[:, 0:w], pimb[:, 0:w]
                    nc.vector.tensor_tensor(t1[:, 0:w], preb_, ccn, mul)
                    nc.vector.tensor_tensor(t2[:, 0:w], pimb_, ssn, mul)
                    nc.vector.tensor_tensor(t3[:, 0:w], preb_, ssn, mul)
                    nc.vector.tensor_tensor(t4[:, 0:w], pimb_, ccn, mul)
                    # sub/add write the fp8 q8/k8 tiles directly (re
                    # half / im half): no relayout copies needed.  Pool
                    # (mostly idle) can take them to unload DVE.
                    eng = nc.gpsimd if SAPOOL else nc.vector
                    o = q8 if g == 0 else k8
                    eng.tensor_tensor(
                        o[:, t0 + a:t0 + b], t1[:, 0:w], t2[:, 0:w], sub)
                    eng.tensor_tensor(
                        o[:, T + t0 + a:T + t0 + b], t3[:, 0:w],
                        t4[:, 0:w], add)

                last = mm("psim", 2 * g + 1, 2)
                return [mm("psre", 2 * g, 0), mm("psre", 2 * g, 1),
                        mm("psre", 2 * g, 2), mm("psim", 2 * g + 1, 0),
                        mm("psim", 2 * g + 1, 1),
                        lambda: (last(), rope())]

            out = []
            for (a, b) in splits:
                out.extend(mk(a, b))
            return out

        def qk_g(n, g, act_bounce=False, splits=((0, CH),)):
            for u in qk_units(n, g, act_bounce, splits):
                u()

        def v_units(n, tb, act_bounce=False):
            """v projection for 128-row block tb of chunk n: 3 chain
            units of ~0.2us; the psum->fp16 bounce rides on the last."""
            t0 = n * CH + tb * 128
            st = {}

            def mm(ci):
                def u():
                    if ci == 0:
                        st["psv"] = psum.tile([128, CH], F32, tag="qk",
                                              name="psv")
                    xs, ws = CHAINS_V[ci]
                    for j in range(4):
                        nc.tensor.matmul(
                            st["psv"][:, 0:256],
                            lhsT=xs[:, :, j * T + t0:j * T + t0 + 128],
                            rhs=ws[:, :, j * 256:(j + 1) * 256],
                            start=(ci == 0 and j == 0),
                            stop=(ci == 2 and j == 3),
                            perf_mode=DR,
                        )
                    if ci == 2:
                        blk = 4 * n + tb
                        dst = vT_v[:, :, blk, 0:64]
                        src = st["psv"][:, 0:256].rearrange(
                            "p (h d) -> p h d", d=64)
                        # gpsimd cannot read PSUM; the bounce (psum holds
                        # 1024*v -- x/w ship pre-scaled x16/x64 to keep
                        # fp8 residuals out of the e4m3 subnormal range)
                        # folds in the 2^-10 fix free.  ACT takes it in
                        # the A/B phases where exp leaves it slack.
                        if act_bounce:
                            nc.scalar.activation(
                                dst, src,
                                mybir.ActivationFunctionType.Copy,
                                scale=1.0 / 1024.0)
                        else:
                            nc.vector.tensor_scalar_mul(
                                dst, src, 1.0 / 1024.0)
                return u

            return [mm(0), mm(1), mm(2)]

        def v_tb(n, tb):
            for u in v_units(n, tb):
                u()

        filler_q = []

        def dummy(n=2):
            # keep-warm matmuls: PE p-state drops 2x after an idle gap and
            # needs 3us of continuous execution to recover; padding known
            # exp-bound stretches keeps the real matmuls at full clock.
            pd = psum.tile([128, 512], F32, tag="qk", name="pd")
            for _ in range(n):
                nc.tensor.matmul(
                    pd[:, 0:512], lhsT=warm[:, 0:128], rhs=warm[:],
                    start=True, stop=True)

        def pump(keep_warm=0):
            if filler_q:
                filler_q.pop(0)()
            elif keep_warm:
                dummy(keep_warm)

        def attention_units(h, q0, qn, pump_every=0, at_blocks=None,
                            keep_warm=0, pair_full=False, norm_act=False):
            """One head, q-cols [q0, q0+qn) as a list of emission units.

            units[kb] = at_blocks/pump + scores(kb) + pv(kb-1); the last
            unit is pv(nkb-1) + final normalize.  The scheduler emits the
            NEXT head's units[0] just before this head's last unit: that
            head-boundary scores runs on PE while the last exp still
            streams on ACT, so ACT never drains between heads.  (The pst
            double-buffer is free at exactly that point: exp(nkb-2) has
            been consumed by the preceding pv.)

            at_blocks: {kb: [unit, ...]} -- mandatory work units emitted
            just before scores_block(kb); used for dependencies of later
            pv_blocks (e.g. v tiles), unlike best-effort pump fillers.
            """
            qv = q8v[32 * h:32 * h + 32]
            kv = k8v[32 * h:32 * h + 32]
            r0 = 64 * (h % 2)
            nkb = (q0 + qn) // 128
            fd = q0 // 128  # first diagonal block
            st_ = {}
            Ps = {}

            def block_off(kb):
                return 128 * (kb - fd) if kb >= fd else 0

            def scores_block(kb):
                if kb == 0:
                    st_["psy"] = psum.tile([128, qn], F32, tag="y",
                                           name="psy", bufs=1)
                off = block_off(kb)
                pst = psum.tile([128, qn], F32, tag="st", name="pst")
                for (a, b) in _splits(off, qn):
                    nc.tensor.matmul(
                        pst[:, a:b],
                        lhsT=kv[:, :, kb * 128:(kb + 1) * 128],
                        rhs=qv[:, :, q0 + a:q0 + b],
                        start=True,
                        stop=True,
                        perf_mode=DR,
                        tile_position=(32 * h, 0),
                    )
                P = pp.tile([128, 1024], F16, tag="P")
                Ps[kb] = (P, 0)
                nc.scalar.activation(
                    P[:, off:qn], pst[:, off:qn],
                    mybir.ActivationFunctionType.Exp, scale=SCALE)
                if kb >= fd:
                    # zero strictly-upper triangle of the leading 128 cols
                    nc.gpsimd.affine_select(
                        out=P[:, off:off + 128],
                        in_=P[:, off:off + 128],
                        compare_op=mybir.AluOpType.is_ge,
                        fill=0.0,
                        base=0,
                        pattern=[[1, 128]],
                        channel_multiplier=-1,
                    )

            def scores_pair(kb):
                # two qn=512 k-blocks share one [128,1024] pst / P pair:
                # a single exp call covers both, halving the ACT per-call
                # access-latency overhead and doubling the exp lookahead
                # the pst double-buffer can hold.  For diagonal blocks the
                # gap columns [512, 512+off1) are dead: matmul start
                # zeroes the bank region, exp of them is masked/unread
                # (pv reads only [off:512] of each half).
                if kb == 0:
                    st_["psy"] = psum.tile([128, qn], F32, tag="y",
                                           name="psy", bufs=1)
                pst = psum.tile([128, 1024], F32, tag="st", name="pst")
                for sub in (0, 1):
                    off = block_off(kb + sub)
                    nc.tensor.matmul(
                        pst[:, 512 * sub + off:512 * sub + 512],
                        lhsT=kv[:, :, (kb + sub) * 128:(kb + sub + 1) * 128],
                        rhs=qv[:, :, q0 + off:q0 + 512],
                        start=True,
                        stop=True,
                        perf_mode=DR,
                        tile_position=(32 * h, 0),
                    )
                off0 = block_off(kb)
                P = pp.tile([128, 1024], F16, tag="P")
                Ps[kb] = (P, 0)
                Ps[kb + 1] = (P, 512)
                nc.scalar.activation(
                    P[:, off0:1024], pst[:, off0:1024],
                    mybir.ActivationFunctionType.Exp, scale=SCALE)
                for sub in (0, 1):
                    off = block_off(kb + sub)
                    if kb + sub >= fd:
                        nc.gpsimd.affine_select(
                            out=P[:, 512 * sub + off:512 * sub + off + 128],
                            in_=P[:, 512 * sub + off:512 * sub + off + 128],
                            compare_op=mybir.AluOpType.is_ge,
                            fill=0.0,
                            base=0,
                            pattern=[[1, 128]],
                            channel_multiplier=-1,
                        )

            def pv_block(kb):
                off = block_off(kb)
                P, coff = Ps.pop(kb)
                psy = st_["psy"]
                for (a, b) in _splits(off, qn):
                    # last writer of the psum bank holding col a is diag
                    # block fd + 4*(a//512) + 3
                    kb_stop = min(fd + 4 * (a // 512) + 3, nkb - 1)
                    nc.tensor.matmul(
                        psy[:, a:b],
                        lhsT=vT_v[:, h, kb, :],
                        rhs=P[:, coff + a:coff + b],
                        start=(kb == 0),
                        stop=(kb == kb_stop),
                    )

            def normalize(a, b):
                # psum rows 64-127 all hold the denominator row l (ones
                # cols of vT): reciprocal + one multiply per psum bank,
                # emitted as soon as that bank's accumulation closes.
                # (walrus forbids two PSUM reads in one TensorTensor, so
                # a single divide is not possible.)
                psy = st_["psy"]
                rlb = nrm.tile([64, 512], F32, tag="rlb")
                nc.vector.reciprocal(rlb[:, 0:b - a], psy[64:128, a:b])
                nc.vector.tensor_tensor(
                    yT[h // 2][r0:r0 + 64, q0 + a:q0 + b],
                    psy[0:64, a:b], rlb[:, 0:b - a], mybir.AluOpType.mult)

            def mk_unit(kb, scores_fn, pvs):
                def u():
                    if at_blocks and kb in at_blocks:
                        for ab in at_blocks[kb]:
                            ab()
                    if pump_every and kb % pump_every == 0:
                        pump(keep_warm)
                    if scores_fn:
                        scores_fn(kb)
                    for pkb in pvs:
                        pv_block(pkb)
                        if pkb == min(fd + 3, nkb - 1) and qn > 512:
                            normalize(0, 512)  # bank 0 closed early
                return u

            units = []
            if PAIR and pair_full and qn == 512 and nkb % 2 == 0:
                # all blocks in exp-pairs; pv's trail one scores event
                pend = []
                for kb in range(0, nkb, 2):
                    units.append(mk_unit(kb, scores_pair, pend))
                    pend = [kb, kb + 1]
                st_["pend"] = pend
            else:
                for kb in range(nkb):
                    units.append(mk_unit(
                        kb, scores_block, [kb - 1] if kb > 0 else []))
                st_["pend"] = [nkb - 1]

            def final():
                for pkb in st_["pend"]:
                    pv_block(pkb)
                normalize(512 if qn > 512 else 0, qn)

            return units + [final]

        def run_heads(seq):
            """Emit head unit-lists with one-unit cross-head lookahead:
            the next head's scores(0) goes out before this head's final
            pv, so the exp stream never drains at a head boundary."""
            for i, units in enumerate(seq):
                for u in units[1 if i else 0:-1]:
                    u()
                if i + 1 < len(seq):
                    seq[i + 1][0]()
                units[-1]()

        def o_proj(nt, mo, tail=False):
            """Output block: feat rows [128*mo ..+128), q [512*nt ..+512)."""
            ob = obp.tile([128, 512], F16, tag="ob", name="ob")
            ps = psum.tile([128, CH], F32, tag="qk", name="psob")
            for kb in range(2):
                nc.tensor.matmul(
                    ps[:, 0:512],
                    lhsT=wo_sb[kb][:, mo * 128:(mo + 1) * 128],
                    rhs=yT[kb][:, nt * 512:(nt + 1) * 512],
                    start=(kb == 0),
                    stop=(kb == 1),
                )
            # in the tail ACT is idle once the exps are done: it takes half
            # the psum bounces there
            if tail and mo % 2 == 1:
                nc.scalar.copy(ob[:], ps[:, 0:512])
            else:
                nc.vector.tensor_copy(ob[:], ps[:, 0:512])
            # keep DMA issue off the ACT queue while exps run; in the tail
            # ACT is free and a second queue doubles drain bandwidth
            ring = nc.scalar if (tail and mo % 2 == 1) else nc.sync
            ring.dma_start(
                out=outT_d[mo * 128:(mo + 1) * 128, nt * 512:(nt + 1) * 512],
                in_=ob[:])

        # ---- schedule ----
        # Emission order == per-engine queue order.  ACT (exp) is the
        # attention pacer now that scores+PV run fp8-DR/fp16 (PE 0.625
        # vs ACT 0.833 ns per score column), so all qkv/o_proj work is
        # pumped into the attention stream as ~0.4us filler units.
        # Dependency safety comes from emission order: a filler is
        # always emitted before the instruction that needs it.
        x_load(0)
        BR = int(os.environ.get("K_BRIDGE", "0"))
        if BR:
            # bridge the mid-chain DMA waits (dwqk8/dx8 arrival) with pad
            # matmuls on the still-live warmup psum so the PE p-state
            # ramp survives into the first real chains
            q0u = qk_units(0, 0, act_bounce=True)
            q0u[0]()
            for _ in range(BR):
                nc.tensor.matmul(pw[:, 0:512], lhsT=warm[:, 0:128],
                                 rhs=warm[:], start=True, stop=True)
            q0u[1]()
            for _ in range(BR // 2):
                nc.tensor.matmul(pw[:, 0:512], lhsT=warm[:, 0:128],
                                 rhs=warm[:], start=True, stop=True)
            for u in q0u[2:]:
                u()
        else:
            qk_g(0, 0, act_bounce=True)
        qk_g(0, 1, act_bounce=True)
        k0rest = []
        nc.scalar.dma_start(out=wv8[:], in_=wv8_d[:])
        nc.scalar.dma_start(out=dwv8[:], in_=dwv8_d[:])
        x_load(1)
        # tile A (q 0-512): needs only chunk 0 q/k; v blocks land as
        # at_blocks just before their pv consumer.  The chunk-1 qk and v
        # work soaks up the rope-latency wait before the first scores.
        filler_q.extend(qk_units(1, 0, act_bounce=True))
        filler_q.extend(qk_units(1, 1, act_bounce=True))
        for tb in range(4):
            filler_q.extend(v_units(1, tb, act_bounce=bool(VB01)))
        run_heads([
            attention_units(0, 0, 512, pump_every=PUMP_A, norm_act=True, at_blocks={
                1: k0rest + v_units(0, 0, act_bounce=bool(VB01)),
                2: v_units(0, 1, act_bounce=bool(VB01)),
                3: v_units(0, 2, act_bounce=bool(VB01))
                   + v_units(0, 3, act_bounce=bool(VB01)),
            }),
            attention_units(1, 0, 512, pump_every=PUMP_A, norm_act=True, pair_full=True),
            attention_units(2, 0, 512, pump_every=PUMP_A, norm_act=True, pair_full=True),
            attention_units(3, 0, 512, pump_every=PUMP_A, norm_act=True, pair_full=True),
        ])
        while filler_q:
            pump()
        # tile B (q 512-1024): fillers: chunks 2,3 qk.
        x_load(2)
        x_load(3)
        nc.scalar.dma_start(out=cc[:, 1024:2048], in_=ccT_d[:, 1024:2048])
        nc.scalar.dma_start(out=ss[:, 1024:2048], in_=ssT_d[:, 1024:2048])
        filler_q.extend(qk_units(2, 0, act_bounce=bool(AB23)))
        filler_q.extend(qk_units(2, 1, act_bounce=bool(AB23)))
        filler_q.extend(qk_units(3, 0, act_bounce=bool(AB23)))
        filler_q.extend(qk_units(3, 1, act_bounce=bool(AB23)))
        run_heads([
            attention_units(0, 512, 512, pump_every=PUMP_B, pair_full=True, norm_act=True),
            attention_units(1, 512, 512, pump_every=PUMP_B, pair_full=True, norm_act=True),
            attention_units(2, 512, 512, pump_every=PUMP_B, pair_full=True, norm_act=True),
            attention_units(3, 512, 512, pump_every=PUMP_B, pair_full=True, norm_act=True),
        ])
        while filler_q:
            pump()
        # tail o_proj helpers (q 1024-2048): per mo one [128, 1024] ob
        # filled in two halves; nt=2 halves are emitted inside the last
        # head's attention as soon as its early psum bank is normalized.
        tail_obs = {}

        # single [128, 8x1024] tile for the q 1024-2048 output: per mo,
        # cols [1024mo, +512) = nt2 half, [+512, +1024) = nt3 half.
        # Keeping it one tile lets the tail flush as FOUR two-mo strided
        # DMAs instead of 16 narrow ones: each DMA costs ~625ns on the
        # shared HWDGE regardless of size, and that serialization was
        # the dominant term of the post-attention tail.
        obt_all = const.tile([128, 8 * 1024], F16, tag="obt", name="obt")

        def tail_half(nt, mo):
            # nt3 runs after the last attention: the st/y psum banks are
            # free, so cycling tags gives 5 buffers in rotation instead
            # of 2 and breaks the psum-recycle serialization of the tail.
            if nt == 3:
                tag, bufs = [("qk", 2), ("st", 2), ("y", 1)][mo % 3]
                ps = psum.tile([128, CH], F32, tag=tag, name="psob",
                               bufs=bufs)
            else:
                ps = psum.tile([128, CH], F32, tag="qk", name="psob")
            for kb in range(2):
                nc.tensor.matmul(
                    ps[:, 0:512],
                    lhsT=wo_sb[kb][:, mo * 128:(mo + 1) * 128],
                    rhs=yT[kb][:, nt * 512:(nt + 1) * 512],
                    start=(kb == 0),
                    stop=(kb == 1),
                )
            c0 = 1024 * mo + 512 * (nt - 2)
            if (nt == 3 and mo % 2 == 1) or (nt == 2 and mo >= 5):
                # true tail: exps are done, ACT is free
                nc.scalar.copy(obt_all[:, c0:c0 + 512], ps[:, 0:512])
            else:
                nc.vector.tensor_copy(obt_all[:, c0:c0 + 512],
                                      ps[:, 0:512])
            if nt == 2 and mo < 5:
                # nt2 halves flush per-mo while attention still runs
                nc.sync.dma_start(
                    out=outT_d[mo * 128:(mo + 1) * 128, 1024:1536],
                    in_=obt_all[:, c0:c0 + 512])
            elif nt == 2 and mo == 7:
                # the three post-attention nt2 halves go as ONE strided
                # DMA: each DMA costs ~625ns on the shared HWDGE, which
                # is the serializer of the final flush
                dst = outT_d.rearrange("(m p) t -> p m t", p=128)
                src_ = obt_all[:].rearrange("p (m t) -> p m t", t=1024)
                nc.sync.dma_start(out=dst[:, 5:8, 1024:1536],
                                  in_=src_[:, 5:8, 0:512])
            elif mo % 2 == 1:
                # nt3: one strided DMA flushes the (mo-1, mo) pair
                ring = nc.scalar if mo % 4 == 1 else nc.sync
                dst = outT_d.rearrange("(m p) t -> p m t", p=128)
                src = obt_all[:].rearrange("p (m t) -> p m t", t=1024)
                ring.dma_start(
                    out=dst[:, mo - 1:mo + 1, 1536:2048],
                    in_=src[:, mo - 1:mo + 1, 512:1024])

        # tile C (q 1024-2048): v chunks 2/3 are emitted at fixed blocks of
        # the first head (hard deps of pv blocks 8-15); o_proj of q 0-1024
        # is order-free filler spread across all four heads.
        filler_q.extend(
            (lambda nt=nt, mo=mo: o_proj(nt, mo))
            for nt in range(2) for mo in range(8))
        run_heads([
            attention_units(0, 1024, 1024, pump_every=PUMP_C0, at_blocks={
                5: v_units(2, 0) + v_units(2, 1),
                7: v_units(2, 2) + v_units(2, 3),
                9: v_units(3, 0) + v_units(3, 1),
                11: v_units(3, 2) + v_units(3, 3),
            }),
            attention_units(1, 1024, 1024, pump_every=PUMP_C,
                            keep_warm=KEEP_WARM),
            attention_units(2, 1024, 1024, pump_every=PUMP_C,
                            keep_warm=KEEP_WARM),
            attention_units(3, 1024, 1024, pump_every=PUMP_C,
                            keep_warm=KW3, at_blocks={
                13: [lambda mo=mo: tail_half(2, mo) for mo in range(2)],
                14: [lambda mo=mo: tail_half(2, mo) for mo in range(2, 5)],
            }),
        ])
        while filler_q:
            pump()
        # tail: keep-warm over the final-normalize wait so the o_proj
        # matmuls run at full p-state; the three remaining nt2 units are
        # emitted here (not as kb15 at_blocks) so their DVE copies queue
        # BEHIND head 3's final normalize instead of delaying it.
        if int(os.environ.get('K_TDUM', '3')):
            dummy(int(os.environ.get('K_TDUM', '3')))
        tail_half(3, 0)
        tail_half(2, 5)
        tail_half(3, 1)
        tail_half(2, 6)
        tail_half(3, 2)
        tail_half(2, 7)
        for mo in range(3, 8):
            tail_half(3, mo)

    nc.compile()
    return nc


def _dr_pack(a, scale):
    """[1024, M] f32 -> fp8 (hi, residual) pair in DR layout [128, 8M].

    DR layout "p (i j m)": element (p, i, j, m) = a[256j + 128i + p, m], so
    one DoubleRow matmul contracts dim pairs (256j+p, 256j+128+p).
    `scale` lifts the values so both them and their residuals quantize in
    the e4m3 normal range; the device compensates (rope tables / v copy).
    """
    import ml_dtypes
    f8 = ml_dtypes.float8_e4m3
    a = a * scale
    M = a.shape[1]
    hi = a.astype(f8)
    lo = (a - hi.astype(np.float32)).astype(f8)
    out = []
    for t in (hi, lo):
        t = t.reshape(4, 2, 128, M).transpose(2, 1, 0, 3).reshape(128, 8 * M)
        out.append(np.ascontiguousarray(t))
    return out


def shard_inputs(x, freqs_cos, freqs_sin, Wqkv, Wo):
    """Build the 8 per-core input maps (host-side sharding)."""
    x = np.asarray(x, dtype=np.float32)
    Wqkv = np.asarray(Wqkv, dtype=np.float32)
    Wo = np.asarray(Wo, dtype=np.float32)
    # cos/sin tables transposed and replicated x4 (one copy per local
    # head), pre-divided by 1024 to undo the x16/x64 fp8 shipping scales
    ccT = np.tile(np.asarray(freqs_cos, dtype=np.float32).T, (4, 1)) / 1024.0
    ssT = np.tile(np.asarray(freqs_sin, dtype=np.float32).T, (4, 1)) / 1024.0
    ccT = np.ascontiguousarray(ccT).astype(np.float16)
    ssT = np.ascontiguousarray(ssT).astype(np.float16)
    x8s = [_dr_pack(x[b].T, 16.0) for b in range(B)]

    in_maps = []
    for c in range(NCORE):
        b, hg = c // 4, c % 4
        re = [np.arange(g * 64, g * 64 + 64, 2)
              for g in range(4 * hg, 4 * hg + 4)]
        im = [np.arange(g * 64 + 1, g * 64 + 64, 2)
              for g in range(4 * hg, 4 * hg + 4)]
        qcols = np.concatenate(re + im)
        kcols = C + qcols
        wqk8, dwqk8 = _dr_pack(Wqkv[:, np.concatenate([qcols, kcols])], 64.0)
        wv8, dwv8 = _dr_pack(
            Wqkv[:, 2 * C + hg * 256: 2 * C + hg * 256 + 256], 64.0)
        wo = np.ascontiguousarray(
            Wo[hg * 256: hg * 256 + 256, :]).astype(np.float16)
        in_maps.append({
            "x8": x8s[b][0], "dx8": x8s[b][1],
            "wqk8": wqk8, "dwqk8": dwqk8, "wv8": wv8, "dwv8": dwv8,
            "wo": wo, "ccT": ccT, "ssT": ssT,
        })
    return in_maps


_NC_CACHE = None


def _get_nc():
    global _NC_CACHE
    if _NC_CACHE is None:
        _NC_CACHE = build_nc()
    return _NC_CACHE


def run(inputs, trace=False):
    from concourse.bass_utils import run_bass_kernel_spmd

    nc = _get_nc()
    in_maps = shard_inputs(**inputs)
    res = run_bass_kernel_spmd(nc, in_maps, list(range(NCORE)), trace=trace)
    out = np.empty((B, T, C), dtype=np.float32)
    for b in range(B):
        acc = res.results[4 * b]["outT"].astype(np.float32)
        for c in range(4 * b + 1, 4 * b + 4):
            acc = acc + res.results[c]["outT"].astype(np.float32)
        out[b] = acc.T
    return out, res


def kernel(**inputs):
    out, _ = run(inputs)
    return out



# revision 6
# speedup vs baseline: 1.0220x; 1.0030x over previous
"""Causal self-attention (B=2, T=2048, C=1024, H=16, D=64) with RoPE on TRN2.

Sharding: 8 cores = 2 (batch) x 4 (head-groups of 4 heads), no
inter-core communication: each core computes qkv + rope + causal
attention + a row-parallel o_proj partial for its 4 heads; the host
sums the 4 fp16 partials per batch.

Precision: x and the q/k/v weights ship as fp8-e4m3 (hi, residual)
pairs, pre-scaled x16 / x64 so the residuals stay out of the e4m3
subnormal range; 3 DoubleRow accumulation chains (x8 w8 + x8 dw8 +
dx8 w8) reproduce the fp16 projection to ~0.2% at 3/8 the PE cost
(DoubleRow contracts 256 dims at 0.5 cycles/col).  The 2^-10 shipping
scale is undone for free in the rope tables (q/k) and the v psum
bounce.  Roped q/k are quantized to fp8 on the fly -- the rope sub/add
writes fp8 tiles directly in a DoubleRow-ready layout (head h in
partitions [32h,32h+32), re dims cols [0,T), im cols [T,2T)) -- so
QK^T runs as one fp8-DR matmul per (head, k-block) at 0.5 cycles/col:
2x the fp16 rate with only a single e4m3 quantization of q/k (rel err
1.65e-2 vs the 2e-2 gate).  P and V stay fp16 (peaked attention rows
pass V errors straight through, so V cannot be fp8), o_proj fp16.

Attention computes S^T = (K Q^T) per 128-row k-block so probabilities
exit exp() already transposed for P^T @ V.  V tiles carry 64 all-ones
columns: the PV matmul then yields psum rows 0-63 = y, rows 64-127 =
the softmax denominator replicated 64x (zero extra PE cycles since
matmul cost is free-dim only) -- normalization is one DVE reciprocal +
multiply per psum bank, emitted as soon as that bank closes.  No max
subtraction: logits are O(+-8) and exp fits fp16 range.

Scheduling: emission order is per-engine queue order.  exp on ACT
(0.833 ns/col) outweighs the fp8-DR scores + fp16 PV (0.625 ns/col),
so attention is ACT-paced and all projection/o_proj work is pumped
into the attention stream as ~0.4us PE filler units (chain-quarters
of the DR accumulations).  Heads are emitted with a one-unit
lookahead (next head's first scores before this head's last pv) so
the exp stream does not drain at head boundaries; rope/v psum bounces
ride on ACT only where exp leaves it slack.  DMA issues are kept off
queues whose sequencer is latency-critical (each issue holds its SEQ
~1.3us): the late-needed weight/table loads are emitted mid-schedule.
The o_proj tail is ordered so head 3's final normalize is not queued
behind tail copies on DVE, pads PE p-state over the normalize wait,
flushes q 1024-2048 as a few wide strided DMAs (each DMA costs
~625ns on the shared HWDGE regardless of size) and cycles its psum
tiles across the qk/st/y tags -- the attention banks are free by
then -- so the final o_proj chain is not serialized on the 2-buffer
psum rotation.
"""

import sys
import os

sys.path.insert(0, "/opt/trn_rl_repo")

import numpy as np
from contextlib import ExitStack

import concourse.bass as bass
import concourse.bacc as bacc
import concourse.mybir as mybir
import concourse.tile as tile

F32 = mybir.dt.float32
F16 = mybir.dt.float16
F8 = mybir.dt.float8e4
DR = mybir.MatmulPerfMode.DoubleRow

# problem constants (hardcoded per contract)
B, T, C, NH, D = 2, 2048, 1024, 16, 64
# schedule knobs (env-overridable for tuning sweeps)
PUMP_A = int(os.environ.get("K_PUMP_A", "2"))
PUMP_B = int(os.environ.get("K_PUMP_B", "3"))
PUMP_C0 = int(os.environ.get("K_PUMP_C0", "5"))
PUMP_C = int(os.environ.get("K_PUMP_C", "4"))
KEEP_WARM = int(os.environ.get("K_KEEP_WARM", "0"))
WARM_N = int(os.environ.get("K_WARM_N", "8"))
PAIR = int(os.environ.get("K_PAIR", "0"))
FASTK = int(os.environ.get("K_FASTK", "0"))
AB23 = int(os.environ.get("K_AB23", "0"))
SAPOOL = int(os.environ.get("K_SAPOOL", "0"))
VB01 = int(os.environ.get("K_VB01", "1"))
RB0 = int(os.environ.get("K_RB0", "1"))
RB1 = int(os.environ.get("K_RB1", "0"))
KW3 = int(os.environ.get("K_KW3", "0"))
HL = 4            # local heads per core
NCORE = 8
CH = 512          # qkv T-chunk width
NCHUNK = T // CH  # 4
SCALE = 1.0 / 8.0  # 1/sqrt(D)
NKB = T // 128    # 16 k-blocks


def _splits(a, b):
    """Split [a, b) at 512 boundaries (psum bank = 512 f32)."""
    out = []
    while a < b:
        nxt = min(b, (a // 512 + 1) * 512)
        out.append((a, nxt))
        a = nxt
    return out


def build_nc():
    nc = bacc.Bacc("TRN2", debug=False, num_devices=NCORE)

    # DR-packed fp8 operands: "p (i j c)" layout pairs contraction dims
    # 256j+128i+p so one DoubleRow matmul contracts 256 dims in 0.5
    # cycles/col.  x and the q/k/v weights ship as (hi, residual) fp8
    # pairs; 3 accumulation chains (x8 w8 + x8 dw8 + dx8 w8) reproduce
    # the fp16 product to ~0.2% at 3/8 the PE cost.
    x8_d = nc.dram_tensor("x8", [128, 8 * T], F8, kind="ExternalInput").ap()
    dx8_d = nc.dram_tensor("dx8", [128, 8 * T], F8, kind="ExternalInput").ap()
    wqk8_d = nc.dram_tensor("wqk8", [128, 4096], F8, kind="ExternalInput").ap()
    dwqk8_d = nc.dram_tensor("dwqk8", [128, 4096], F8,
                             kind="ExternalInput").ap()
    wv8_d = nc.dram_tensor("wv8", [128, 2048], F8, kind="ExternalInput").ap()
    dwv8_d = nc.dram_tensor("dwv8", [128, 2048], F8,
                            kind="ExternalInput").ap()
    wo_d = nc.dram_tensor("wo", [256, C], F16, kind="ExternalInput").ap()
    ccT_d = nc.dram_tensor("ccT", [128, T], F16, kind="ExternalInput").ap()
    ssT_d = nc.dram_tensor("ssT", [128, T], F16, kind="ExternalInput").ap()
    outT_d = nc.dram_tensor("outT", [C, T], F16, kind="ExternalOutput").ap()

    with tile.TileContext(nc) as tc, ExitStack() as ctx:
        const = ctx.enter_context(tc.tile_pool(name="const", bufs=1))
        rtp = ctx.enter_context(tc.tile_pool(name="rtp", bufs=2))
        pp = ctx.enter_context(tc.tile_pool(name="pp", bufs=5))
        nrm = ctx.enter_context(tc.tile_pool(name="nrm", bufs=3))
        obp = ctx.enter_context(tc.tile_pool(name="obp", bufs=4))
        psum = ctx.enter_context(tc.tile_pool(name="psum", bufs=2, space="PSUM"))

        # ---- persistent SBUF tensors ----
        # q/k weights (hi + residual) land first so the first real
        # matmuls can start while the rest is still in flight.
        wqk8 = const.tile([128, 4096], F8, tag="wqk8", name="wqk8")
        dwqk8 = const.tile([128, 4096], F8, tag="dwqk8", name="dwqk8")
        nc.scalar.dma_start(out=wqk8[:], in_=wqk8_d[:])
        nc.scalar.dma_start(out=dwqk8[:], in_=dwqk8_d[:])
        # views "p i (j m)": i-halves of the 256-dim contraction pairs
        wqk8_v = wqk8[:].rearrange("p (i jm) -> p i jm", i=2)
        dwqk8_v = dwqk8[:].rearrange("p (i jm) -> p i jm", i=2)

        # remaining input DMAs stay OFF the scalar queue: each DMA issue
        # holds its SEQ ~1.3us, and ACT's sequencer must be free for the
        # first rope bounces.  Early-needed tensors ride the sync queue;
        # late-needed ones go through Pool's SWDGE (Pool is idle early).
        cc = const.tile([128, T], F16, tag="cc")
        ss = const.tile([128, T], F16, tag="ss")
        nc.scalar.dma_start(out=cc[:, 0:1024], in_=ccT_d[:, 0:1024])
        nc.scalar.dma_start(out=ss[:, 0:1024], in_=ssT_d[:, 0:1024])

        wv8 = const.tile([128, 2048], F8, tag="wv8", name="wv8")
        dwv8 = const.tile([128, 2048], F8, tag="dwv8", name="dwv8")
        wv8_v = wv8[:].rearrange("p (i jd) -> p i jd", i=2)
        dwv8_v = dwv8[:].rearrange("p (i jd) -> p i jd", i=2)

        # PE warm-up: dependency-free matmuls that cover the input-DMA wait
        # and carry the tensor engine through its p-state ramp before the
        # first real matmul issues.
        warm = const.tile([128, 512], F16, tag="warm")
        nc.gpsimd.memset(warm[:], 0.0)
        pw = psum.tile([128, 512], F32, tag="qk", name="pw")
        for _ in range(WARM_N):
            nc.tensor.matmul(
                pw[:, 0:512], lhsT=warm[:, 0:128], rhs=warm[:],
                start=True, stop=True)

        wo_all = const.tile([128, 2 * C], F16, tag="wo", name="wo")
        nc.sync.dma_start(
            out=wo_all[:].rearrange("p (kt n) -> p kt n", n=C),
            in_=wo_d.rearrange("(kt p) n -> p kt n", p=128))
        wo_sb = [wo_all[:, kb * C:(kb + 1) * C] for kb in range(2)]

        # q8/k8: fp8e4 tiles [128, 2T].  head h lives in partitions
        # [32h, 32h+32); re dims at cols [0,T), im dims at cols [T,2T).
        # This matches the wqk column order (re of h0..h3, im of h0..h3),
        # so rope's sub/add write them directly -- no relayout copies --
        # and QK^T runs as one fp8 DoubleRow matmul per (head, k-block):
        # lhsT=[32,2,128] k-view, rhs=[32,2,n] q-view contract 64 dims.
        q8 = const.tile([128, 2 * T], F8, tag="q8", name="q8")
        k8 = const.tile([128, 2 * T], F8, tag="k8", name="k8")
        q8v = q8[:].rearrange("p (i t) -> p i t", i=2)
        k8v = k8[:].rearrange("p (i t) -> p i t", i=2)
        # v: [128 kpos, 4 heads x 16 blocks x 128] fp16; cols 0-63 of each
        # block = v dims, cols 64-127 = ones (denominator rows of PV psum)
        vT = const.tile([128, HL * NKB * 128], F16, tag="vT", name="vT")
        vT_v = vT[:].rearrange("p (h b c) -> p h b c", h=HL, b=NKB)
        nc.gpsimd.memset(vT_v[:, :, :, 64:128], 1.0)
        # y^T tiles: [128, T] x2 (4 heads x 64 dims)
        yT = [const.tile([128, T], F16, tag=f"yT{kb}", name=f"yT{kb}")
              for kb in range(2)]

        # x (hi + residual) fp8, persistent DR layout "p (i j t)"
        x8 = const.tile([128, 8 * T], F8, tag="x8", name="x8")
        dx8 = const.tile([128, 8 * T], F8, tag="dx8", name="dx8")
        x8_v = x8[:].rearrange("p (i jt) -> p i jt", i=2)
        dx8_v = dx8[:].rearrange("p (i jt) -> p i jt", i=2)
        x8_4v = x8[:].rearrange("p (i j t) -> p i j t", i=2, j=4)
        dx8_4v = dx8[:].rearrange("p (i j t) -> p i j t", i=2, j=4)
        x8_d4v = x8_d.rearrange("p (i j t) -> p i j t", i=2, j=4)
        dx8_d4v = dx8_d.rearrange("p (i j t) -> p i j t", i=2, j=4)

        def x_load(n, ways=2):
            t0 = n * CH
            nc.sync.dma_start(out=x8_4v[:, :, :, t0:t0 + CH],
                              in_=x8_d4v[:, :, :, t0:t0 + CH])
            nc.sync.dma_start(out=dx8_4v[:, :, :, t0:t0 + CH],
                              in_=dx8_d4v[:, :, :, t0:t0 + CH])

        CHAINS_QK = ((x8_v, wqk8_v), (x8_v, dwqk8_v), (dx8_v, wqk8_v))
        CHAINS_V = ((x8_v, wv8_v), (x8_v, dwv8_v), (dx8_v, wv8_v))

        def qk_units(n, g, act_bounce=False, splits=((0, CH),),
                     tags=(("qk", 2), ("qk", 2))):
            """q (g=0) / k (g=1) projection + rope for chunk n as a list
            of ~0.4us PE filler units (chain-quarters of the pre/pim psum
            accumulations; rope rides on the last one).  `splits` breaks
            the chunk into column ranges emitted as separate unit groups
            (used to fast-path the first k-block before the first exp)."""
            t0 = n * CH

            def mk(a, b):
                st = {}

                def mm(name, m, ci):
                    def u():
                        if ci == 0:
                            tg, bf = tags[0 if name == "psre" else 1]
                            st[name] = psum.tile([128, CH], F32, tag=tg,
                                                 name=name, bufs=bf)
                        xs, ws = CHAINS_QK[ci]
                        for j in range(4):
                            nc.tensor.matmul(
                                st[name][:, a:b],
                                lhsT=ws[:, :, j * 512 + m * 128:
                                        j * 512 + (m + 1) * 128],
                                rhs=xs[:, :, j * T + t0 + a:j * T + t0 + b],
                                start=(ci == 0 and j == 0),
                                stop=(ci == 2 and j == 3),
                                perf_mode=DR,
                            )
                    return u

                def rope():
                    mul = mybir.AluOpType.mult
                    sub = mybir.AluOpType.subtract
                    add = mybir.AluOpType.add
                    pre, pim = st["psre"], st["psim"]
                    w = b - a
                    ccn = cc[:, t0 + a:t0 + b]
                    ssn = ss[:, t0 + a:t0 + b]
                    t1 = rtp.tile([128, CH], F16, tag="t1")
                    t2 = rtp.tile([128, CH], F16, tag="t2")
                    t3 = rtp.tile([128, CH], F16, tag="t3")
                    t4 = rtp.tile([128, CH], F16, tag="t4")
                    # psum -> fp16 bounce: ACT while it still has slack
                    # (early chunks), DVE once exp saturates ACT; the
                    # rope arithmetic runs in DVE 4x fp16 mode.
                    preb = rtp.tile([128, CH], F16, tag="preb")
                    pimb = rtp.tile([128, CH], F16, tag="pimb")
                    if act_bounce:
                        nc.scalar.copy(preb[:, 0:w], pre[:, a:b])
                        nc.scalar.copy(pimb[:, 0:w], pim[:, a:b])
                    else:
                        nc.vector.tensor_copy(preb[:, 0:w], pre[:, a:b])
                        nc.vector.tensor_copy(pimb[:, 0:w], pim[:, a:b])
                    preb_, pimb_ = preb[:, 0:w], pimb[:, 0:w]
                    nc.vector.tensor_tensor(t1[:, 0:w], preb_, ccn, mul)
                    nc.vector.tensor_tensor(t2[:, 0:w], pimb_, ssn, mul)
                    nc.vector.tensor_tensor(t3[:, 0:w], preb_, ssn, mul)
                    nc.vector.tensor_tensor(t4[:, 0:w], pimb_, ccn, mul)
                    # sub/add write the fp8 q8/k8 tiles directly (re
                    # half / im half): no relayout copies needed.  Pool
                    # (mostly idle) can take them to unload DVE.
                    eng = nc.gpsimd if SAPOOL else nc.vector
                    o = q8 if g == 0 else k8
                    eng.tensor_tensor(
                        o[:, t0 + a:t0 + b], t1[:, 0:w], t2[:, 0:w], sub)
                    eng.tensor_tensor(
                        o[:, T + t0 + a:T + t0 + b], t3[:, 0:w],
                        t4[:, 0:w], add)

                last = mm("psim", 2 * g + 1, 2)
                return [mm("psre", 2 * g, 0), mm("psre", 2 * g, 1),
                        mm("psre", 2 * g, 2), mm("psim", 2 * g + 1, 0),
                        mm("psim", 2 * g + 1, 1),
                        lambda: (last(), rope())]

            out = []
            for (a, b) in splits:
                out.extend(mk(a, b))
            return out

        def qk_g(n, g, act_bounce=False, splits=((0, CH),)):
            for u in qk_units(n, g, act_bounce, splits):
                u()

        def v_units(n, tb, act_bounce=False):
            """v projection for 128-row block tb of chunk n: 3 chain
            units of ~0.2us; the psum->fp16 bounce rides on the last."""
            t0 = n * CH + tb * 128
            st = {}

            def mm(ci):
                def u():
                    if ci == 0:
                        st["psv"] = psum.tile([128, CH], F32, tag="qk",
                                              name="psv")
                    xs, ws = CHAINS_V[ci]
                    for j in range(4):
                        nc.tensor.matmul(
                            st["psv"][:, 0:256],
                            lhsT=xs[:, :, j * T + t0:j * T + t0 + 128],
                            rhs=ws[:, :, j * 256:(j + 1) * 256],
                            start=(ci == 0 and j == 0),
                            stop=(ci == 2 and j == 3),
                            perf_mode=DR,
                        )
                    if ci == 2:
                        blk = 4 * n + tb
                        dst = vT_v[:, :, blk, 0:64]
                        src = st["psv"][:, 0:256].rearrange(
                            "p (h d) -> p h d", d=64)
                        # gpsimd cannot read PSUM; the bounce (psum holds
                        # 1024*v -- x/w ship pre-scaled x16/x64 to keep
                        # fp8 residuals out of the e4m3 subnormal range)
                        # folds in the 2^-10 fix free.  ACT takes it in
                        # the A/B phases where exp leaves it slack.
                        if act_bounce:
                            nc.scalar.activation(
                                dst, src,
                                mybir.ActivationFunctionType.Copy,
                                scale=1.0 / 1024.0)
                        else:
                            nc.vector.tensor_scalar_mul(
                                dst, src, 1.0 / 1024.0)
                return u

            return [mm(0), mm(1), mm(2)]

        def v_tb(n, tb):
            for u in v_units(n, tb):
                u()

        filler_q = []

        def dummy(n=2):
            # keep-warm matmuls: PE p-state drops 2x after an idle gap and
            # needs 3us of continuous execution to recover; padding known
            # exp-bound stretches keeps the real matmuls at full clock.
            pd = psum.tile([128, 512], F32, tag="qk", name="pd")
            for _ in range(n):
                nc.tensor.matmul(
                    pd[:, 0:512], lhsT=warm[:, 0:128], rhs=warm[:],
                    start=True, stop=True)

        def pump(keep_warm=0):
            if filler_q:
                filler_q.pop(0)()
            elif keep_warm:
                dummy(keep_warm)

        def attention_units(h, q0, qn, pump_every=0, at_blocks=None,
                            keep_warm=0, pair_full=False, norm_act=False):
            """One head, q-cols [q0, q0+qn) as a list of emission units.

            units[kb] = at_blocks/pump + scores(kb) + pv(kb-1); the last
            unit is pv(nkb-1) + final normalize.  The scheduler emits the
            NEXT head's units[0] just before this head's last unit: that
            head-boundary scores runs on PE while the last exp still
            streams on ACT, so ACT never drains between heads.  (The pst
            double-buffer is free at exactly that point: exp(nkb-2) has
            been consumed by the preceding pv.)

            at_blocks: {kb: [unit, ...]} -- mandatory work units emitted
            just before scores_block(kb); used for dependencies of later
            pv_blocks (e.g. v tiles), unlike best-effort pump fillers.
            """
            qv = q8v[32 * h:32 * h + 32]
            kv = k8v[32 * h:32 * h + 32]
            r0 = 64 * (h % 2)
            nkb = (q0 + qn) // 128
            fd = q0 // 128  # first diagonal block
            st_ = {}
            Ps = {}

            def block_off(kb):
                return 128 * (kb - fd) if kb >= fd else 0

            def scores_block(kb):
                if kb == 0:
                    st_["psy"] = psum.tile([128, qn], F32, tag="y",
                                           name="psy", bufs=1)
                off = block_off(kb)
                pst = psum.tile([128, qn], F32, tag="st", name="pst")
                for (a, b) in _splits(off, qn):
                    nc.tensor.matmul(
                        pst[:, a:b],
                        lhsT=kv[:, :, kb * 128:(kb + 1) * 128],
                        rhs=qv[:, :, q0 + a:q0 + b],
                        start=True,
                        stop=True,
                        perf_mode=DR,
                        tile_position=(32 * h, 0),
                    )
                P = pp.tile([128, 1024], F16, tag="P")
                Ps[kb] = (P, 0)
                nc.scalar.activation(
                    P[:, off:qn], pst[:, off:qn],
                    mybir.ActivationFunctionType.Exp, scale=SCALE)
                if kb >= fd:
                    # zero strictly-upper triangle of the leading 128 cols
                    nc.gpsimd.affine_select(
                        out=P[:, off:off + 128],
                        in_=P[:, off:off + 128],
                        compare_op=mybir.AluOpType.is_ge,
                        fill=0.0,
                        base=0,
                        pattern=[[1, 128]],
                        channel_multiplier=-1,
                    )

            def scores_pair(kb):
                # two qn=512 k-blocks share one [128,1024] pst / P pair:
                # a single exp call covers both, halving the ACT per-call
                # access-latency overhead and doubling the exp lookahead
                # the pst double-buffer can hold.  For diagonal blocks the
                # gap columns [512, 512+off1) are dead: matmul start
                # zeroes the bank region, exp of them is masked/unread
                # (pv reads only [off:512] of each half).
                if kb == 0:
                    st_["psy"] = psum.tile([128, qn], F32, tag="y",
                                           name="psy", bufs=1)
                pst = psum.tile([128, 1024], F32, tag="st", name="pst")
                for sub in (0, 1):
                    off = block_off(kb + sub)
                    nc.tensor.matmul(
                        pst[:, 512 * sub + off:512 * sub + 512],
                        lhsT=kv[:, :, (kb + sub) * 128:(kb + sub + 1) * 128],
                        rhs=qv[:, :, q0 + off:q0 + 512],
                        start=True,
                        stop=True,
                        perf_mode=DR,
                        tile_position=(32 * h, 0),
                    )
                off0 = block_off(kb)
                P = pp.tile([128, 1024], F16, tag="P")
                Ps[kb] = (P, 0)
                Ps[kb + 1] = (P, 512)
                nc.scalar.activation(
                    P[:, off0:1024], pst[:, off0:1024],
                    mybir.ActivationFunctionType.Exp, scale=SCALE)
                for sub in (0, 1):
                    off = block_off(kb + sub)
                    if kb + sub >= fd:
                        nc.gpsimd.affine_select(
                            out=P[:, 512 * sub + off:512 * sub + off + 128],
                            in_=P[:, 512 * sub + off:512 * sub + off + 128],
                            compare_op=mybir.AluOpType.is_ge,
                            fill=0.0,
                            base=0,
                            pattern=[[1, 128]],
                            channel_multiplier=-1,
                        )

            def pv_block(kb):
                off = block_off(kb)
                P, coff = Ps.pop(kb)
                psy = st_["psy"]
                for (a, b) in _splits(off, qn):
                    # last writer of the psum bank holding col a is diag
                    # block fd + 4*(a//512) + 3
                    kb_stop = min(fd + 4 * (a // 512) + 3, nkb - 1)
                    nc.tensor.matmul(
                        psy[:, a:b],
                        lhsT=vT_v[:, h, kb, :],
                        rhs=P[:, coff + a:coff + b],
                        start=(kb == 0),
                        stop=(kb == kb_stop),
                    )

            def normalize(a, b):
                # psum rows 64-127 all hold the denominator row l (ones
                # cols of vT): reciprocal + one multiply per psum bank,
                # emitted as soon as that bank's accumulation closes.
                # (walrus forbids two PSUM reads in one TensorTensor, so
                # a single divide is not possible.)
                psy = st_["psy"]
                rlb = nrm.tile([64, 512], F32, tag="rlb")
                nc.vector.reciprocal(rlb[:, 0:b - a], psy[64:128, a:b])
                nc.vector.tensor_tensor(
                    yT[h // 2][r0:r0 + 64, q0 + a:q0 + b],
                    psy[0:64, a:b], rlb[:, 0:b - a], mybir.AluOpType.mult)

            def mk_unit(kb, scores_fn, pvs):
                def u():
                    if at_blocks and kb in at_blocks:
                        for ab in at_blocks[kb]:
                            ab()
                    if pump_every and kb % pump_every == 0:
                        pump(keep_warm)
                    if scores_fn:
                        scores_fn(kb)
                    for pkb in pvs:
                        pv_block(pkb)
                        if pkb == min(fd + 3, nkb - 1) and qn > 512:
                            normalize(0, 512)  # bank 0 closed early
                return u

            units = []
            if PAIR and pair_full and qn == 512 and nkb % 2 == 0:
                # all blocks in exp-pairs; pv's trail one scores event
                pend = []
                for kb in range(0, nkb, 2):
                    units.append(mk_unit(kb, scores_pair, pend))
                    pend = [kb, kb + 1]
                st_["pend"] = pend
            else:
                for kb in range(nkb):
                    units.append(mk_unit(
                        kb, scores_block, [kb - 1] if kb > 0 else []))
                st_["pend"] = [nkb - 1]

            def final():
                for pkb in st_["pend"]:
                    pv_block(pkb)
                normalize(512 if qn > 512 else 0, qn)

            return units + [final]

        def run_heads(seq):
            """Emit head unit-lists with one-unit cross-head lookahead:
            the next head's scores(0) goes out before this head's final
            pv, so the exp stream never drains at a head boundary."""
            for i, units in enumerate(seq):
                for u in units[1 if i else 0:-1]:
                    u()
                if i + 1 < len(seq):
                    seq[i + 1][0]()
                units[-1]()

        def o_proj(nt, mo, tail=False):
            """Output block: feat rows [128*mo ..+128), q [512*nt ..+512)."""
            ob = obp.tile([128, 512], F16, tag="ob", name="ob")
            ps = psum.tile([128, CH], F32, tag="qk", name="psob")
            for kb in range(2):
                nc.tensor.matmul(
                    ps[:, 0:512],
                    lhsT=wo_sb[kb][:, mo * 128:(mo + 1) * 128],
                    rhs=yT[kb][:, nt * 512:(nt + 1) * 512],
                    start=(kb == 0),
                    stop=(kb == 1),
                )
            # in the tail ACT is idle once the exps are done: it takes half
            # the psum bounces there
            if tail and mo % 2 == 1:
                nc.scalar.copy(ob[:], ps[:, 0:512])
            else:
                nc.vector.tensor_copy(ob[:], ps[:, 0:512])
            # keep DMA issue off the ACT queue while exps run; in the tail
            # ACT is free and a second queue doubles drain bandwidth
            ring = nc.scalar if (tail and mo % 2 == 1) else nc.sync
            ring.dma_start(
                out=outT_d[mo * 128:(mo + 1) * 128, nt * 512:(nt + 1) * 512],
                in_=ob[:])

        # ---- schedule ----
        # Emission order == per-engine queue order.  ACT (exp) is the
        # attention pacer now that scores+PV run fp8-DR/fp16 (PE 0.625
        # vs ACT 0.833 ns per score column), so all qkv/o_proj work is
        # pumped into the attention stream as ~0.4us filler units.
        # Dependency safety comes from emission order: a filler is
        # always emitted before the instruction that needs it.
        x_load(0)
        BR = int(os.environ.get("K_BRIDGE", "0"))
        if BR:
            # bridge the mid-chain DMA waits (dwqk8/dx8 arrival) with pad
            # matmuls on the still-live warmup psum so the PE p-state
            # ramp survives into the first real chains
            q0u = qk_units(0, 0, act_bounce=True)
            q0u[0]()
            for _ in range(BR):
                nc.tensor.matmul(pw[:, 0:512], lhsT=warm[:, 0:128],
                                 rhs=warm[:], start=True, stop=True)
            q0u[1]()
            for _ in range(BR // 2):
                nc.tensor.matmul(pw[:, 0:512], lhsT=warm[:, 0:128],
                                 rhs=warm[:], start=True, stop=True)
            for u in q0u[2:]:
                u()
        else:
            qk_g(0, 0, act_bounce=bool(int(os.environ.get("K_RB0Q", "0"))))
        qk_g(0, 1, act_bounce=bool(int(os.environ.get("K_RB0K", "1"))))
        k0rest = []
        nc.scalar.dma_start(out=wv8[:], in_=wv8_d[:])
        nc.scalar.dma_start(out=dwv8[:], in_=dwv8_d[:])
        x_load(1)
        # tile A (q 0-512): needs only chunk 0 q/k; v blocks land as
        # at_blocks just before their pv consumer.  The chunk-1 qk and v
        # work soaks up the rope-latency wait before the first scores.
        filler_q.extend(qk_units(1, 0, act_bounce=bool(
            int(os.environ.get("K_RB1Q", "0")))))
        filler_q.extend(qk_units(1, 1, act_bounce=bool(
            int(os.environ.get("K_RB1K", "1")))))
        for tb in range(4):
            filler_q.extend(v_units(1, tb, act_bounce=bool(VB01)))
        run_heads([
            attention_units(0, 0, 512, pump_every=PUMP_A, norm_act=True, at_blocks={
                1: k0rest + v_units(0, 0, act_bounce=bool(VB01)),
                2: v_units(0, 1, act_bounce=bool(VB01)),
                3: v_units(0, 2, act_bounce=bool(VB01))
                   + v_units(0, 3, act_bounce=bool(VB01)),
            }),
            attention_units(1, 0, 512, pump_every=PUMP_A, norm_act=True, pair_full=True),
            attention_units(2, 0, 512, pump_every=PUMP_A, norm_act=True, pair_full=True),
            attention_units(3, 0, 512, pump_every=PUMP_A, norm_act=True, pair_full=True),
        ])
        while filler_q:
            pump()
        # tile B (q 512-1024): fillers: chunks 2,3 qk.
        x_load(2)
        x_load(3)
        nc.scalar.dma_start(out=cc[:, 1024:2048], in_=ccT_d[:, 1024:2048])
        nc.scalar.dma_start(out=ss[:, 1024:2048], in_=ssT_d[:, 1024:2048])
        filler_q.extend(qk_units(2, 0, act_bounce=bool(
            int(os.environ.get("K_RB2Q", "0")))))
        filler_q.extend(qk_units(2, 1, act_bounce=bool(
            int(os.environ.get("K_RB2K", "0")))))
        filler_q.extend(qk_units(3, 0, act_bounce=bool(
            int(os.environ.get("K_RB3Q", "0")))))
        filler_q.extend(qk_units(3, 1, act_bounce=bool(
            int(os.environ.get("K_RB3K", "1")))))
        run_heads([
            attention_units(0, 512, 512, pump_every=PUMP_B, pair_full=True, norm_act=True),
            attention_units(1, 512, 512, pump_every=PUMP_B, pair_full=True, norm_act=True),
            attention_units(2, 512, 512, pump_every=PUMP_B, pair_full=True, norm_act=True),
            attention_units(3, 512, 512, pump_every=PUMP_B, pair_full=True, norm_act=True),
        ])
        while filler_q:
            pump()
        # tail o_proj helpers (q 1024-2048): per mo one [128, 1024] ob
        # filled in two halves; nt=2 halves are emitted inside the last
        # head's attention as soon as its early psum bank is normalized.
        tail_obs = {}

        # single [128, 8x1024] tile for the q 1024-2048 output: per mo,
        # cols [1024mo, +512) = nt2 half, [+512, +1024) = nt3 half.
        # Keeping it one tile lets the tail flush as FOUR two-mo strided
        # DMAs instead of 16 narrow ones: each DMA costs ~625ns on the
        # shared HWDGE regardless of size, and that serialization was
        # the dominant term of the post-attention tail.
        obt_all = const.tile([128, 8 * 1024], F16, tag="obt", name="obt")

        def tail_half(nt, mo):
            # nt3 runs after the last attention: the st/y psum banks are
            # free, so cycling tags gives 5 buffers in rotation instead
            # of 2 and breaks the psum-recycle serialization of the tail.
            if nt == 3:
                tag, bufs = [("qk", 2), ("st", 2), ("y", 1)][mo % 3]
                ps = psum.tile([128, CH], F32, tag=tag, name="psob",
                               bufs=bufs)
            else:
                ps = psum.tile([128, CH], F32, tag="qk", name="psob")
            for kb in range(2):
                nc.tensor.matmul(
                    ps[:, 0:512],
                    lhsT=wo_sb[kb][:, mo * 128:(mo + 1) * 128],
                    rhs=yT[kb][:, nt * 512:(nt + 1) * 512],
                    start=(kb == 0),
                    stop=(kb == 1),
                )
            c0 = 1024 * mo + 512 * (nt - 2)
            if (nt == 3 and mo % 2 == 1) or (nt == 2 and mo >= 5):
                # true tail: exps are done, ACT is free
                nc.scalar.copy(obt_all[:, c0:c0 + 512], ps[:, 0:512])
            else:
                nc.vector.tensor_copy(obt_all[:, c0:c0 + 512],
                                      ps[:, 0:512])
            if nt == 2 and mo < 5:
                # nt2 halves flush per-mo while attention still runs
                nc.sync.dma_start(
                    out=outT_d[mo * 128:(mo + 1) * 128, 1024:1536],
                    in_=obt_all[:, c0:c0 + 512])
            elif nt == 2 and mo == 7:
                # the three post-attention nt2 halves go as ONE strided
                # DMA: each DMA costs ~625ns on the shared HWDGE, which
                # is the serializer of the final flush
                dst = outT_d.rearrange("(m p) t -> p m t", p=128)
                src_ = obt_all[:].rearrange("p (m t) -> p m t", t=1024)
                nc.sync.dma_start(out=dst[:, 5:8, 1024:1536],
                                  in_=src_[:, 5:8, 0:512])
            elif mo % 2 == 1:
                # nt3: one strided DMA flushes the (mo-1, mo) pair
                ring = nc.scalar if mo % 4 == 1 else nc.sync
                dst = outT_d.rearrange("(m p) t -> p m t", p=128)
                src = obt_all[:].rearrange("p (m t) -> p m t", t=1024)
                ring.dma_start(
                    out=dst[:, mo - 1:mo + 1, 1536:2048],
                    in_=src[:, mo - 1:mo + 1, 512:1024])

        # tile C (q 1024-2048): v chunks 2/3 are emitted at fixed blocks of
        # the first head (hard deps of pv blocks 8-15); o_proj of q 0-1024
        # is order-free filler spread across all four heads.
        filler_q.extend(
            (lambda nt=nt, mo=mo: o_proj(nt, mo))
            for nt in range(2) for mo in range(8))
        run_heads([
            attention_units(0, 1024, 1024, pump_every=PUMP_C0, at_blocks=(
                {
                    4: v_units(2, 0), 5: v_units(2, 1),
                    6: v_units(2, 2), 7: v_units(2, 3),
                    8: v_units(3, 0), 9: v_units(3, 1),
                    10: v_units(3, 2), 11: v_units(3, 3),
                } if int(os.environ.get("K_VSPREAD", "0")) else {
                    5: v_units(2, 0) + v_units(2, 1),
                    7: v_units(2, 2) + v_units(2, 3),
                    9: v_units(3, 0) + v_units(3, 1),
                    11: v_units(3, 2) + v_units(3, 3),
                })),
            attention_units(1, 1024, 1024, pump_every=PUMP_C,
                            keep_warm=KEEP_WARM),
            attention_units(2, 1024, 1024, pump_every=PUMP_C,
                            keep_warm=KEEP_WARM),
            attention_units(3, 1024, 1024, pump_every=PUMP_C,
                            keep_warm=KW3, at_blocks={
                13: [lambda mo=mo: tail_half(2, mo) for mo in range(2)],
                14: [lambda mo=mo: tail_half(2, mo) for mo in range(2, 5)],
            }),
        ])
        while filler_q:
            pump()
        # tail: keep-warm over the final-normalize wait so the o_proj
        # matmuls run at full p-state; the three remaining nt2 units are
        # emitted here (not as kb15 at_blocks) so their DVE copies queue
        # BEHIND head 3's final normalize instead of delaying it.
        if int(os.environ.get('K_TDUM', '3')):
            dummy(int(os.environ.get('K_TDUM', '3')))
        tail_half(3, 0)
        tail_half(2, 5)
        tail_half(3, 1)
        tail_half(2, 6)
        tail_half(3, 2)
        tail_half(2, 7)
        for mo in range(3, 8):
            tail_half(3, mo)

    nc.compile()
    return nc


def _dr_pack(a, scale):
    """[1024, M] f32 -> fp8 (hi, residual) pair in DR layout [128, 8M].

    DR layout "p (i j m)": element (p, i, j, m) = a[256j + 128i + p, m], so
    one DoubleRow matmul contracts dim pairs (256j+p, 256j+128+p).
    `scale` lifts the values so both them and their residuals quantize in
    the e4m3 normal range; the device compensates (rope tables / v copy).
    """
    import ml_dtypes
    f8 = ml_dtypes.float8_e4m3
    a = a * scale
    M = a.shape[1]
    hi = a.astype(f8)
    lo = (a - hi.astype(np.float32)).astype(f8)
    out = []
    for t in (hi, lo):
        t = t.reshape(4, 2, 128, M).transpose(2, 1, 0, 3).reshape(128, 8 * M)
        out.append(np.ascontiguousarray(t))
    return out


def shard_inputs(x, freqs_cos, freqs_sin, Wqkv, Wo):
    """Build the 8 per-core input maps (host-side sharding)."""
    x = np.asarray(x, dtype=np.float32)
    Wqkv = np.asarray(Wqkv, dtype=np.float32)
    Wo = np.asarray(Wo, dtype=np.float32)
    # cos/sin tables transposed and replicated x4 (one copy per local
    # head), pre-divided by 1024 to undo the x16/x64 fp8 shipping scales
    ccT = np.tile(np.asarray(freqs_cos, dtype=np.float32).T, (4, 1)) / 1024.0
    ssT = np.tile(np.asarray(freqs_sin, dtype=np.float32).T, (4, 1)) / 1024.0
    ccT = np.ascontiguousarray(ccT).astype(np.float16)
    ssT = np.ascontiguousarray(ssT).astype(np.float16)
    x8s = [_dr_pack(x[b].T, 16.0) for b in range(B)]

    in_maps = []
    for c in range(NCORE):
        b, hg = c // 4, c % 4
        re = [np.arange(g * 64, g * 64 + 64, 2)
              for g in range(4 * hg, 4 * hg + 4)]
        im = [np.arange(g * 64 + 1, g * 64 + 64, 2)
              for g in range(4 * hg, 4 * hg + 4)]
        qcols = np.concatenate(re + im)
        kcols = C + qcols
        wqk8, dwqk8 = _dr_pack(Wqkv[:, np.concatenate([qcols, kcols])], 64.0)
        wv8, dwv8 = _dr_pack(
            Wqkv[:, 2 * C + hg * 256: 2 * C + hg * 256 + 256], 64.0)
        wo = np.ascontiguousarray(
            Wo[hg * 256: hg * 256 + 256, :]).astype(np.float16)
        in_maps.append({
            "x8": x8s[b][0], "dx8": x8s[b][1],
            "wqk8": wqk8, "dwqk8": dwqk8, "wv8": wv8, "dwv8": dwv8,
            "wo": wo, "ccT": ccT, "ssT": ssT,
        })
    return in_maps


_NC_CACHE = None


def _get_nc():
    global _NC_CACHE
    if _NC_CACHE is None:
        _NC_CACHE = build_nc()
    return _NC_CACHE


def run(inputs, trace=False):
    from concourse.bass_utils import run_bass_kernel_spmd

    nc = _get_nc()
    in_maps = shard_inputs(**inputs)
    res = run_bass_kernel_spmd(nc, in_maps, list(range(NCORE)), trace=trace)
    out = np.empty((B, T, C), dtype=np.float32)
    for b in range(B):
        acc = res.results[4 * b]["outT"].astype(np.float32)
        for c in range(4 * b + 1, 4 * b + 4):
            acc = acc + res.results[c]["outT"].astype(np.float32)
        out[b] = acc.T
    return out, res


def kernel(**inputs):
    out, _ = run(inputs)
    return out



# revision 7
# speedup vs baseline: 1.0250x; 1.0030x over previous
"""Causal self-attention (B=2, T=2048, C=1024, H=16, D=64) with RoPE on TRN2.

Sharding: 8 cores = 2 (batch) x 4 (head-groups of 4 heads), no
inter-core communication: each core computes qkv + rope + causal
attention + a row-parallel o_proj partial for its 4 heads; the host
sums the 4 fp16 partials per batch.

Precision: x and the q/k/v weights ship as fp8-e4m3 (hi, residual)
pairs, pre-scaled x16 / x64 so the residuals stay out of the e4m3
subnormal range; 3 DoubleRow accumulation chains (x8 w8 + x8 dw8 +
dx8 w8) reproduce the fp16 projection to ~0.2% at 3/8 the PE cost
(DoubleRow contracts 256 dims at 0.5 cycles/col).  The 2^-10 shipping
scale is undone for free in the rope tables (q/k) and the v psum
bounce.  Roped q/k are quantized to fp8 on the fly -- the rope sub/add
writes fp8 tiles directly in a DoubleRow-ready layout (head h in
partitions [32h,32h+32), re dims cols [0,T), im cols [T,2T)) -- so
QK^T runs as one fp8-DR matmul per (head, k-block) at 0.5 cycles/col:
2x the fp16 rate with only a single e4m3 quantization of q/k (rel err
1.65e-2 vs the 2e-2 gate).  P and V stay fp16 (peaked attention rows
pass V errors straight through, so V cannot be fp8), o_proj fp16.

Attention computes S^T = (K Q^T) per 128-row k-block so probabilities
exit exp() already transposed for P^T @ V.  V tiles carry 64 all-ones
columns: the PV matmul then yields psum rows 0-63 = y, rows 64-127 =
the softmax denominator replicated 64x (zero extra PE cycles since
matmul cost is free-dim only) -- normalization is one DVE reciprocal +
multiply per psum bank, emitted as soon as that bank closes.  No max
subtraction: logits are O(+-8) and exp fits fp16 range.

Scheduling: emission order is per-engine queue order.  exp on ACT
(0.833 ns/col) outweighs the fp8-DR scores + fp16 PV (0.625 ns/col),
so attention is ACT-paced and all projection/o_proj work is pumped
into the attention stream as ~0.4us PE filler units (chain-quarters
of the DR accumulations).  Heads are emitted with a one-unit
lookahead (next head's first scores before this head's last pv) so
the exp stream does not drain at head boundaries; rope/v psum bounces
ride on ACT only where exp leaves it slack.  DMA issues are kept off
queues whose sequencer is latency-critical (each issue holds its SEQ
~1.3us): the late-needed weight/table loads are emitted mid-schedule.
The o_proj tail is ordered so head 3's final normalize is not queued
behind tail copies on DVE, pads PE p-state over the normalize wait,
flushes q 1024-2048 as a few wide strided DMAs (each DMA costs
~625ns on the shared HWDGE regardless of size) and cycles its psum
tiles across the qk/st/y tags -- the attention banks are free by
then -- so the final o_proj chain is not serialized on the 2-buffer
psum rotation.
"""

import sys
import os

sys.path.insert(0, "/opt/trn_rl_repo")

import numpy as np
from contextlib import ExitStack

import concourse.bass as bass
import concourse.bacc as bacc
import concourse.mybir as mybir
import concourse.tile as tile

F32 = mybir.dt.float32
F16 = mybir.dt.float16
F8 = mybir.dt.float8e4
DR = mybir.MatmulPerfMode.DoubleRow

# problem constants (hardcoded per contract)
B, T, C, NH, D = 2, 2048, 1024, 16, 64
# schedule knobs (env-overridable for tuning sweeps)
PUMP_A = int(os.environ.get("K_PUMP_A", "2"))
PUMP_B = int(os.environ.get("K_PUMP_B", "3"))
PUMP_C0 = int(os.environ.get("K_PUMP_C0", "5"))
PUMP_C = int(os.environ.get("K_PUMP_C", "4"))
KEEP_WARM = int(os.environ.get("K_KEEP_WARM", "0"))
WARM_N = int(os.environ.get("K_WARM_N", "8"))
PAIR = int(os.environ.get("K_PAIR", "0"))
FASTK = int(os.environ.get("K_FASTK", "0"))
AB23 = int(os.environ.get("K_AB23", "0"))
SAPOOL = int(os.environ.get("K_SAPOOL", "0"))
VB01 = int(os.environ.get("K_VB01", "1"))
RB0 = int(os.environ.get("K_RB0", "1"))
RB1 = int(os.environ.get("K_RB1", "0"))
KW3 = int(os.environ.get("K_KW3", "0"))
VB2 = bool(int(os.environ.get("K_VB2", "0")))
VB3 = bool(int(os.environ.get("K_VB3", "1")))
OBA = int(os.environ.get("K_OBA", "0"))
HL = 4            # local heads per core
NCORE = 8
CH = 512          # qkv T-chunk width
NCHUNK = T // CH  # 4
SCALE = 1.0 / 8.0  # 1/sqrt(D)
NKB = T // 128    # 16 k-blocks


def _splits(a, b):
    """Split [a, b) at 512 boundaries (psum bank = 512 f32)."""
    out = []
    while a < b:
        nxt = min(b, (a // 512 + 1) * 512)
        out.append((a, nxt))
        a = nxt
    return out


def build_nc():
    nc = bacc.Bacc("TRN2", debug=False, num_devices=NCORE)

    # DR-packed fp8 operands: "p (i j c)" layout pairs contraction dims
    # 256j+128i+p so one DoubleRow matmul contracts 256 dims in 0.5
    # cycles/col.  x and the q/k/v weights ship as (hi, residual) fp8
    # pairs; 3 accumulation chains (x8 w8 + x8 dw8 + dx8 w8) reproduce
    # the fp16 product to ~0.2% at 3/8 the PE cost.
    x8_d = nc.dram_tensor("x8", [128, 8 * T], F8, kind="ExternalInput").ap()
    dx8_d = nc.dram_tensor("dx8", [128, 8 * T], F8, kind="ExternalInput").ap()
    wqk8_d = nc.dram_tensor("wqk8", [128, 4096], F8, kind="ExternalInput").ap()
    dwqk8_d = nc.dram_tensor("dwqk8", [128, 4096], F8,
                             kind="ExternalInput").ap()
    wv8_d = nc.dram_tensor("wv8", [128, 2048], F8, kind="ExternalInput").ap()
    dwv8_d = nc.dram_tensor("dwv8", [128, 2048], F8,
                            kind="ExternalInput").ap()
    wo_d = nc.dram_tensor("wo", [256, C], F16, kind="ExternalInput").ap()
    ccT_d = nc.dram_tensor("ccT", [128, T], F16, kind="ExternalInput").ap()
    ssT_d = nc.dram_tensor("ssT", [128, T], F16, kind="ExternalInput").ap()
    outT_d = nc.dram_tensor("outT", [C, T], F16, kind="ExternalOutput").ap()

    with tile.TileContext(nc) as tc, ExitStack() as ctx:
        const = ctx.enter_context(tc.tile_pool(name="const", bufs=1))
        rtp = ctx.enter_context(tc.tile_pool(name="rtp", bufs=2))
        pp = ctx.enter_context(tc.tile_pool(name="pp", bufs=5))
        nrm = ctx.enter_context(tc.tile_pool(name="nrm", bufs=3))
        obp = ctx.enter_context(tc.tile_pool(name="obp", bufs=4))
        psum = ctx.enter_context(tc.tile_pool(name="psum", bufs=2, space="PSUM"))

        # ---- persistent SBUF tensors ----
        # q/k weights (hi + residual) land first so the first real
        # matmuls can start while the rest is still in flight.
        wqk8 = const.tile([128, 4096], F8, tag="wqk8", name="wqk8")
        dwqk8 = const.tile([128, 4096], F8, tag="dwqk8", name="dwqk8")
        nc.scalar.dma_start(out=wqk8[:], in_=wqk8_d[:])
        nc.scalar.dma_start(out=dwqk8[:], in_=dwqk8_d[:])
        # views "p i (j m)": i-halves of the 256-dim contraction pairs
        wqk8_v = wqk8[:].rearrange("p (i jm) -> p i jm", i=2)
        dwqk8_v = dwqk8[:].rearrange("p (i jm) -> p i jm", i=2)

        # remaining input DMAs stay OFF the scalar queue: each DMA issue
        # holds its SEQ ~1.3us, and ACT's sequencer must be free for the
        # first rope bounces.  Early-needed tensors ride the sync queue;
        # late-needed ones go through Pool's SWDGE (Pool is idle early).
        cc = const.tile([128, T], F16, tag="cc")
        ss = const.tile([128, T], F16, tag="ss")
        nc.scalar.dma_start(out=cc[:, 0:1024], in_=ccT_d[:, 0:1024])
        nc.scalar.dma_start(out=ss[:, 0:1024], in_=ssT_d[:, 0:1024])

        wv8 = const.tile([128, 2048], F8, tag="wv8", name="wv8")
        dwv8 = const.tile([128, 2048], F8, tag="dwv8", name="dwv8")
        wv8_v = wv8[:].rearrange("p (i jd) -> p i jd", i=2)
        dwv8_v = dwv8[:].rearrange("p (i jd) -> p i jd", i=2)

        # PE warm-up: dependency-free matmuls that cover the input-DMA wait
        # and carry the tensor engine through its p-state ramp before the
        # first real matmul issues.
        warm = const.tile([128, 512], F16, tag="warm")
        nc.gpsimd.memset(warm[:], 0.0)
        pw = psum.tile([128, 512], F32, tag="qk", name="pw")
        for _ in range(WARM_N):
            nc.tensor.matmul(
                pw[:, 0:512], lhsT=warm[:, 0:128], rhs=warm[:],
                start=True, stop=True)

        wo_all = const.tile([128, 2 * C], F16, tag="wo", name="wo")
        nc.sync.dma_start(
            out=wo_all[:].rearrange("p (kt n) -> p kt n", n=C),
            in_=wo_d.rearrange("(kt p) n -> p kt n", p=128))
        wo_sb = [wo_all[:, kb * C:(kb + 1) * C] for kb in range(2)]

        # q8/k8: fp8e4 tiles [128, 2T].  head h lives in partitions
        # [32h, 32h+32); re dims at cols [0,T), im dims at cols [T,2T).
        # This matches the wqk column order (re of h0..h3, im of h0..h3),
        # so rope's sub/add write them directly -- no relayout copies --
        # and QK^T runs as one fp8 DoubleRow matmul per (head, k-block):
        # lhsT=[32,2,128] k-view, rhs=[32,2,n] q-view contract 64 dims.
        q8 = const.tile([128, 2 * T], F8, tag="q8", name="q8")
        k8 = const.tile([128, 2 * T], F8, tag="k8", name="k8")
        q8v = q8[:].rearrange("p (i t) -> p i t", i=2)
        k8v = k8[:].rearrange("p (i t) -> p i t", i=2)
        # v: [128 kpos, 4 heads x 16 blocks x 128] fp16; cols 0-63 of each
        # block = v dims, cols 64-127 = ones (denominator rows of PV psum)
        vT = const.tile([128, HL * NKB * 128], F16, tag="vT", name="vT")
        vT_v = vT[:].rearrange("p (h b c) -> p h b c", h=HL, b=NKB)
        nc.gpsimd.memset(vT_v[:, :, :, 64:128], 1.0)
        # y^T tiles: [128, T] x2 (4 heads x 64 dims)
        yT = [const.tile([128, T], F16, tag=f"yT{kb}", name=f"yT{kb}")
              for kb in range(2)]

        # x (hi + residual) fp8, persistent DR layout "p (i j t)"
        x8 = const.tile([128, 8 * T], F8, tag="x8", name="x8")
        dx8 = const.tile([128, 8 * T], F8, tag="dx8", name="dx8")
        x8_v = x8[:].rearrange("p (i jt) -> p i jt", i=2)
        dx8_v = dx8[:].rearrange("p (i jt) -> p i jt", i=2)
        x8_4v = x8[:].rearrange("p (i j t) -> p i j t", i=2, j=4)
        dx8_4v = dx8[:].rearrange("p (i j t) -> p i j t", i=2, j=4)
        x8_d4v = x8_d.rearrange("p (i j t) -> p i j t", i=2, j=4)
        dx8_d4v = dx8_d.rearrange("p (i j t) -> p i j t", i=2, j=4)

        def x_load(n, ways=2):
            t0 = n * CH
            nc.sync.dma_start(out=x8_4v[:, :, :, t0:t0 + CH],
                              in_=x8_d4v[:, :, :, t0:t0 + CH])
            nc.sync.dma_start(out=dx8_4v[:, :, :, t0:t0 + CH],
                              in_=dx8_d4v[:, :, :, t0:t0 + CH])

        CHAINS_QK = ((x8_v, wqk8_v), (x8_v, dwqk8_v), (dx8_v, wqk8_v))
        CHAINS_V = ((x8_v, wv8_v), (x8_v, dwv8_v), (dx8_v, wv8_v))

        def qk_units(n, g, act_bounce=False, splits=((0, CH),),
                     tags=(("qk", 2), ("qk", 2))):
            """q (g=0) / k (g=1) projection + rope for chunk n as a list
            of ~0.4us PE filler units (chain-quarters of the pre/pim psum
            accumulations; rope rides on the last one).  `splits` breaks
            the chunk into column ranges emitted as separate unit groups
            (used to fast-path the first k-block before the first exp)."""
            t0 = n * CH

            def mk(a, b):
                st = {}

                def mm(name, m, ci):
                    def u():
                        if ci == 0:
                            tg, bf = tags[0 if name == "psre" else 1]
                            st[name] = psum.tile([128, CH], F32, tag=tg,
                                                 name=name, bufs=bf)
                        xs, ws = CHAINS_QK[ci]
                        for j in range(4):
                            nc.tensor.matmul(
                                st[name][:, a:b],
                                lhsT=ws[:, :, j * 512 + m * 128:
                                        j * 512 + (m + 1) * 128],
                                rhs=xs[:, :, j * T + t0 + a:j * T + t0 + b],
                                start=(ci == 0 and j == 0),
                                stop=(ci == 2 and j == 3),
                                perf_mode=DR,
                            )
                    return u

                def rope():
                    mul = mybir.AluOpType.mult
                    sub = mybir.AluOpType.subtract
                    add = mybir.AluOpType.add
                    pre, pim = st["psre"], st["psim"]
                    w = b - a
                    ccn = cc[:, t0 + a:t0 + b]
                    ssn = ss[:, t0 + a:t0 + b]
                    t1 = rtp.tile([128, CH], F16, tag="t1")
                    t2 = rtp.tile([128, CH], F16, tag="t2")
                    t3 = rtp.tile([128, CH], F16, tag="t3")
                    t4 = rtp.tile([128, CH], F16, tag="t4")
                    # psum -> fp16 bounce: ACT while it still has slack
                    # (early chunks), DVE once exp saturates ACT; the
                    # rope arithmetic runs in DVE 4x fp16 mode.
                    preb = rtp.tile([128, CH], F16, tag="preb")
                    pimb = rtp.tile([128, CH], F16, tag="pimb")
                    if act_bounce:
                        nc.scalar.copy(preb[:, 0:w], pre[:, a:b])
                        nc.scalar.copy(pimb[:, 0:w], pim[:, a:b])
                    else:
                        nc.vector.tensor_copy(preb[:, 0:w], pre[:, a:b])
                        nc.vector.tensor_copy(pimb[:, 0:w], pim[:, a:b])
                    preb_, pimb_ = preb[:, 0:w], pimb[:, 0:w]
                    nc.vector.tensor_tensor(t1[:, 0:w], preb_, ccn, mul)
                    nc.vector.tensor_tensor(t2[:, 0:w], pimb_, ssn, mul)
                    nc.vector.tensor_tensor(t3[:, 0:w], preb_, ssn, mul)
                    nc.vector.tensor_tensor(t4[:, 0:w], pimb_, ccn, mul)
                    # sub/add write the fp8 q8/k8 tiles directly (re
                    # half / im half): no relayout copies needed.  Pool
                    # (mostly idle) can take them to unload DVE.
                    eng = nc.gpsimd if SAPOOL else nc.vector
                    o = q8 if g == 0 else k8
                    eng.tensor_tensor(
                        o[:, t0 + a:t0 + b], t1[:, 0:w], t2[:, 0:w], sub)
                    eng.tensor_tensor(
                        o[:, T + t0 + a:T + t0 + b], t3[:, 0:w],
                        t4[:, 0:w], add)

                last = mm("psim", 2 * g + 1, 2)
                return [mm("psre", 2 * g, 0), mm("psre", 2 * g, 1),
                        mm("psre", 2 * g, 2), mm("psim", 2 * g + 1, 0),
                        mm("psim", 2 * g + 1, 1),
                        lambda: (last(), rope())]

            out = []
            for (a, b) in splits:
                out.extend(mk(a, b))
            return out

        def qk_g(n, g, act_bounce=False, splits=((0, CH),)):
            for u in qk_units(n, g, act_bounce, splits):
                u()

        def v_units(n, tb, act_bounce=False):
            """v projection for 128-row block tb of chunk n: 3 chain
            units of ~0.2us; the psum->fp16 bounce rides on the last."""
            t0 = n * CH + tb * 128
            st = {}

            def mm(ci):
                def u():
                    if ci == 0:
                        st["psv"] = psum.tile([128, CH], F32, tag="qk",
                                              name="psv")
                    xs, ws = CHAINS_V[ci]
                    for j in range(4):
                        nc.tensor.matmul(
                            st["psv"][:, 0:256],
                            lhsT=xs[:, :, j * T + t0:j * T + t0 + 128],
                            rhs=ws[:, :, j * 256:(j + 1) * 256],
                            start=(ci == 0 and j == 0),
                            stop=(ci == 2 and j == 3),
                            perf_mode=DR,
                        )
                    if ci == 2:
                        blk = 4 * n + tb
                        dst = vT_v[:, :, blk, 0:64]
                        src = st["psv"][:, 0:256].rearrange(
                            "p (h d) -> p h d", d=64)
                        # gpsimd cannot read PSUM; the bounce (psum holds
                        # 1024*v -- x/w ship pre-scaled x16/x64 to keep
                        # fp8 residuals out of the e4m3 subnormal range)
                        # folds in the 2^-10 fix free.  ACT takes it in
                        # the A/B phases where exp leaves it slack.
                        if act_bounce:
                            nc.scalar.activation(
                                dst, src,
                                mybir.ActivationFunctionType.Copy,
                                scale=1.0 / 1024.0)
                        else:
                            nc.vector.tensor_scalar_mul(
                                dst, src, 1.0 / 1024.0)
                return u

            return [mm(0), mm(1), mm(2)]

        def v_tb(n, tb):
            for u in v_units(n, tb):
                u()

        filler_q = []

        def dummy(n=2):
            # keep-warm matmuls: PE p-state drops 2x after an idle gap and
            # needs 3us of continuous execution to recover; padding known
            # exp-bound stretches keeps the real matmuls at full clock.
            pd = psum.tile([128, 512], F32, tag="qk", name="pd")
            for _ in range(n):
                nc.tensor.matmul(
                    pd[:, 0:512], lhsT=warm[:, 0:128], rhs=warm[:],
                    start=True, stop=True)

        def pump(keep_warm=0):
            if filler_q:
                filler_q.pop(0)()
            elif keep_warm:
                dummy(keep_warm)

        def attention_units(h, q0, qn, pump_every=0, at_blocks=None,
                            keep_warm=0, pair_full=False, norm_act=False):
            """One head, q-cols [q0, q0+qn) as a list of emission units.

            units[kb] = at_blocks/pump + scores(kb) + pv(kb-1); the last
            unit is pv(nkb-1) + final normalize.  The scheduler emits the
            NEXT head's units[0] just before this head's last unit: that
            head-boundary scores runs on PE while the last exp still
            streams on ACT, so ACT never drains between heads.  (The pst
            double-buffer is free at exactly that point: exp(nkb-2) has
            been consumed by the preceding pv.)

            at_blocks: {kb: [unit, ...]} -- mandatory work units emitted
            just before scores_block(kb); used for dependencies of later
            pv_blocks (e.g. v tiles), unlike best-effort pump fillers.
            """
            qv = q8v[32 * h:32 * h + 32]
            kv = k8v[32 * h:32 * h + 32]
            r0 = 64 * (h % 2)
            nkb = (q0 + qn) // 128
            fd = q0 // 128  # first diagonal block
            st_ = {}
            Ps = {}

            def block_off(kb):
                return 128 * (kb - fd) if kb >= fd else 0

            def scores_block(kb):
                if kb == 0:
                    st_["psy"] = psum.tile([128, qn], F32, tag="y",
                                           name="psy", bufs=1)
                off = block_off(kb)
                pst = psum.tile([128, qn], F32, tag="st", name="pst")
                for (a, b) in _splits(off, qn):
                    nc.tensor.matmul(
                        pst[:, a:b],
                        lhsT=kv[:, :, kb * 128:(kb + 1) * 128],
                        rhs=qv[:, :, q0 + a:q0 + b],
                        start=True,
                        stop=True,
                        perf_mode=DR,
                        tile_position=(32 * h, 0),
                    )
                P = pp.tile([128, 1024], F16, tag="P")
                Ps[kb] = (P, 0)
                nc.scalar.activation(
                    P[:, off:qn], pst[:, off:qn],
                    mybir.ActivationFunctionType.Exp, scale=SCALE)
                if kb >= fd:
                    # zero strictly-upper triangle of the leading 128 cols
                    nc.gpsimd.affine_select(
                        out=P[:, off:off + 128],
                        in_=P[:, off:off + 128],
                        compare_op=mybir.AluOpType.is_ge,
                        fill=0.0,
                        base=0,
                        pattern=[[1, 128]],
                        channel_multiplier=-1,
                    )

            def scores_pair(kb):
                # two qn=512 k-blocks share one [128,1024] pst / P pair:
                # a single exp call covers both, halving the ACT per-call
                # access-latency overhead and doubling the exp lookahead
                # the pst double-buffer can hold.  For diagonal blocks the
                # gap columns [512, 512+off1) are dead: matmul start
                # zeroes the bank region, exp of them is masked/unread
                # (pv reads only [off:512] of each half).
                if kb == 0:
                    st_["psy"] = psum.tile([128, qn], F32, tag="y",
                                           name="psy", bufs=1)
                pst = psum.tile([128, 1024], F32, tag="st", name="pst")
                for sub in (0, 1):
                    off = block_off(kb + sub)
                    nc.tensor.matmul(
                        pst[:, 512 * sub + off:512 * sub + 512],
                        lhsT=kv[:, :, (kb + sub) * 128:(kb + sub + 1) * 128],
                        rhs=qv[:, :, q0 + off:q0 + 512],
                        start=True,
                        stop=True,
                        perf_mode=DR,
                        tile_position=(32 * h, 0),
                    )
                off0 = block_off(kb)
                P = pp.tile([128, 1024], F16, tag="P")
                Ps[kb] = (P, 0)
                Ps[kb + 1] = (P, 512)
                nc.scalar.activation(
                    P[:, off0:1024], pst[:, off0:1024],
                    mybir.ActivationFunctionType.Exp, scale=SCALE)
                for sub in (0, 1):
                    off = block_off(kb + sub)
                    if kb + sub >= fd:
                        nc.gpsimd.affine_select(
                            out=P[:, 512 * sub + off:512 * sub + off + 128],
                            in_=P[:, 512 * sub + off:512 * sub + off + 128],
                            compare_op=mybir.AluOpType.is_ge,
                            fill=0.0,
                            base=0,
                            pattern=[[1, 128]],
                            channel_multiplier=-1,
                        )

            def pv_block(kb):
                off = block_off(kb)
                P, coff = Ps.pop(kb)
                psy = st_["psy"]
                for (a, b) in _splits(off, qn):
                    # last writer of the psum bank holding col a is diag
                    # block fd + 4*(a//512) + 3
                    kb_stop = min(fd + 4 * (a // 512) + 3, nkb - 1)
                    nc.tensor.matmul(
                        psy[:, a:b],
                        lhsT=vT_v[:, h, kb, :],
                        rhs=P[:, coff + a:coff + b],
                        start=(kb == 0),
                        stop=(kb == kb_stop),
                    )

            def normalize(a, b):
                # psum rows 64-127 all hold the denominator row l (ones
                # cols of vT): reciprocal + one multiply per psum bank,
                # emitted as soon as that bank's accumulation closes.
                # (walrus forbids two PSUM reads in one TensorTensor, so
                # a single divide is not possible.)
                psy = st_["psy"]
                rlb = nrm.tile([64, 512], F32, tag="rlb")
                nc.vector.reciprocal(rlb[:, 0:b - a], psy[64:128, a:b])
                nc.vector.tensor_tensor(
                    yT[h // 2][r0:r0 + 64, q0 + a:q0 + b],
                    psy[0:64, a:b], rlb[:, 0:b - a], mybir.AluOpType.mult)

            def mk_unit(kb, scores_fn, pvs):
                def u():
                    if at_blocks and kb in at_blocks:
                        for ab in at_blocks[kb]:
                            ab()
                    if pump_every and kb % pump_every == 0:
                        pump(keep_warm)
                    if scores_fn:
                        scores_fn(kb)
                    for pkb in pvs:
                        pv_block(pkb)
                        if pkb == min(fd + 3, nkb - 1) and qn > 512:
                            normalize(0, 512)  # bank 0 closed early
                return u

            units = []
            if PAIR and pair_full and qn == 512 and nkb % 2 == 0:
                # all blocks in exp-pairs; pv's trail one scores event
                pend = []
                for kb in range(0, nkb, 2):
                    units.append(mk_unit(kb, scores_pair, pend))
                    pend = [kb, kb + 1]
                st_["pend"] = pend
            else:
                for kb in range(nkb):
                    units.append(mk_unit(
                        kb, scores_block, [kb - 1] if kb > 0 else []))
                st_["pend"] = [nkb - 1]

            def final():
                for pkb in st_["pend"]:
                    pv_block(pkb)
                normalize(512 if qn > 512 else 0, qn)

            return units + [final]

        def run_heads(seq):
            """Emit head unit-lists with one-unit cross-head lookahead:
            the next head's scores(0) goes out before this head's final
            pv, so the exp stream never drains at a head boundary."""
            for i, units in enumerate(seq):
                for u in units[1 if i else 0:-1]:
                    u()
                if i + 1 < len(seq):
                    seq[i + 1][0]()
                units[-1]()

        def o_proj(nt, mo, tail=False):
            """Output block: feat rows [128*mo ..+128), q [512*nt ..+512)."""
            ob = obp.tile([128, 512], F16, tag="ob", name="ob")
            ps = psum.tile([128, CH], F32, tag="qk", name="psob")
            for kb in range(2):
                nc.tensor.matmul(
                    ps[:, 0:512],
                    lhsT=wo_sb[kb][:, mo * 128:(mo + 1) * 128],
                    rhs=yT[kb][:, nt * 512:(nt + 1) * 512],
                    start=(kb == 0),
                    stop=(kb == 1),
                )
            # in the tail ACT is idle once the exps are done: it takes half
            # the psum bounces there
            if (tail and mo % 2 == 1) or (OBA and mo % 2 == 1):
                nc.scalar.copy(ob[:], ps[:, 0:512])
            else:
                nc.vector.tensor_copy(ob[:], ps[:, 0:512])
            # keep DMA issue off the ACT queue while exps run; in the tail
            # ACT is free and a second queue doubles drain bandwidth
            ring = nc.scalar if (tail and mo % 2 == 1) else nc.sync
            ring.dma_start(
                out=outT_d[mo * 128:(mo + 1) * 128, nt * 512:(nt + 1) * 512],
                in_=ob[:])

        # ---- schedule ----
        # Emission order == per-engine queue order.  ACT (exp) is the
        # attention pacer now that scores+PV run fp8-DR/fp16 (PE 0.625
        # vs ACT 0.833 ns per score column), so all qkv/o_proj work is
        # pumped into the attention stream as ~0.4us filler units.
        # Dependency safety comes from emission order: a filler is
        # always emitted before the instruction that needs it.
        x_load(0)
        BR = int(os.environ.get("K_BRIDGE", "0"))
        if BR:
            # bridge the mid-chain DMA waits (dwqk8/dx8 arrival) with pad
            # matmuls on the still-live warmup psum so the PE p-state
            # ramp survives into the first real chains
            q0u = qk_units(0, 0, act_bounce=True)
            q0u[0]()
            for _ in range(BR):
                nc.tensor.matmul(pw[:, 0:512], lhsT=warm[:, 0:128],
                                 rhs=warm[:], start=True, stop=True)
            q0u[1]()
            for _ in range(BR // 2):
                nc.tensor.matmul(pw[:, 0:512], lhsT=warm[:, 0:128],
                                 rhs=warm[:], start=True, stop=True)
            for u in q0u[2:]:
                u()
        else:
            qk_g(0, 0, act_bounce=bool(int(os.environ.get("K_RB0Q", "0"))))
        qk_g(0, 1, act_bounce=bool(int(os.environ.get("K_RB0K", "1"))))
        k0rest = []
        nc.scalar.dma_start(out=wv8[:], in_=wv8_d[:])
        nc.scalar.dma_start(out=dwv8[:], in_=dwv8_d[:])
        x_load(1)
        # tile A (q 0-512): needs only chunk 0 q/k; v blocks land as
        # at_blocks just before their pv consumer.  The chunk-1 qk and v
        # work soaks up the rope-latency wait before the first scores.
        filler_q.extend(qk_units(1, 0, act_bounce=bool(
            int(os.environ.get("K_RB1Q", "0")))))
        filler_q.extend(qk_units(1, 1, act_bounce=bool(
            int(os.environ.get("K_RB1K", "1")))))
        for tb in range(4):
            filler_q.extend(v_units(1, tb, act_bounce=bool(VB01)))
        run_heads([
            attention_units(0, 0, 512, pump_every=PUMP_A, norm_act=True, at_blocks={
                1: k0rest + v_units(0, 0, act_bounce=bool(VB01)),
                2: v_units(0, 1, act_bounce=bool(VB01)),
                3: v_units(0, 2, act_bounce=bool(VB01))
                   + v_units(0, 3, act_bounce=bool(VB01)),
            }),
            attention_units(1, 0, 512, pump_every=PUMP_A, norm_act=True, pair_full=True),
            attention_units(2, 0, 512, pump_every=PUMP_A, norm_act=True, pair_full=True),
            attention_units(3, 0, 512, pump_every=PUMP_A, norm_act=True, pair_full=True),
        ])
        while filler_q:
            pump()
        # tile B (q 512-1024): fillers: chunks 2,3 qk.
        x_load(2)
        x_load(3)
        nc.scalar.dma_start(out=cc[:, 1024:2048], in_=ccT_d[:, 1024:2048])
        nc.scalar.dma_start(out=ss[:, 1024:2048], in_=ssT_d[:, 1024:2048])
        filler_q.extend(qk_units(2, 0, act_bounce=bool(
            int(os.environ.get("K_RB2Q", "0")))))
        filler_q.extend(qk_units(2, 1, act_bounce=bool(
            int(os.environ.get("K_RB2K", "0")))))
        filler_q.extend(qk_units(3, 0, act_bounce=bool(
            int(os.environ.get("K_RB3Q", "0")))))
        filler_q.extend(qk_units(3, 1, act_bounce=bool(
            int(os.environ.get("K_RB3K", "1")))))
        run_heads([
            attention_units(0, 512, 512, pump_every=PUMP_B, pair_full=True, norm_act=True),
            attention_units(1, 512, 512, pump_every=PUMP_B, pair_full=True, norm_act=True),
            attention_units(2, 512, 512, pump_every=PUMP_B, pair_full=True, norm_act=True),
            attention_units(3, 512, 512, pump_every=PUMP_B, pair_full=True, norm_act=True),
        ])
        while filler_q:
            pump()
        # tail o_proj helpers (q 1024-2048): per mo one [128, 1024] ob
        # filled in two halves; nt=2 halves are emitted inside the last
        # head's attention as soon as its early psum bank is normalized.
        tail_obs = {}

        # single [128, 8x1024] tile for the q 1024-2048 output: per mo,
        # cols [1024mo, +512) = nt2 half, [+512, +1024) = nt3 half.
        # Keeping it one tile lets the tail flush as FOUR two-mo strided
        # DMAs instead of 16 narrow ones: each DMA costs ~625ns on the
        # shared HWDGE regardless of size, and that serialization was
        # the dominant term of the post-attention tail.
        obt_all = const.tile([128, 8 * 1024], F16, tag="obt", name="obt")

        def tail_half(nt, mo):
            # nt3 runs after the last attention: the st/y psum banks are
            # free, so cycling tags gives 5 buffers in rotation instead
            # of 2 and breaks the psum-recycle serialization of the tail.
            if nt == 3:
                tag, bufs = [("qk", 2), ("st", 2), ("y", 1)][mo % 3]
                ps = psum.tile([128, CH], F32, tag=tag, name="psob",
                               bufs=bufs)
            else:
                ps = psum.tile([128, CH], F32, tag="qk", name="psob")
            for kb in range(2):
                nc.tensor.matmul(
                    ps[:, 0:512],
                    lhsT=wo_sb[kb][:, mo * 128:(mo + 1) * 128],
                    rhs=yT[kb][:, nt * 512:(nt + 1) * 512],
                    start=(kb == 0),
                    stop=(kb == 1),
                )
            c0 = 1024 * mo + 512 * (nt - 2)
            if (nt == 3 and mo % 2 == 1) or (nt == 2 and mo >= 5):
                # true tail: exps are done, ACT is free
                nc.scalar.copy(obt_all[:, c0:c0 + 512], ps[:, 0:512])
            else:
                nc.vector.tensor_copy(obt_all[:, c0:c0 + 512],
                                      ps[:, 0:512])
            if nt == 2 and mo < 5:
                # nt2 halves flush per-mo while attention still runs
                nc.sync.dma_start(
                    out=outT_d[mo * 128:(mo + 1) * 128, 1024:1536],
                    in_=obt_all[:, c0:c0 + 512])
            elif nt == 2 and mo == 7:
                # the three post-attention nt2 halves go as ONE strided
                # DMA: each DMA costs ~625ns on the shared HWDGE, which
                # is the serializer of the final flush
                dst = outT_d.rearrange("(m p) t -> p m t", p=128)
                src_ = obt_all[:].rearrange("p (m t) -> p m t", t=1024)
                nc.sync.dma_start(out=dst[:, 5:8, 1024:1536],
                                  in_=src_[:, 5:8, 0:512])
            elif mo % 2 == 1:
                # nt3: one strided DMA flushes the (mo-1, mo) pair
                ring = nc.scalar if mo % 4 == 1 else nc.sync
                dst = outT_d.rearrange("(m p) t -> p m t", p=128)
                src = obt_all[:].rearrange("p (m t) -> p m t", t=1024)
                ring.dma_start(
                    out=dst[:, mo - 1:mo + 1, 1536:2048],
                    in_=src[:, mo - 1:mo + 1, 512:1024])

        # tile C (q 1024-2048): v chunks 2/3 are emitted at fixed blocks of
        # the first head (hard deps of pv blocks 8-15); o_proj of q 0-1024
        # is order-free filler spread across all four heads.
        filler_q.extend(
            (lambda nt=nt, mo=mo: o_proj(nt, mo))
            for nt in range(2) for mo in range(8))
        run_heads([
            attention_units(0, 1024, 1024, pump_every=PUMP_C0, at_blocks=(
                {
                    5: v_units(2, 0, VB2) + v_units(2, 1, VB2),
                    7: v_units(2, 2, VB2) + v_units(2, 3, VB2),
                    9: v_units(3, 0, VB3) + v_units(3, 1, VB3),
                    11: v_units(3, 2, VB3) + v_units(3, 3, VB3),
                })),
            attention_units(1, 1024, 1024, pump_every=PUMP_C,
                            keep_warm=KEEP_WARM),
            attention_units(2, 1024, 1024, pump_every=PUMP_C,
                            keep_warm=KEEP_WARM),
            attention_units(3, 1024, 1024, pump_every=PUMP_C,
                            keep_warm=KW3, at_blocks={
                13: [lambda mo=mo: tail_half(2, mo) for mo in range(2)],
                14: [lambda mo=mo: tail_half(2, mo) for mo in range(2, 5)],
            }),
        ])
        while filler_q:
            pump()
        # tail: keep-warm over the final-normalize wait so the o_proj
        # matmuls run at full p-state; the three remaining nt2 units are
        # emitted here (not as kb15 at_blocks) so their DVE copies queue
        # BEHIND head 3's final normalize instead of delaying it.
        if int(os.environ.get('K_TDUM', '3')):
            dummy(int(os.environ.get('K_TDUM', '3')))
        tail_half(3, 0)
        tail_half(2, 5)
        tail_half(3, 1)
        tail_half(2, 6)
        tail_half(3, 2)
        tail_half(2, 7)
        for mo in range(3, 8):
            tail_half(3, mo)

    nc.compile()
    return nc


def _dr_pack(a, scale):
    """[1024, M] f32 -> fp8 (hi, residual) pair in DR layout [128, 8M].

    DR layout "p (i j m)": element (p, i, j, m) = a[256j + 128i + p, m], so
    one DoubleRow matmul contracts dim pairs (256j+p, 256j+128+p).
    `scale` lifts the values so both them and their residuals quantize in
    the e4m3 normal range; the device compensates (rope tables / v copy).
    """
    import ml_dtypes
    f8 = ml_dtypes.float8_e4m3
    a = a * scale
    M = a.shape[1]
    hi = a.astype(f8)
    lo = (a - hi.astype(np.float32)).astype(f8)
    out = []
    for t in (hi, lo):
        t = t.reshape(4, 2, 128, M).transpose(2, 1, 0, 3).reshape(128, 8 * M)
        out.append(np.ascontiguousarray(t))
    return out


def shard_inputs(x, freqs_cos, freqs_sin, Wqkv, Wo):
    """Build the 8 per-core input maps (host-side sharding)."""
    x = np.asarray(x, dtype=np.float32)
    Wqkv = np.asarray(Wqkv, dtype=np.float32)
    Wo = np.asarray(Wo, dtype=np.float32)
    # cos/sin tables transposed and replicated x4 (one copy per local
    # head), pre-divided by 1024 to undo the x16/x64 fp8 shipping scales
    ccT = np.tile(np.asarray(freqs_cos, dtype=np.float32).T, (4, 1)) / 1024.0
    ssT = np.tile(np.asarray(freqs_sin, dtype=np.float32).T, (4, 1)) / 1024.0
    ccT = np.ascontiguousarray(ccT).astype(np.float16)
    ssT = np.ascontiguousarray(ssT).astype(np.float16)
    x8s = [_dr_pack(x[b].T, 16.0) for b in range(B)]

    in_maps = []
    for c in range(NCORE):
        b, hg = c // 4, c % 4
        re = [np.arange(g * 64, g * 64 + 64, 2)
              for g in range(4 * hg, 4 * hg + 4)]
        im = [np.arange(g * 64 + 1, g * 64 + 64, 2)
              for g in range(4 * hg, 4 * hg + 4)]
        qcols = np.concatenate(re + im)
        kcols = C + qcols
        wqk8, dwqk8 = _dr_pack(Wqkv[:, np.concatenate([qcols, kcols])], 64.0)
        wv8, dwv8 = _dr_pack(
            Wqkv[:, 2 * C + hg * 256: 2 * C + hg * 256 + 256], 64.0)
        wo = np.ascontiguousarray(
            Wo[hg * 256: hg * 256 + 256, :]).astype(np.float16)
        in_maps.append({
            "x8": x8s[b][0], "dx8": x8s[b][1],
            "wqk8": wqk8, "dwqk8": dwqk8, "wv8": wv8, "dwv8": dwv8,
            "wo": wo, "ccT": ccT, "ssT": ssT,
        })
    return in_maps


_NC_CACHE = None


def _get_nc():
    global _NC_CACHE
    if _NC_CACHE is None:
        _NC_CACHE = build_nc()
    return _NC_CACHE


def run(inputs, trace=False):
    from concourse.bass_utils import run_bass_kernel_spmd

    nc = _get_nc()
    in_maps = shard_inputs(**inputs)
    res = run_bass_kernel_spmd(nc, in_maps, list(range(NCORE)), trace=trace)
    out = np.empty((B, T, C), dtype=np.float32)
    for b in range(B):
        acc = res.results[4 * b]["outT"].astype(np.float32)
        for c in range(4 * b + 1, 4 * b + 4):
            acc = acc + res.results[c]["outT"].astype(np.float32)
        out[b] = acc.T
    return out, res


def kernel(**inputs):
    out, _ = run(inputs)
    return out

